# revision 1
# baseline (speedup 1.0000x reference)
"""Trainium2 Bass kernel for DiscreteBundleSheafDiffusion (D=2, FD=3, HID=32).

Redesign vs baseline: all per-edge gathers go through batched dma_gather
(1024 indices per call, int16 wrapped+replicated index tables) instead of
canonical [128,1] indirect DMAs; one-hot row-selection matrices for the
segment-sum matmuls are generated on-chip per chunk via tensor_scalar
is_equal (4x DVE mode) instead of streamed from HBM; the gathered feature
table holds dinv[v] * (Wl (x) I) xc[v] in bf16 (so per-edge work is only a
2D rotation + w2 scale, and the dinv AllGather disappears); aggregation
matmuls run in bf16 with f32 PSUM accumulate.

Tables: contribs (4 sheaf/weight projections per node) live 2-node-packed
in XC2 [NPAD/2, 64] f32 (256B rows, parity-selected after gather, index =
node>>1 fits int16); features live in XF [NPAD, 128] bf16 (256B rows) with
chunks class-sorted by col < 32768 (LO) vs >= 32768 (HI) so gathers address
XF[0:HALF] / XF[HALF:] with int16 indices. Per-window chunk capacities
QL/QH are data-derived maxima, uniform across cores (single SPMD program).
"""
import sys
sys.path.insert(0, '/opt/trn_rl_repo')
import numpy as np

N_NODES = 50000
E0 = 200000
IN_CH = 128
OUT_CH = 32
N_LAYERS = 2
FD, HID = 3, 32
F = FD * HID
NCORES = 8
SHR = 6250
SH = 6272
NW = SH // 128
NPAD = NCORES * SH
SPLA = 3200          # class-A rows per core slice (25 windows)
CALL = 1024           # indices per dma_gather call
CCH = CALL // 128     # chunks per feature/contrib call (8)

_CACHE = {}


def _spectral_normalize_np(W, iters=20):
    W = np.asarray(W, np.float32)
    u = np.full((W.shape[0],), 1.0 / np.sqrt(W.shape[0]), np.float32)
    for _ in range(iters):
        v = W.T @ u
        v = v / (np.linalg.norm(v) + np.float32(1e-12))
        u2 = W @ v
        u = u2 / (np.linalg.norm(u2) + np.float32(1e-12))
    v = W.T @ u
    v = v / (np.linalg.norm(v) + np.float32(1e-12))
    sigma = u @ W @ v
    return W / sigma


def _wrap_calls(seq2d):
    """seq2d: [ncalls, 1024] int -> [128, ncalls*64] int16 wrapped+replicated."""
    ncalls = seq2d.shape[0]
    out = np.zeros((128, ncalls * 64), np.int16)
    for j in range(ncalls):
        w = seq2d[j].reshape(64, 16).T.astype(np.int16)   # [16, 64]
        out[:, j * 64:(j + 1) * 64] = np.tile(w, (8, 1))
    return out


def _plan_chunks(edge_index):
    """Class-sorted chunk packing; returns plan dict + per-core edge arrays."""
    ei = np.asarray(edge_index)
    row = ei[0].astype(np.int64)
    col = ei[1].astype(np.int64)
    n_ids = np.arange(N_NODES)
    pad_id = (n_ids // SHR) * SH + (n_ids % SHR)
    rowp = pad_id[row]
    colp = pad_id[col]

    cores = []
    ql = qh = 0
    for c in range(NCORES):
        m = (rowp // SH) == c
        r = (rowp[m] - c * SH).astype(np.int64)
        cl = colp[m].astype(np.int64)
        hi = ((cl % SH) >= SPLA).astype(np.int64)
        order = np.lexsort((cl, r, hi, r // 128))
        r, cl, hi = r[order], cl[order], hi[order]
        w = r // 128
        for ww in range(NW):
            nlo = int(((w == ww) & (hi == 0)).sum())
            nhi = int(((w == ww) & (hi == 1)).sum())
            ql = max(ql, (nlo + 127) // 128)
            qh = max(qh, (nhi + 127) // 128)
        cores.append((r, cl, hi))

    nlo_p = -(-NW * ql // CCH) * CCH
    nhi_p = -(-NW * qh // CCH) * CCH
    nca = nlo_p + nhi_p
    plan = dict(QL=ql, QH=qh, NLOP=nlo_p, NHIP=nhi_p, NCA=nca)

    # chunk k -> (region, window, start, stop) in k order
    winchunks = []
    for k in range(nca):
        if k < nlo_p:
            reg, q, base = 0, ql, 0
            w = min(k // ql, NW - 1)
            k0 = w * ql
            k1 = nlo_p if w == NW - 1 else (w + 1) * ql
        else:
            reg, q, base = 1, qh, nlo_p
            kk = k - nlo_p
            w = min(kk // qh, NW - 1)
            k0 = base + w * qh
            k1 = nca if w == NW - 1 else base + (w + 1) * qh
        winchunks.append((k, reg, w, k == k0, k == k1 - 1))
    plan['winchunks'] = winchunks
    return plan, cores, pad_id


def _host_prep(x, edge_index, W1, b1, W2, b2, W_left, W_right, eps,
               W_sheaf, W_wt):
    plan, cores, pad_id = _plan_chunks(edge_index)
    _CACHE['plan'] = plan
    ql, qh = plan['QL'], plan['QH']
    nlo_p, nca = plan['NLOP'], plan['NCA']
    x = np.asarray(x, np.float32)

    in_maps = []
    for c in range(NCORES):
        r, cl, hi = cores[c]
        colp_arr = np.zeros((nca, 128), np.int64)      # global padded col id
        rloc = np.full((nca, 128), -1, np.int64)
        rglob = np.zeros((nca, 128), np.int64)         # global padded row id
        valid = np.zeros((nca, 128), bool)
        w = r // 128
        for ww in range(NW):
            for reg in range(2):
                msel = (w == ww) & (hi == reg)
                rw, cw = r[msel], cl[msel]
                cnt = rw.shape[0]
                cap = (ql if reg == 0 else qh) * 128
                assert cnt <= cap, f"window overflow {cnt} > {cap}"
                base = ww * ql if reg == 0 else nlo_p + ww * qh
                for qq in range((cnt + 127) // 128):
                    a, b = qq * 128, min(qq * 128 + 128, cnt)
                    k = base + qq
                    colp_arr[k, :b - a] = cw[a:b]
                    rloc[k, :b - a] = rw[a:b] - ww * 128
                    rglob[k, :b - a] = c * SH + rw[a:b]
                    valid[k, :b - a] = True

        # gather index sequences per call of CCH chunks
        ncalls = nca // CCH
        iC = (colp_arr >> 1).reshape(ncalls, CALL)
        iR = np.where(valid, rglob - c * SH, 0).reshape(ncalls, CALL)
        core_of = colp_arr // SH
        rloc_of = colp_arr % SH
        iF = np.where(rloc_of < SPLA,
                      core_of * SPLA + rloc_of,
                      core_of * (SH - SPLA) + rloc_of - SPLA)
        iF[~valid] = 0
        iF = iF.reshape(ncalls, CALL)
        idxc_w = _wrap_calls(iC)
        idxr_w = _wrap_calls(iR)
        idxf_w = _wrap_calls(iF)
        mcolT = (colp_arr & 1).T.astype(np.float32).copy()   # [128, NCA]
        mrowT = (rglob & 1).T.astype(np.float32).copy()
        rloc_b = rloc.T.astype(np.float32).copy()             # [128, NCA]
        in_maps.append({
            "idxc_w": idxc_w, "idxr_w": idxr_w, "idxf_w": idxf_w,
            "mcolT": mcolT, "mrowT": mrowT, "rloc_b": rloc_b,
        })

    import ml_dtypes
    W1 = np.asarray(W1, np.float32); b1 = np.asarray(b1, np.float32)
    W2 = np.asarray(W2, np.float32); b2 = np.asarray(b2, np.float32)
    NL = N_LAYERS
    w4 = np.zeros((F, NL * 4), np.float32)
    wmt = np.zeros((F, NL * F), np.float32)
    wrkt = np.zeros((F, NL * F), np.float32)
    wlit = np.zeros((F, NL * F), np.float32)
    cfb = np.zeros((128, NL * FD), np.float32)
    for l in range(NL):
        sh_row = np.asarray(W_sheaf[l][1], np.float32)
        wt_row = np.asarray(W_wt[l][0], np.float32)
        w4[:, l * 4 + 0] = sh_row[:F]
        w4[:, l * 4 + 1] = sh_row[F:]
        w4[:, l * 4 + 2] = wt_row[:F]
        w4[:, l * 4 + 3] = wt_row[F:]
        Wl = _spectral_normalize_np(np.asarray(W_left[l], np.float32))
        Wr = _spectral_normalize_np(np.asarray(W_right[l], np.float32))
        wmt[:, l * F:(l + 1) * F] = np.kron(Wl, Wr).astype(np.float32).T
        wrkt[:, l * F:(l + 1) * F] = \
            np.kron(np.eye(FD, dtype=np.float32), Wr).astype(np.float32).T
        wlit[:, l * F:(l + 1) * F] = \
            np.kron(Wl, np.eye(HID, dtype=np.float32)).astype(np.float32).T
        cfb[:, l * FD:(l + 1) * FD] = \
            (1.0 + np.tanh(np.asarray(eps[l], np.float32))).reshape(1, FD)

    xp = np.zeros((NPAD, IN_CH), np.float32)
    xp[pad_id] = x
    iota_b = np.tile(np.arange(128, dtype=np.float32)[None, :],
                     (128, 1)).astype(ml_dtypes.bfloat16)
    shared = {
        "w1t": W1.T.copy(), "b1f": b1.reshape(F, 1).copy(),
        "w2t": W2.T.copy(), "b2": b2.reshape(OUT_CH, 1).copy(),
        "w4": w4, "wmt": wmt, "wrkt": wrkt, "wlit": wlit, "cfb": cfb,
        "iota_b": iota_b, "ident": np.eye(128, dtype=np.float32),
    }
    for c in range(NCORES):
        in_maps[c]["x_sh"] = xp[c * SH:(c + 1) * SH].T.copy()
        in_maps[c].update(shared)
    return in_maps, pad_id


# =================== bass program ===================
def _build_program(plan):
    import concourse.bacc as bacc
    import concourse.bass as bass
    import concourse.mybir as mybir
    from concourse import tile

    ql, qh = plan['QL'], plan['QH']
    nlo_p, nca = plan['NLOP'], plan['NCA']
    winchunks = plan['winchunks']
    ncalls = nca // CCH
    NL = N_LAYERS
    f32 = mybir.dt.float32
    bf16 = mybir.dt.bfloat16
    i16 = mybir.dt.int16
    AF = mybir.ActivationFunctionType
    ALU = mybir.AluOpType

    nc = bacc.Bacc("TRN2", target_bir_lowering=False, debug=False,
                   num_swdge_queues=2)

    def raw_gather(gps, out_ap, in_ap, idxs_ap, num_idxs, elem_size,
                   elem_step, queue_num=0):
        """dma_gather allowing elem_size < 256B (row stride must be %256B)."""
        stride_bytes = elem_step * mybir.dt.size(in_ap.dtype)
        assert stride_bytes % 256 == 0
        assert in_ap.ap[0][0] == elem_step
        assert in_ap.ap[-1][1] == out_ap.ap[-1][1] == elem_size
        _in_ap = gps.lower_ap_dma(in_ap, for_custom_bir_dma=True)
        _idxs_ap = gps.lower_ap(idxs_ap)
        _out_ap = gps.lower_ap(out_ap)
        return gps.add_instruction(
            mybir.InstDMAGatherAnt(
                name=gps.bass.get_next_instruction_name(),
                ins=[*_in_ap, _idxs_ap,
                     gps.lower_val_access(gps.to_reg(num_idxs))],
                outs=[_out_ap],
                transpose=False,
                num_idxs=num_idxs,
                elem_size=elem_size,
                stride_bytes_256=stride_bytes // 256,
                gen_mode=0,
                single_packet=True,
                queue_num=queue_num,
                sbuf_tokens_per_rank=0,
                sbuf_free_dim_per_rank=0,
                sbuf_free_dim_pad_per_rank=0,
                sbuf_byte_offset=0,
            ))


    x_sh = nc.dram_tensor("x_sh", [IN_CH, SH], f32, kind="ExternalInput").ap()
    idxc_d = nc.dram_tensor("idxc_w", [128, ncalls * 64], i16, kind="ExternalInput").ap()
    idxr_d = nc.dram_tensor("idxr_w", [128, ncalls * 64], i16, kind="ExternalInput").ap()
    idxf_d = nc.dram_tensor("idxf_w", [128, ncalls * 64], i16, kind="ExternalInput").ap()
    mcol_d = nc.dram_tensor("mcolT", [128, nca], f32, kind="ExternalInput").ap()
    rloc_d = nc.dram_tensor("rloc_b", [128, nca], f32, kind="ExternalInput").ap()
    iota_d = nc.dram_tensor("iota_b", [128, 128], bf16, kind="ExternalInput").ap()
    w1t_d = nc.dram_tensor("w1t", [IN_CH, F], f32, kind="ExternalInput").ap()
    b1f_d = nc.dram_tensor("b1f", [F, 1], f32, kind="ExternalInput").ap()
    w2t_d = nc.dram_tensor("w2t", [F, OUT_CH], f32, kind="ExternalInput").ap()
    b2_d = nc.dram_tensor("b2", [OUT_CH, 1], f32, kind="ExternalInput").ap()
    w4_d = nc.dram_tensor("w4", [F, NL * 4], f32, kind="ExternalInput").ap()
    wmt_d = nc.dram_tensor("wmt", [F, NL * F], f32, kind="ExternalInput").ap()
    wrkt_d = nc.dram_tensor("wrkt", [F, NL * F], f32, kind="ExternalInput").ap()
    wlit_d = nc.dram_tensor("wlit", [F, NL * F], f32, kind="ExternalInput").ap()
    cfb_d = nc.dram_tensor("cfb", [128, NL * FD], f32, kind="ExternalInput").ap()
    ident_d = nc.dram_tensor("ident", [128, 128], f32, kind="ExternalInput").ap()
    out_d = nc.dram_tensor("out", [SH, OUT_CH], f32, kind="ExternalOutput").ap()

    xcs = nc.dram_tensor("xcs", [SH, 4], f32)
    XCFULL = nc.dram_tensor("XCFULL", [NPAD, 4], f32, addr_space="Shared")
    XC2 = nc.dram_tensor("XC2", [NPAD // 2, 64], f32)
    XR2 = nc.dram_tensor("XR2", [SH, 64], f32)
    xfs = nc.dram_tensor("xfs", [SH, 128], bf16)
    XFA = nc.dram_tensor("XFA", [NCORES * SPLA, 128], bf16, addr_space="Shared")
    XFB = nc.dram_tensor("XFB", [NCORES * (SH - SPLA), 128], bf16, addr_space="Shared")
    RG = [list(range(NCORES))]

    with tile.TileContext(nc) as tc:
        with tc.tile_pool(name="const", bufs=1) as constp, \
             tc.tile_pool(name="big", bufs=1) as bigp, \
             tc.tile_pool(name="wide", bufs=1) as widep, \
             tc.tile_pool(name="gath", bufs=3) as gathp, \
             tc.tile_pool(name="fea", bufs=4) as feap, \
             tc.tile_pool(name="selp", bufs=6) as selp, \
             tc.tile_pool(name="work", bufs=3) as workp, \
             tc.tile_pool(name="msgp", bufs=4) as msgp, \
             tc.tile_pool(name="ps", bufs=2, space="PSUM") as psp, \
             tc.tile_pool(name="ps2", bufs=2, space="PSUM") as ps2p, \
             tc.tile_pool(name="ps1", bufs=4, space="PSUM") as ps1p:

            def C(name, shape, src, dt=f32):
                t = constp.tile(shape, dt, tag=name, name=name)
                nc.sync.dma_start(t[:], src)
                return t

            ident = C("ident", [128, 128], ident_d[:])
            iota_b = C("iota", [128, 128], iota_d[:], dt=bf16)
            w1t = C("w1t", [IN_CH, F], w1t_d[:])
            b1f = C("b1f", [F, 1], b1f_d[:])
            w2t = C("w2t", [F, OUT_CH], w2t_d[:])
            b2sb = C("b2", [OUT_CH, 1], b2_d[:])
            w4sb = C("w4", [F, NL * 4], w4_d[:])
            wmt = C("wmt", [F, NL * F], wmt_d[:])
            wrkt = C("wrkt", [F, NL * F], wrkt_d[:])
            wlit = C("wlit", [F, NL * F], wlit_d[:])
            cfb = C("cfb", [128, NL * FD], cfb_d[:])
            mcol = C("mcol", [128, nca], mcol_d[:])
            rloc_b = C("rloc", [128, nca], rloc_d[:])
            idxc_sb = C("idxc", [128, ncalls * 64], idxc_d[:], dt=i16)
            idxr_sb = C("idxr", [128, ncalls * 64], idxr_d[:], dt=i16)
            idxf_sb = C("idxf", [128, ncalls * 64], idxf_d[:], dt=i16)

            slabT = bigp.tile([128, NW, F], f32, tag="slabT")
            ctile = bigp.tile([128, NW, 4], f32, tag="ctile")
            aggsh = bigp.tile([128, NW, F], f32, tag="aggsh")
            xfN = bigp.tile([128, NW, F], bf16, tag="xfN")
            ywl = bigp.tile([128, NW, F], bf16, tag="ywl")
            z1a = bigp.tile([128, NW, F], f32, tag="z1a")
            dinv_sh = bigp.tile([128, NW], f32, tag="dinvsh")
            diag_sh = bigp.tile([128, NW], f32, tag="diagsh")
            ccall = bigp.tile([128, nca, 4], f32, tag="ccall")
            rcall = bigp.tile([128, nca, 4], f32, tag="rcall")
            c2b = bigp.tile([128, nca], bf16, tag="c2b")
            s2b = bigp.tile([128, nca], bf16, tag="s2b")
            w2b = bigp.tile([128, nca], bf16, tag="w2b")

            def tpose(src_ap, pdim, fdim, tag="tx"):
                pt = ps1p.tile([128, 128], f32, tag="tp", name="tp")
                nc.tensor.transpose(pt[:fdim, :pdim], src_ap,
                                    ident[:pdim, :pdim])
                dst = workp.tile([128, 128], f32, tag=tag, name=tag)
                nc.scalar.copy(dst[:fdim, :pdim], pt[:fdim, :pdim])
                return dst

            def contrib_write(L):
                """ctile -> xcs -> AllGather -> XCFULL -> expand into XC2.
                Also writes the local 2-node-packed row table XR2 (pre-AG so
                row-side gathers can overlap the collective)."""
                nc.sync.dma_start(
                    XR2[:, 0:4].rearrange("(w p) f -> p w f", p=128),
                    ctile[:, :, :])
                nc.sync.dma_start(
                    xcs[:].rearrange("(w p) f -> p w f", p=128), ctile[:, :, :])
                nc.gpsimd.collective_compute(
                    "AllGather", ALU.bypass, replica_groups=RG,
                    ins=[xcs[:]], outs=[XCFULL[:]])
                ct2 = widep.tile([128, NPAD // 256, 8], f32, tag="ct2",
                                 name="ct2")
                nc.sync.dma_start(
                    ct2[:, :, :],
                    XCFULL[:].rearrange("(c p two) f -> p c (two f)",
                                        p=128, two=2))
                nc.sync.dma_start(
                    XC2[:, 0:8].rearrange("(c p) f8 -> p c f8", p=128),
                    ct2[:, :, :])

            # ---------------- lin1 + layer-0 contribs ----------------
            for t in range(NW):
                xT = workp.tile([128, 128], f32, tag="xt")
                nc.sync.dma_start(xT[:], x_sh[:, t * 128:(t + 1) * 128])
                hp = psp.tile([128, 128], f32, tag="mm")
                nc.tensor.matmul(hp[:F, :128], w1t[:], xT[:IN_CH, :128],
                                 start=True, stop=True)
                e1 = workp.tile([F, 128], f32, tag="e1")
                nc.scalar.activation(e1[:, :], hp[:F, :128], AF.Exp,
                                     bias=b1f[:, :])
                nc.vector.tensor_scalar(e1[:, :], e1[:, :], 1.0, -1.0,
                                        ALU.min, ALU.add)
                r1 = workp.tile([F, 128], f32, tag="r1")
                nc.scalar.activation(r1[:, :], hp[:F, :128], AF.Relu,
                                     bias=b1f[:, :])
                hF = workp.tile([F, 128], f32, tag="hF")
                nc.vector.tensor_add(hF[:, :], e1[:, :], r1[:, :])
                cp4 = ps1p.tile([128, 128], f32, tag="tp")
                nc.tensor.matmul(cp4[:4, :128], w4sb[:, 0:4], hF[:, :128],
                                 start=True, stop=True)
                c4s = workp.tile([4, 128], f32, tag="c4s")
                nc.scalar.copy(c4s[:, :], cp4[:4, :128])
                hN = ps1p.tile([128, 128], f32, tag="tp")
                nc.tensor.transpose(hN[:128, :F], hF[:, :128], ident[:F, :F])
                nc.vector.tensor_copy(slabT[:, t, 0:F], hN[:128, :F])
                cN = ps1p.tile([128, 128], f32, tag="tp")
                nc.tensor.transpose(cN[:128, :4], c4s[:, :128], ident[:4, :4])
                nc.vector.tensor_copy(ctile[:, t, :], cN[:128, :4])
                ywp = psp.tile([128, 128], f32, tag="mm")
                nc.tensor.matmul(ywp[:F, :128], wlit[:, 0:F], hF[:, :128],
                                 start=True, stop=True)
                ywT = workp.tile([F, 128], f32, tag="tsum")
                nc.vector.tensor_copy(ywT[:, :], ywp[:F, :128])
                ywN = ps1p.tile([128, 128], f32, tag="tp")
                nc.tensor.transpose(ywN[:128, :F], ywT[:, :128], ident[:F, :F])
                nc.vector.tensor_copy(ywl[:, t, 0:F], ywN[:128, :F])
            contrib_write(0)

            # =================== layers ===================
            for L in range(NL):
                # ---- phase 1: contrib gathers ----
                # rows first: depend only on local XR2, so they run under the
                # XC AllGather + expand; cols wait on XC2.
                GB = 4  # calls per select batch (32 chunks)

                def psel(gt, mt, dst, ks, nk):
                    d = msgp.tile([128, GB * CCH, 4], f32, tag="d4", name="d4")
                    nc.vector.tensor_sub(d[:, :nk, :], gt[:, :nk, 4:8],
                                         gt[:, :nk, 0:4])
                    nc.vector.tensor_mul(
                        d[:, :nk, :], d[:, :nk, :],
                        mt[:, ks].unsqueeze(2).broadcast_to([128, nk, 4]))
                    nc.vector.tensor_add(dst[:, ks, :], gt[:, :nk, 0:4],
                                         d[:, :nk, :])

                for g in range(ncalls):
                    sl = slice(g * 64, (g + 1) * 64)
                    raw_gather(
                        nc.gpsimd, rcall[:, g * CCH:(g + 1) * CCH, :],
                        XR2[:, 0:4], idxr_sb[:, sl], CALL, 4, 64,
                        queue_num=g % 2)
                for g0 in range(0, ncalls, GB):
                    gb = min(GB, ncalls - g0)
                    gt = gathp.tile([128, GB * CCH, 8], f32, tag="gc",
                                    name="gc")
                    for j in range(gb):
                        g = g0 + j
                        sl = slice(g * 64, (g + 1) * 64)
                        raw_gather(
                            nc.gpsimd, gt[:, j * CCH:(j + 1) * CCH, :],
                            XC2[:, 0:8], idxc_sb[:, sl], CALL, 8, 64,
                            queue_num=g % 2)
                    psel(gt, mcol, ccall,
                         slice(g0 * CCH, (g0 + gb) * CCH), gb * CCH)

                # ---- learner algebra (f32, [128, NCA]) ----
                def wt(tag):
                    return widep.tile([128, nca], f32, tag=tag, name=tag)
                rc, cc = rcall, ccall
                ta, tb = wt("ta"), wt("tb")
                nc.vector.tensor_add(ta[:, :], rc[:, :, 0], cc[:, :, 1])
                nc.vector.tensor_add(tb[:, :], cc[:, :, 0], rc[:, :, 1])
                af, ab = wt("af"), wt("ab")
                nc.scalar.activation(af[:, :], ta[:, :], AF.Tanh)
                nc.scalar.activation(ab[:, :], tb[:, :], AF.Tanh)
                nc.vector.tensor_add(ta[:, :], rc[:, :, 2], cc[:, :, 3])
                nc.vector.tensor_add(tb[:, :], cc[:, :, 2], rc[:, :, 3])
                u1, u2 = wt("u1"), wt("u2")
                nc.scalar.activation(u1[:, :], ta[:, :], AF.Tanh, scale=0.5)
                nc.scalar.activation(u2[:, :], tb[:, :], AF.Tanh, scale=0.5)
                w2e, t1, t2 = wt("w2e"), wt("t1"), wt("t2")
                nc.vector.tensor_mul(t1[:, :], u1[:, :], u2[:, :])
                nc.vector.tensor_add(t2[:, :], u1[:, :], u2[:, :])
                nc.vector.tensor_add(t1[:, :], t1[:, :], t2[:, :])
                nc.vector.tensor_scalar(w2e[:, :], t1[:, :], 0.25, 0.25,
                                        ALU.mult, ALU.add)
                nc.vector.tensor_mul(w2e[:, :], w2e[:, :], w2e[:, :])
                A2, R2 = wt("A2"), wt("R2")
                nc.vector.tensor_mul(A2[:, :], af[:, :], af[:, :])
                nc.vector.tensor_mul(R2[:, :], ab[:, :], ab[:, :])
                de, dr = wt("de"), wt("dr")
                nc.vector.tensor_scalar(de[:, :], A2[:, :], 1.0, None, ALU.add)
                nc.vector.reciprocal(de[:, :], de[:, :])
                nc.vector.tensor_scalar(dr[:, :], R2[:, :], 1.0, None, ALU.add)
                nc.vector.reciprocal(dr[:, :], dr[:, :])
                ce, se, cr, sr = wt("ta"), wt("tb"), wt("u1"), wt("u2")
                nc.vector.tensor_scalar(t1[:, :], A2[:, :], -1.0, 1.0,
                                        ALU.mult, ALU.add)
                nc.vector.tensor_mul(ce[:, :], t1[:, :], de[:, :])
                nc.vector.tensor_scalar(t1[:, :], af[:, :], 2.0, None, ALU.mult)
                nc.vector.tensor_mul(se[:, :], t1[:, :], de[:, :])
                nc.vector.tensor_scalar(t1[:, :], R2[:, :], -1.0, 1.0,
                                        ALU.mult, ALU.add)
                nc.vector.tensor_mul(cr[:, :], t1[:, :], dr[:, :])
                nc.vector.tensor_scalar(t1[:, :], ab[:, :], 2.0, None, ALU.mult)
                nc.vector.tensor_mul(sr[:, :], t1[:, :], dr[:, :])
                c_e, s_e = wt("A2"), wt("R2")
                nc.vector.tensor_mul(t1[:, :], ce[:, :], cr[:, :])
                nc.vector.tensor_mul(t2[:, :], se[:, :], sr[:, :])
                nc.vector.tensor_add(c_e[:, :], t1[:, :], t2[:, :])
                nc.vector.tensor_mul(t1[:, :], sr[:, :], ce[:, :])
                nc.vector.tensor_mul(t2[:, :], se[:, :], cr[:, :])
                nc.vector.tensor_sub(s_e[:, :], t1[:, :], t2[:, :])
                # rotation coefs in bf16 (w2 folded in)
                nc.vector.tensor_mul(c2b[:, :], c_e[:, :], w2e[:, :])
                nc.vector.tensor_mul(s2b[:, :], s_e[:, :], w2e[:, :])
                nc.vector.tensor_copy(w2b[:, :], w2e[:, :])

                # ---- deg (on-chip one-hot matmuls, LO + HI PSUM passes) ----
                degLt = psp.tile([128, 128], f32, tag="mm", name="degL")
                degHt = psp.tile([128, 128], f32, tag="mm", name="degH")
                degL = degLt[:, 0:NW]
                degH = degHt[:, 0:NW]
                for (k, reg, w, st, sp) in winchunks:
                    sel = selp.tile([128, 128], bf16, tag="sel", name="sel")
                    nc.vector.tensor_scalar(sel[:, :], iota_b[:, :],
                                            rloc_b[:, k:k + 1], None,
                                            ALU.is_equal)
                    degP = degL if reg == 0 else degH
                    nc.tensor.matmul(degP[:, w:w + 1], sel[:],
                                     w2b[:, k:k + 1], start=st, stop=sp)
                deg = wt("ta")
                nc.vector.tensor_copy(deg[:, 0:NW], degL)
                nc.vector.tensor_add(deg[:, 0:NW], deg[:, 0:NW], degH)
                nc.vector.tensor_scalar(diag_sh[:, :], deg[:, 0:NW], 1e30, 1.0,
                                        ALU.mult, ALU.min)
                nc.vector.tensor_scalar(deg[:, 0:NW], deg[:, 0:NW], 1e-30,
                                        None, ALU.max)
                rrec = wt("tb")
                nc.vector.reciprocal(rrec[:, 0:NW], deg[:, 0:NW])
                nc.scalar.activation(dinv_sh[:, :], rrec[:, 0:NW], AF.Sqrt)
                ny = wt("u1")
                nc.vector.tensor_mul(ny[:, 0:NW], dinv_sh[:, :], dinv_sh[:, :])
                nc.vector.tensor_mul(ny[:, 0:NW], ny[:, 0:NW], deg[:, 0:NW])
                nc.vector.tensor_scalar(ny[:, 0:NW], ny[:, 0:NW], -0.5, 1.5,
                                        ALU.mult, ALU.add)
                nc.vector.tensor_mul(dinv_sh[:, :], dinv_sh[:, :], ny[:, 0:NW])
                nc.vector.tensor_mul(dinv_sh[:, :], dinv_sh[:, :],
                                     diag_sh[:, :])

                # ---- feature table: dinv * (Wl (x) I) xc (ywl precomputed) ----
                for t in range(NW):
                    nc.scalar.activation(xfN[:, t, 0:F], ywl[:, t, 0:F],
                                         AF.Identity,
                                         scale=dinv_sh[:, t:t + 1])
                xfsv = xfs[:].rearrange("(w p) f -> p w f", p=128)[:, :, 0:F]
                nc.sync.dma_start(xfsv[:, 0:25, :], xfN[:, 0:25, :])
                nc.sync.dma_start(xfsv[:, 25:NW, :], xfN[:, 25:NW, :])

                # ---- y-branch (diag * (Wl (x) Wr) x): overlaps the AG ----
                for t in range(NW):
                    x0T = tpose(slabT[:, t, 0:F], 128, F)
                    yTp = ps1p.tile([128, 128], f32, tag="tp")
                    nc.tensor.matmul(yTp[:F, :128],
                                     wmt[:, L * F:(L + 1) * F],
                                     x0T[:F, :128], start=True, stop=True)
                    yT = workp.tile([F, 128], f32, tag="yT")
                    nc.scalar.copy(yT[:, :], yTp[:F, :128])
                    yN = ps1p.tile([128, 128], f32, tag="tp")
                    nc.tensor.transpose(yN[:128, :F], yT[:, :128], ident[:F, :F])
                    nc.scalar.activation(z1a[:, t, 0:F], yN[:128, :F],
                                         AF.Identity,
                                         scale=diag_sh[:, t:t + 1])

                nc.gpsimd.collective_compute(
                    "AllGather", ALU.bypass, replica_groups=RG,
                    ins=[xfs[0:SPLA, :]], outs=[XFA[:]])
                nc.gpsimd.collective_compute(
                    "AllGather", ALU.bypass, replica_groups=RG,
                    ins=[xfs[SPLA:SH, :]], outs=[XFB[:]])

                # ---- phase 4: x-update per window (fused into messages) ----
                def phase4_win(t):
                    aT = tpose(aggsh[:, t, :], 128, F)
                    awp = ps2p.tile([128, 128], f32, tag="m2")
                    nc.tensor.matmul(awp[:F, :128],
                                     wrkt[:, L * F:(L + 1) * F],
                                     aT[:F, :128], start=True, stop=True)
                    awT = workp.tile([F, 128], f32, tag="tsum")
                    nc.scalar.copy(awT[:, :], awp[:F, :128])
                    awN = ps1p.tile([128, 128], f32, tag="tp")
                    nc.tensor.transpose(awN[:128, :F], awT[:, :128],
                                        ident[:F, :F])
                    z1 = workp.tile([128, F], f32, tag="z1")
                    z2 = workp.tile([128, F], f32, tag="z2")
                    nc.scalar.activation(z2[:, :], awN[:128, :F], AF.Identity,
                                         scale=dinv_sh[:, t:t + 1])
                    nc.vector.tensor_sub(z1[:, :], z1a[:, t, 0:F], z2[:, :])
                    ez = workp.tile([128, F], f32, tag="ez")
                    nc.scalar.activation(ez[:, :], z1[:, :], AF.Exp)
                    nc.vector.tensor_scalar(ez[:, :], ez[:, :], 1.0, -1.0,
                                            ALU.min, ALU.add)
                    rz = workp.tile([128, F], f32, tag="rz")
                    nc.scalar.activation(rz[:, :], z1[:, :], AF.Relu)
                    nc.vector.tensor_add(ez[:, :], ez[:, :], rz[:, :])
                    for i in range(FD):
                        blk = slice(i * HID, (i + 1) * HID)
                        cf = cfb[:, L * FD + i:L * FD + i + 1]
                        nc.vector.tensor_scalar(slabT[:, t, blk],
                                                slabT[:, t, blk], cf, None,
                                                ALU.mult)
                    nc.vector.tensor_sub(slabT[:, t, 0:F], slabT[:, t, 0:F],
                                         ez[:, :])
                    if L + 1 < NL:
                        xpT = tpose(slabT[:, t, 0:F], 128, F)
                        cp4 = ps2p.tile([128, 128], f32, tag="m2")
                        nc.tensor.matmul(cp4[:4, :128],
                                         w4sb[:, (L + 1) * 4:(L + 2) * 4],
                                         xpT[:F, :128], start=True, stop=True)
                        c4s = workp.tile([4, 128], f32, tag="c4s")
                        nc.scalar.copy(c4s[:, :], cp4[:4, :128])
                        cN = ps1p.tile([128, 128], f32, tag="tp")
                        nc.tensor.transpose(cN[:128, :4], c4s[:, :128],
                                            ident[:4, :4])
                        nc.vector.tensor_copy(ctile[:, t, :], cN[:128, :4])
                        ywp = ps2p.tile([128, 128], f32, tag="m2")
                        nc.tensor.matmul(ywp[:F, :128],
                                         wlit[:, (L + 1) * F:(L + 2) * F],
                                         xpT[:F, :128], start=True, stop=True)
                        ywT = workp.tile([F, 128], f32, tag="tsum")
                        nc.scalar.copy(ywT[:, :], ywp[:F, :128])
                        ywN = ps1p.tile([128, 128], f32, tag="tp")
                        nc.tensor.transpose(ywN[:128, :F], ywT[:, :128],
                                            ident[:F, :F])
                        nc.scalar.copy(ywl[:, t, 0:F], ywN[:128, :F])
                def lin2_win(t):
                    xT = tpose(slabT[:, t, 0:F], 128, F)
                    op = ps1p.tile([128, 128], f32, tag="tp")
                    nc.tensor.matmul(op[:OUT_CH, :128], w2t[:, :],
                                     xT[:F, :128], start=True, stop=True)
                    ob = workp.tile([OUT_CH, 128], f32, tag="l2ob")
                    nc.scalar.activation(ob[:, :], op[:OUT_CH, :128],
                                         AF.Identity, bias=b2sb[:, :])
                    oN = ps1p.tile([128, 128], f32, tag="tp")
                    nc.tensor.transpose(oN[:128, :OUT_CH], ob[:, :128],
                                        ident[:OUT_CH, :OUT_CH])
                    os_ = workp.tile([128, OUT_CH], f32, tag="l2os")
                    nc.vector.tensor_copy(os_[:, :], oN[:128, :OUT_CH])
                    nc.sync.dma_start(out_d[t * 128:(t + 1) * 128, :],
                                      os_[:, :])


                # ---- messages: gather + rotate + aggregate ----
                aggP = None
                cur = None
                for ci in range(ncalls):
                    k0 = ci * CCH
                    reg0 = 0 if k0 < nlo_p else 1
                    src = (XFA[:, 0:F] if reg0 == 0 else XFB[:, 0:F])
                    gf = feap.tile([128, CCH, F], bf16, tag="gf", name="gf")
                    raw_gather(
                        nc.gpsimd, gf[:, :, :], src,
                        idxf_sb[:, ci * 64:(ci + 1) * 64], CALL, F, 128,
                        queue_num=ci % 2)
                    msg = msgp.tile([128, CCH, F], bf16, tag="msg", name="msg")
                    ksl = slice(k0, k0 + CCH)
                    c2r = c2b[:, ksl].unsqueeze(2).broadcast_to([128, CCH, HID])
                    s2r = s2b[:, ksl].unsqueeze(2).broadcast_to([128, CCH, HID])
                    w2r = w2b[:, ksl].unsqueeze(2).broadcast_to([128, CCH, HID])
                    g0b = gf[:, :, 0:HID]
                    g1b = gf[:, :, HID:2 * HID]
                    g2b = gf[:, :, 2 * HID:3 * HID]
                    tA = msgp.tile([128, CCH, HID], bf16, tag="tA", name="tA")
                    tB = msgp.tile([128, CCH, HID], bf16, tag="tB", name="tB")
                    nc.vector.tensor_mul(tA[:, :, :], g0b, c2r)
                    nc.vector.tensor_mul(tB[:, :, :], g1b, s2r)
                    nc.vector.tensor_sub(msg[:, :, 0:HID], tA[:, :, :],
                                         tB[:, :, :])
                    nc.vector.tensor_mul(tA[:, :, :], g0b, s2r)
                    nc.vector.tensor_mul(tB[:, :, :], g1b, c2r)
                    nc.vector.tensor_add(msg[:, :, HID:2 * HID], tA[:, :, :],
                                         tB[:, :, :])
                    nc.vector.tensor_mul(msg[:, :, 2 * HID:3 * HID], g2b, w2r)
                    for j in range(CCH):
                        (k, reg, w, st, sp) = winchunks[k0 + j]
                        sel = selp.tile([128, 128], bf16, tag="sel", name="sel")
                        nc.vector.tensor_scalar(sel[:, :], iota_b[:, :],
                                                rloc_b[:, k:k + 1], None,
                                                ALU.is_equal)
                        if st:
                            aggPt = psp.tile([128, 128], f32, tag="mm")
                            aggP = aggPt[:, 0:F]
                            cur = (reg, w)
                        assert cur == (reg, w)
                        nc.tensor.matmul(aggP, sel[:], msg[:, j, :],
                                         start=st, stop=sp)
                        if sp:
                            if reg == 0:
                                nc.scalar.copy(aggsh[:, w, :], aggP)
                            else:
                                nc.vector.tensor_add(aggsh[:, w, :],
                                                     aggsh[:, w, :],
                                                     aggP)
                                phase4_win(w)
                                if L + 1 == NL:
                                    lin2_win(w)

                if L + 1 < NL:
                    contrib_write(L + 1)

    nc.compile()
    return nc


def kernel(x, edge_index, W1, b1, W2, b2, W_left, W_right, eps,
           W_sheaf, W_wt):
    from concourse.bass_utils import run_bass_kernel_spmd
    in_maps, pad_id = _host_prep(x, edge_index, W1, b1, W2, b2, W_left,
                                 W_right, eps, W_sheaf, W_wt)
    plan = _CACHE['plan']
    key = (plan['QL'], plan['QH'])
    if _CACHE.get('key') != key:
        _CACHE['nc'] = _build_program(plan)
        _CACHE['key'] = key
    nc = _CACHE['nc']
    res = run_bass_kernel_spmd(nc, in_maps, list(range(NCORES)))
    full = np.concatenate([res.results[c]["out"] for c in range(NCORES)],
                          axis=0)
    return full[pad_id].astype(np.float32)



# revision 15
# speedup vs baseline: 2.2170x; 2.2170x over previous
"""Trainium2 Bass kernel for DiscreteBundleSheafDiffusion (D=2, FD=3, HID=32).

Redesign vs baseline: all per-edge gathers go through batched dma_gather
(1024 indices per call, int16 wrapped+replicated index tables) instead of
canonical [128,1] indirect DMAs; one-hot row-selection matrices for the
segment-sum matmuls are generated on-chip per chunk via tensor_scalar
is_equal (4x DVE mode) instead of streamed from HBM; the gathered feature
table holds dinv[v] * (Wl (x) I) xc[v] in bf16 (so per-edge work is only a
2D rotation + w2 scale, and the dinv AllGather disappears); aggregation
matmuls run in bf16 with f32 PSUM accumulate.

Tables: contribs (4 sheaf/weight projections per node) live 2-node-packed
in XC2 [NPAD/2, 64] f32 (256B rows, parity-selected after gather, index =
node>>1 fits int16); features live in XF [NPAD, 128] bf16 (256B rows) with
chunks class-sorted by col < 32768 (LO) vs >= 32768 (HI) so gathers address
XF[0:HALF] / XF[HALF:] with int16 indices. Per-window chunk capacities
QL/QH are data-derived maxima, uniform across cores (single SPMD program).
"""
import sys
sys.path.insert(0, '/opt/trn_rl_repo')
import numpy as np

N_NODES = 50000
E0 = 200000
IN_CH = 128
OUT_CH = 32
N_LAYERS = 2
FD, HID = 3, 32
F = FD * HID
NCORES = 8
SHR = 6250
SH = 6272
NW = SH // 128
NPAD = NCORES * SH
SPLA = 3200          # class-A rows per core slice (25 windows)
CALL = 1024           # indices per dma_gather call
CCH = CALL // 128     # chunks per feature/contrib call (8)

_CACHE = {}


def _spectral_normalize_np(W, iters=20):
    W = np.asarray(W, np.float32)
    u = np.full((W.shape[0],), 1.0 / np.sqrt(W.shape[0]), np.float32)
    for _ in range(iters):
        v = W.T @ u
        v = v / (np.linalg.norm(v) + np.float32(1e-12))
        u2 = W @ v
        u = u2 / (np.linalg.norm(u2) + np.float32(1e-12))
    v = W.T @ u
    v = v / (np.linalg.norm(v) + np.float32(1e-12))
    sigma = u @ W @ v
    return W / sigma


def _wrap_calls(seq2d):
    """seq2d: [ncalls, 1024] int -> [128, ncalls*64] int16 wrapped+replicated."""
    ncalls = seq2d.shape[0]
    out = np.zeros((128, ncalls * 64), np.int16)
    for j in range(ncalls):
        w = seq2d[j].reshape(64, 16).T.astype(np.int16)   # [16, 64]
        out[:, j * 64:(j + 1) * 64] = np.tile(w, (8, 1))
    return out


def _plan_chunks(edge_index):
    """Class-sorted chunk packing; returns plan dict + per-core edge arrays."""
    ei = np.asarray(edge_index)
    row = ei[0].astype(np.int64)
    col = ei[1].astype(np.int64)
    n_ids = np.arange(N_NODES)
    pad_id = (n_ids // SHR) * SH + (n_ids % SHR)
    rowp = pad_id[row]
    colp = pad_id[col]

    cores = []
    ql = qh = 0
    for c in range(NCORES):
        m = (rowp // SH) == c
        r = (rowp[m] - c * SH).astype(np.int64)
        cl = colp[m].astype(np.int64)
        hi = ((cl % SH) >= SPLA).astype(np.int64)
        order = np.lexsort((cl, r, hi, r // 128))
        r, cl, hi = r[order], cl[order], hi[order]
        w = r // 128
        for ww in range(NW):
            nlo = int(((w == ww) & (hi == 0)).sum())
            nhi = int(((w == ww) & (hi == 1)).sum())
            ql = max(ql, (nlo + 127) // 128)
            qh = max(qh, (nhi + 127) // 128)
        cores.append((r, cl, hi))

    nlo_p = -(-NW * ql // CCH) * CCH
    nhi_p = -(-NW * qh // CCH) * CCH
    nca = nlo_p + nhi_p
    plan = dict(QL=ql, QH=qh, NLOP=nlo_p, NHIP=nhi_p, NCA=nca)

    # chunk k -> (region, window, start, stop) in k order
    winchunks = []
    for k in range(nca):
        if k < nlo_p:
            reg, q, base = 0, ql, 0
            w = min(k // ql, NW - 1)
            k0 = w * ql
            k1 = nlo_p if w == NW - 1 else (w + 1) * ql
        else:
            reg, q, base = 1, qh, nlo_p
            kk = k - nlo_p
            w = min(kk // qh, NW - 1)
            k0 = base + w * qh
            k1 = nca if w == NW - 1 else base + (w + 1) * qh
        winchunks.append((k, reg, w, k == k0, k == k1 - 1))
    plan['winchunks'] = winchunks
    return plan, cores, pad_id


def _host_prep(x, edge_index, W1, b1, W2, b2, W_left, W_right, eps,
               W_sheaf, W_wt):
    plan, cores, pad_id = _plan_chunks(edge_index)
    _CACHE['plan'] = plan
    ql, qh = plan['QL'], plan['QH']
    nlo_p, nca = plan['NLOP'], plan['NCA']
    x = np.asarray(x, np.float32)

    in_maps = []
    for c in range(NCORES):
        r, cl, hi = cores[c]
        colp_arr = np.zeros((nca, 128), np.int64)      # global padded col id
        rloc = np.full((nca, 128), -1, np.int64)
        rglob = np.zeros((nca, 128), np.int64)         # global padded row id
        valid = np.zeros((nca, 128), bool)
        w = r // 128
        for ww in range(NW):
            for reg in range(2):
                msel = (w == ww) & (hi == reg)
                rw, cw = r[msel], cl[msel]
                cnt = rw.shape[0]
                cap = (ql if reg == 0 else qh) * 128
                assert cnt <= cap, f"window overflow {cnt} > {cap}"
                base = ww * ql if reg == 0 else nlo_p + ww * qh
                for qq in range((cnt + 127) // 128):
                    a, b = qq * 128, min(qq * 128 + 128, cnt)
                    k = base + qq
                    colp_arr[k, :b - a] = cw[a:b]
                    rloc[k, :b - a] = rw[a:b] - ww * 128
                    rglob[k, :b - a] = c * SH + rw[a:b]
                    valid[k, :b - a] = True

        # gather index sequences per call of CCH chunks
        ncalls = nca // CCH
        iC = (colp_arr >> 1).reshape(ncalls, CALL)
        iR = np.where(valid, rglob - c * SH, 0).reshape(ncalls, CALL)
        core_of = colp_arr // SH
        rloc_of = colp_arr % SH
        iF = np.where(rloc_of < SPLA,
                      core_of * SPLA + rloc_of,
                      core_of * (SH - SPLA) + rloc_of - SPLA)
        iF[~valid] = 0
        iF = iF.reshape(ncalls, CALL)
        idxc_w = _wrap_calls(iC)
        idxr_w = _wrap_calls(iR)
        idxf_w = _wrap_calls(iF)
        import ml_dtypes as _mldt
        mcolT = (colp_arr & 1).T.astype(np.float32).copy()   # [128, NCA]
        mrowT = (rglob & 1).T.astype(np.float32).copy()
        rloc_b = rloc.T.astype(_mldt.bfloat16).copy()         # [128, NCA]
        in_maps.append({
            "idxc_w": idxc_w, "idxr_w": idxr_w, "idxf_w": idxf_w,
            "mcolT": mcolT, "mrowT": mrowT, "rloc_b": rloc_b,
        })

    import ml_dtypes
    W1 = np.asarray(W1, np.float32); b1 = np.asarray(b1, np.float32)
    W2 = np.asarray(W2, np.float32); b2 = np.asarray(b2, np.float32)
    NL = N_LAYERS
    w4 = np.zeros((F, NL * 4), np.float32)
    wmt = np.zeros((F, NL * F), np.float32)
    wrkt = np.zeros((F, NL * F), np.float32)
    wlit = np.zeros((F, NL * F), np.float32)
    cfb = np.zeros((128, NL * FD), np.float32)
    for l in range(NL):
        sh_row = np.asarray(W_sheaf[l][1], np.float32)
        wt_row = np.asarray(W_wt[l][0], np.float32)
        w4[:, l * 4 + 0] = sh_row[:F]
        w4[:, l * 4 + 1] = sh_row[F:]
        w4[:, l * 4 + 2] = wt_row[:F]
        w4[:, l * 4 + 3] = wt_row[F:]
        Wl = _spectral_normalize_np(np.asarray(W_left[l], np.float32))
        Wr = _spectral_normalize_np(np.asarray(W_right[l], np.float32))
        wmt[:, l * F:(l + 1) * F] = np.kron(Wl, Wr).astype(np.float32).T
        wrkt[:, l * F:(l + 1) * F] = \
            np.kron(np.eye(FD, dtype=np.float32), Wr).astype(np.float32).T
        wlit[:, l * F:(l + 1) * F] = \
            np.kron(Wl, np.eye(HID, dtype=np.float32)).astype(np.float32).T
        cfb[:, l * FD:(l + 1) * FD] = \
            (1.0 + np.tanh(np.asarray(eps[l], np.float32))).reshape(1, FD)

    xp = np.zeros((NPAD, IN_CH), np.float32)
    xp[pad_id] = x
    iota_b = np.tile(np.arange(128, dtype=np.float32)[None, :],
                     (128, 1)).astype(ml_dtypes.bfloat16)
    shared = {
        "w1t": W1.T.copy(), "b1f": b1.reshape(F, 1).copy(),
        "w2t": W2.T.copy(), "b2": b2.reshape(OUT_CH, 1).copy(),
        "w4": w4, "wmt": wmt, "wrkt": wrkt, "wlit": wlit, "cfb": cfb,
        "iota_b": iota_b, "ident": np.eye(128, dtype=np.float32),
    }
    for c in range(NCORES):
        in_maps[c]["x_sh"] = xp[c * SH:(c + 1) * SH].T.copy()
        in_maps[c].update(shared)
    return in_maps, pad_id


# =================== bass program ===================
def _build_program(plan):
    import concourse.bacc as bacc
    import concourse.bass as bass
    import concourse.mybir as mybir
    from concourse import tile

    ql, qh = plan['QL'], plan['QH']
    nlo_p, nca = plan['NLOP'], plan['NCA']
    winchunks = plan['winchunks']
    ncalls = nca // CCH
    NL = N_LAYERS
    f32 = mybir.dt.float32
    bf16 = mybir.dt.bfloat16
    i16 = mybir.dt.int16
    AF = mybir.ActivationFunctionType
    ALU = mybir.AluOpType

    nc = bacc.Bacc("TRN2", target_bir_lowering=False, debug=False,
                   num_swdge_queues=4)

    def raw_gather(gps, out_ap, in_ap, idxs_ap, num_idxs, elem_size,
                   elem_step, queue_num=0):
        """dma_gather allowing elem_size < 256B (row stride must be %256B)."""
        stride_bytes = elem_step * mybir.dt.size(in_ap.dtype)
        assert stride_bytes % 256 == 0
        assert in_ap.ap[0][0] == elem_step
        assert in_ap.ap[-1][1] == out_ap.ap[-1][1] == elem_size
        _in_ap = gps.lower_ap_dma(in_ap, for_custom_bir_dma=True)
        _idxs_ap = gps.lower_ap(idxs_ap)
        _out_ap = gps.lower_ap(out_ap)
        return gps.add_instruction(
            mybir.InstDMAGatherAnt(
                name=gps.bass.get_next_instruction_name(),
                ins=[*_in_ap, _idxs_ap,
                     gps.lower_val_access(gps.to_reg(num_idxs))],
                outs=[_out_ap],
                transpose=False,
                num_idxs=num_idxs,
                elem_size=elem_size,
                stride_bytes_256=stride_bytes // 256,
                gen_mode=0,
                single_packet=True,
                queue_num=queue_num,
                sbuf_tokens_per_rank=0,
                sbuf_free_dim_per_rank=0,
                sbuf_free_dim_pad_per_rank=0,
                sbuf_byte_offset=0,
            ))


    x_sh = nc.dram_tensor("x_sh", [IN_CH, SH], f32, kind="ExternalInput").ap()
    idxc_d = nc.dram_tensor("idxc_w", [128, ncalls * 64], i16, kind="ExternalInput").ap()
    idxr_d = nc.dram_tensor("idxr_w", [128, ncalls * 64], i16, kind="ExternalInput").ap()
    idxf_d = nc.dram_tensor("idxf_w", [128, ncalls * 64], i16, kind="ExternalInput").ap()
    mcol_d = nc.dram_tensor("mcolT", [128, nca], f32, kind="ExternalInput").ap()
    rloc_d = nc.dram_tensor("rloc_b", [128, nca], bf16, kind="ExternalInput").ap()
    iota_d = nc.dram_tensor("iota_b", [128, 128], bf16, kind="ExternalInput").ap()
    w1t_d = nc.dram_tensor("w1t", [IN_CH, F], f32, kind="ExternalInput").ap()
    b1f_d = nc.dram_tensor("b1f", [F, 1], f32, kind="ExternalInput").ap()
    w2t_d = nc.dram_tensor("w2t", [F, OUT_CH], f32, kind="ExternalInput").ap()
    b2_d = nc.dram_tensor("b2", [OUT_CH, 1], f32, kind="ExternalInput").ap()
    w4_d = nc.dram_tensor("w4", [F, NL * 4], f32, kind="ExternalInput").ap()
    wmt_d = nc.dram_tensor("wmt", [F, NL * F], f32, kind="ExternalInput").ap()
    wrkt_d = nc.dram_tensor("wrkt", [F, NL * F], f32, kind="ExternalInput").ap()
    wlit_d = nc.dram_tensor("wlit", [F, NL * F], f32, kind="ExternalInput").ap()
    cfb_d = nc.dram_tensor("cfb", [128, NL * FD], f32, kind="ExternalInput").ap()
    ident_d = nc.dram_tensor("ident", [128, 128], f32, kind="ExternalInput").ap()
    out_d = nc.dram_tensor("out", [SH, OUT_CH], f32, kind="ExternalOutput").ap()

    xcs = nc.dram_tensor("xcs", [SH, 4], f32)
    XCFULL = nc.dram_tensor("XCFULL", [NPAD, 4], f32, addr_space="Shared")
    XC2 = nc.dram_tensor("XC2", [NPAD // 2, 64], f32)
    XR2 = nc.dram_tensor("XR2", [SH, 64], f32)
    xfs = nc.dram_tensor("xfs", [SH, 128], bf16)
    XFA = nc.dram_tensor("XFA", [NCORES * SPLA, 128], bf16, addr_space="Shared")
    XFB = nc.dram_tensor("XFB", [NCORES * (SH - SPLA), 128], bf16, addr_space="Shared")
    RG = [list(range(NCORES))]

    with tile.TileContext(nc) as tc:
        with tc.tile_pool(name="const", bufs=1) as constp, \
             tc.tile_pool(name="big", bufs=1) as bigp, \
             tc.tile_pool(name="wide", bufs=1) as widep, \
             tc.tile_pool(name="gath", bufs=3) as gathp, \
             tc.tile_pool(name="fea", bufs=4) as feap, \
             tc.tile_pool(name="selp", bufs=6) as selp, \
             tc.tile_pool(name="work", bufs=3) as workp, \
             tc.tile_pool(name="msgp", bufs=4) as msgp, \
             tc.tile_pool(name="ps", bufs=2, space="PSUM") as psp, \
             tc.tile_pool(name="ps2", bufs=2, space="PSUM") as ps2p, \
             tc.tile_pool(name="ps1", bufs=4, space="PSUM") as ps1p:

            def C(name, shape, src, dt=f32):
                t = constp.tile(shape, dt, tag=name, name=name)
                nc.sync.dma_start(t[:], src)
                return t

            ident = C("ident", [128, 128], ident_d[:])
            iota_b = C("iota", [128, 128], iota_d[:], dt=bf16)
            w1t = C("w1t", [IN_CH, F], w1t_d[:])
            b1f = C("b1f", [F, 1], b1f_d[:])
            w2t = C("w2t", [F, OUT_CH], w2t_d[:])
            b2sb = C("b2", [OUT_CH, 1], b2_d[:])
            w4sb = C("w4", [F, NL * 4], w4_d[:])
            wmt = C("wmt", [F, NL * F], wmt_d[:])
            wrkt = C("wrkt", [F, NL * F], wrkt_d[:])
            wlit = C("wlit", [F, NL * F], wlit_d[:])
            cfb = C("cfb", [128, NL * FD], cfb_d[:])
            mcol = C("mcol", [128, nca], mcol_d[:])
            rloc_b = C("rloc", [128, nca], rloc_d[:], dt=bf16)
            idxc_sb = C("idxc", [128, ncalls * 64], idxc_d[:], dt=i16)
            idxr_sb = C("idxr", [128, ncalls * 64], idxr_d[:], dt=i16)
            idxf_sb = C("idxf", [128, ncalls * 64], idxf_d[:], dt=i16)

            slabT = bigp.tile([128, NW, F], f32, tag="slabT")
            ctile = bigp.tile([128, NW, 4], f32, tag="ctile")
            aggsh = bigp.tile([128, NW, F], f32, tag="aggsh")
            xfN = bigp.tile([128, NW, F], bf16, tag="xfN")
            ywl = bigp.tile([128, NW, F], bf16, tag="ywl")
            z1a = bigp.tile([128, NW, F], f32, tag="z1a")
            dinv_sh = bigp.tile([128, NW], f32, tag="dinvsh")
            diag_sh = bigp.tile([128, NW], f32, tag="diagsh")
            ccall = bigp.tile([128, nca, 4], f32, tag="ccall")
            rcall = bigp.tile([128, nca, 4], f32, tag="rcall")
            c2b = bigp.tile([128, nca], bf16, tag="c2b")
            s2b = bigp.tile([128, nca], bf16, tag="s2b")
            w2b = bigp.tile([128, nca], bf16, tag="w2b")

            def tpose(src_ap, pdim, fdim, tag="tx"):
                pt = ps1p.tile([128, 128], f32, tag="tp", name="tp")
                nc.tensor.transpose(pt[:fdim, :pdim], src_ap,
                                    ident[:pdim, :pdim])
                dst = workp.tile([128, 128], f32, tag=tag, name=tag)
                nc.scalar.copy(dst[:fdim, :pdim], pt[:fdim, :pdim])
                return dst

            def contrib_write(L):
                """ctile -> xcs -> AllGather -> XCFULL -> expand into XC2.
                Also writes the local 2-node-packed row table XR2 (pre-AG so
                row-side gathers can overlap the collective)."""
                nc.sync.dma_start(
                    XR2[:, 0:4].rearrange("(w p) f -> p w f", p=128),
                    ctile[:, :, :])
                nc.sync.dma_start(
                    xcs[:].rearrange("(w p) f -> p w f", p=128), ctile[:, :, :])
                nc.gpsimd.collective_compute(
                    "AllGather", ALU.bypass, replica_groups=RG,
                    ins=[xcs[:]], outs=[XCFULL[:]])
                ct2 = widep.tile([128, NPAD // 256, 8], f32, tag="ct2",
                                 name="ct2")
                nc.sync.dma_start(
                    ct2[:, :, :],
                    XCFULL[:].rearrange("(c p two) f -> p c (two f)",
                                        p=128, two=2))
                nc.sync.dma_start(
                    XC2[:, 0:8].rearrange("(c p) f8 -> p c f8", p=128),
                    ct2[:, :, :])

            # ---------------- lin1 + layer-0 contribs ----------------
            for t in range(NW):
                xT = workp.tile([128, 128], f32, tag="xt")
                nc.sync.dma_start(xT[:], x_sh[:, t * 128:(t + 1) * 128])
                hp = psp.tile([128, 128], f32, tag="mm")
                nc.tensor.matmul(hp[:F, :128], w1t[:], xT[:IN_CH, :128],
                                 start=True, stop=True)
                e1 = workp.tile([F, 128], f32, tag="e1")
                nc.scalar.activation(e1[:, :], hp[:F, :128], AF.Exp,
                                     bias=b1f[:, :])
                nc.vector.tensor_scalar(e1[:, :], e1[:, :], 1.0, -1.0,
                                        ALU.min, ALU.add)
                r1 = workp.tile([F, 128], f32, tag="r1")
                nc.scalar.activation(r1[:, :], hp[:F, :128], AF.Relu,
                                     bias=b1f[:, :])
                hF = workp.tile([F, 128], f32, tag="hF")
                nc.vector.tensor_add(hF[:, :], e1[:, :], r1[:, :])
                cp4 = ps1p.tile([128, 128], f32, tag="tp")
                nc.tensor.matmul(cp4[:4, :128], w4sb[:, 0:4], hF[:, :128],
                                 start=True, stop=True)
                c4s = workp.tile([4, 128], f32, tag="c4s")
                nc.scalar.copy(c4s[:, :], cp4[:4, :128])
                hN = ps1p.tile([128, 128], f32, tag="tp")
                nc.tensor.transpose(hN[:128, :F], hF[:, :128], ident[:F, :F])
                nc.vector.tensor_copy(slabT[:, t, 0:F], hN[:128, :F])
                cN = ps1p.tile([128, 128], f32, tag="tp")
                nc.tensor.transpose(cN[:128, :4], c4s[:, :128], ident[:4, :4])
                nc.vector.tensor_copy(ctile[:, t, :], cN[:128, :4])
                ywp = psp.tile([128, 128], f32, tag="mm")
                nc.tensor.matmul(ywp[:F, :128], wlit[:, 0:F], hF[:, :128],
                                 start=True, stop=True)
                ywT = workp.tile([F, 128], f32, tag="tsum")
                nc.vector.tensor_copy(ywT[:, :], ywp[:F, :128])
                ywN = ps1p.tile([128, 128], f32, tag="tp")
                nc.tensor.transpose(ywN[:128, :F], ywT[:, :128], ident[:F, :F])
                nc.vector.tensor_copy(ywl[:, t, 0:F], ywN[:128, :F])
                ymp = ps2p.tile([128, 128], f32, tag="m2")
                nc.tensor.matmul(ymp[:F, :128], wmt[:, 0:F], hF[:, :128],
                                 start=True, stop=True)
                ymT = workp.tile([F, 128], f32, tag="tsum")
                nc.scalar.copy(ymT[:, :], ymp[:F, :128])
                ymN = ps1p.tile([128, 128], f32, tag="tp")
                nc.tensor.transpose(ymN[:128, :F], ymT[:, :128], ident[:F, :F])
                nc.vector.tensor_copy(z1a[:, t, 0:F], ymN[:128, :F])
            contrib_write(0)

            # =================== layers ===================
            for L in range(NL):
                # ---- phase 1: contrib gathers ----
                # rows first: depend only on local XR2, so they run under the
                # XC AllGather + expand; cols wait on XC2.
                GB = 4  # calls per select batch (32 chunks)

                def psel(gt, mt, dst, ks, nk):
                    d = msgp.tile([128, GB * CCH, 4], f32, tag="d4", name="d4")
                    nc.vector.tensor_sub(d[:, :nk, :], gt[:, :nk, 4:8],
                                         gt[:, :nk, 0:4])
                    nc.vector.tensor_mul(
                        d[:, :nk, :], d[:, :nk, :],
                        mt[:, ks].unsqueeze(2).broadcast_to([128, nk, 4]))
                    nc.vector.tensor_add(dst[:, ks, :], gt[:, :nk, 0:4],
                                         d[:, :nk, :])

                for g in range(ncalls):
                    sl = slice(g * 64, (g + 1) * 64)
                    raw_gather(
                        nc.gpsimd, rcall[:, g * CCH:(g + 1) * CCH, :],
                        XR2[:, 0:4], idxr_sb[:, sl], CALL, 4, 64,
                        queue_num=g % 4)
                for g0 in range(0, ncalls, GB):
                    gb = min(GB, ncalls - g0)
                    gt = gathp.tile([128, GB * CCH, 8], f32, tag="gc",
                                    name="gc")
                    for j in range(gb):
                        g = g0 + j
                        sl = slice(g * 64, (g + 1) * 64)
                        raw_gather(
                            nc.gpsimd, gt[:, j * CCH:(j + 1) * CCH, :],
                            XC2[:, 0:8], idxc_sb[:, sl], CALL, 8, 64,
                            queue_num=g % 4)
                    psel(gt, mcol, ccall,
                         slice(g0 * CCH, (g0 + gb) * CCH), gb * CCH)

                # ---- learner algebra (f32, [128, NCA]) ----
                def wt(tag):
                    return widep.tile([128, nca], f32, tag=tag, name=tag)
                rc, cc = rcall, ccall
                ta, tb = wt("ta"), wt("tb")
                nc.vector.tensor_add(ta[:, :], rc[:, :, 0], cc[:, :, 1])
                nc.vector.tensor_add(tb[:, :], cc[:, :, 0], rc[:, :, 1])
                af, ab = wt("af"), wt("ab")
                nc.scalar.activation(af[:, :], ta[:, :], AF.Tanh)
                nc.scalar.activation(ab[:, :], tb[:, :], AF.Tanh)
                nc.vector.tensor_add(ta[:, :], rc[:, :, 2], cc[:, :, 3])
                nc.vector.tensor_add(tb[:, :], cc[:, :, 2], rc[:, :, 3])
                u1, u2 = wt("u1"), wt("u2")
                nc.scalar.activation(u1[:, :], ta[:, :], AF.Tanh, scale=0.5)
                nc.scalar.activation(u2[:, :], tb[:, :], AF.Tanh, scale=0.5)
                w2e, t1, t2 = wt("w2e"), wt("t1"), wt("t2")
                nc.vector.tensor_mul(t1[:, :], u1[:, :], u2[:, :])
                nc.vector.tensor_add(t2[:, :], u1[:, :], u2[:, :])
                nc.vector.tensor_add(t1[:, :], t1[:, :], t2[:, :])
                nc.vector.tensor_scalar(w2e[:, :], t1[:, :], 0.25, 0.25,
                                        ALU.mult, ALU.add)
                nc.vector.tensor_mul(w2e[:, :], w2e[:, :], w2e[:, :])
                A2, R2 = wt("A2"), wt("R2")
                nc.vector.tensor_mul(A2[:, :], af[:, :], af[:, :])
                nc.vector.tensor_mul(R2[:, :], ab[:, :], ab[:, :])
                de, dr = wt("de"), wt("dr")
                nc.vector.tensor_scalar(de[:, :], A2[:, :], 1.0, None, ALU.add)
                nc.vector.reciprocal(de[:, :], de[:, :])
                nc.vector.tensor_scalar(dr[:, :], R2[:, :], 1.0, None, ALU.add)
                nc.vector.reciprocal(dr[:, :], dr[:, :])
                ce, se, cr, sr = wt("ta"), wt("tb"), wt("u1"), wt("u2")
                nc.vector.tensor_scalar(t1[:, :], A2[:, :], -1.0, 1.0,
                                        ALU.mult, ALU.add)
                nc.vector.tensor_mul(ce[:, :], t1[:, :], de[:, :])
                nc.vector.tensor_scalar(t1[:, :], af[:, :], 2.0, None, ALU.mult)
                nc.vector.tensor_mul(se[:, :], t1[:, :], de[:, :])
                nc.vector.tensor_scalar(t1[:, :], R2[:, :], -1.0, 1.0,
                                        ALU.mult, ALU.add)
                nc.vector.tensor_mul(cr[:, :], t1[:, :], dr[:, :])
                nc.vector.tensor_scalar(t1[:, :], ab[:, :], 2.0, None, ALU.mult)
                nc.vector.tensor_mul(sr[:, :], t1[:, :], dr[:, :])
                c_e, s_e = wt("A2"), wt("R2")
                nc.vector.tensor_mul(t1[:, :], ce[:, :], cr[:, :])
                nc.vector.tensor_mul(t2[:, :], se[:, :], sr[:, :])
                nc.vector.tensor_add(c_e[:, :], t1[:, :], t2[:, :])
                nc.vector.tensor_mul(t1[:, :], sr[:, :], ce[:, :])
                nc.vector.tensor_mul(t2[:, :], se[:, :], cr[:, :])
                nc.vector.tensor_sub(s_e[:, :], t1[:, :], t2[:, :])
                # rotation coefs in bf16 (w2 folded in)
                nc.vector.tensor_mul(c2b[:, :], c_e[:, :], w2e[:, :])
                nc.vector.tensor_mul(s2b[:, :], s_e[:, :], w2e[:, :])
                nc.vector.tensor_copy(w2b[:, :], w2e[:, :])

                # ---- deg (on-chip one-hot matmuls, LO + HI PSUM passes) ----
                degLt = psp.tile([128, 128], f32, tag="mm", name="degL")
                degHt = psp.tile([128, 128], f32, tag="mm", name="degH")
                degL = degLt[:, 0:NW]
                degH = degHt[:, 0:NW]
                for g0 in range(0, nca, CCH):
                    selw = selp.tile([128, CCH, 128], bf16, tag="selw",
                                     name="selw")
                    nc.vector.tensor_tensor(
                        selw[:, :, :],
                        iota_b[:, :].unsqueeze(1).broadcast_to(
                            [128, CCH, 128]),
                        rloc_b[:, g0:g0 + CCH].unsqueeze(2).broadcast_to(
                            [128, CCH, 128]),
                        ALU.is_equal)
                    for j in range(CCH):
                        (k, reg, w, st, sp) = winchunks[g0 + j]
                        degP = degL if reg == 0 else degH
                        nc.tensor.matmul(degP[:, w:w + 1], selw[:, j, :],
                                         w2b[:, k:k + 1], start=st, stop=sp)
                deg = wt("ta")
                nc.vector.tensor_copy(deg[:, 0:NW], degL)
                nc.vector.tensor_add(deg[:, 0:NW], deg[:, 0:NW], degH)
                nc.vector.tensor_scalar(diag_sh[:, :], deg[:, 0:NW], 1e30, 1.0,
                                        ALU.mult, ALU.min)
                nc.vector.tensor_scalar(deg[:, 0:NW], deg[:, 0:NW], 1e-30,
                                        None, ALU.max)
                rrec = wt("tb")
                nc.vector.reciprocal(rrec[:, 0:NW], deg[:, 0:NW])
                nc.scalar.activation(dinv_sh[:, :], rrec[:, 0:NW], AF.Sqrt)
                ny = wt("u1")
                nc.vector.tensor_mul(ny[:, 0:NW], dinv_sh[:, :], dinv_sh[:, :])
                nc.vector.tensor_mul(ny[:, 0:NW], ny[:, 0:NW], deg[:, 0:NW])
                nc.vector.tensor_scalar(ny[:, 0:NW], ny[:, 0:NW], -0.5, 1.5,
                                        ALU.mult, ALU.add)
                nc.vector.tensor_mul(dinv_sh[:, :], dinv_sh[:, :], ny[:, 0:NW])
                nc.vector.tensor_mul(dinv_sh[:, :], dinv_sh[:, :],
                                     diag_sh[:, :])

                # ---- feature table: dinv * (Wl (x) I) xc (ywl precomputed) ----
                for t in range(NW):
                    nc.scalar.activation(xfN[:, t, 0:F], ywl[:, t, 0:F],
                                         AF.Identity,
                                         scale=dinv_sh[:, t:t + 1])
                xfsv = xfs[:].rearrange("(w p) f -> p w f", p=128)[:, :, 0:F]
                nc.sync.dma_start(xfsv[:, 0:25, :], xfN[:, 0:25, :])
                nc.sync.dma_start(xfsv[:, 25:NW, :], xfN[:, 25:NW, :])

                nc.gpsimd.collective_compute(
                    "AllGather", ALU.bypass, replica_groups=RG,
                    ins=[xfs[0:SPLA, :]], outs=[XFA[:]])
                nc.gpsimd.collective_compute(
                    "AllGather", ALU.bypass, replica_groups=RG,
                    ins=[xfs[SPLA:SH, :]], outs=[XFB[:]])

                # ---- phase 4: x-update per window (fused into messages) ----
                def phase4_win(t):
                    aT = tpose(aggsh[:, t, :], 128, F)
                    awp = ps2p.tile([128, 128], f32, tag="m2")
                    nc.tensor.matmul(awp[:F, :128],
                                     wrkt[:, L * F:(L + 1) * F],
                                     aT[:F, :128], start=True, stop=True)
                    awT = workp.tile([F, 128], f32, tag="tsum")
                    nc.scalar.copy(awT[:, :], awp[:F, :128])
                    awN = ps1p.tile([128, 128], f32, tag="tp")
                    nc.tensor.transpose(awN[:128, :F], awT[:, :128],
                                        ident[:F, :F])
                    z1 = workp.tile([128, F], f32, tag="z1")
                    z2 = workp.tile([128, F], f32, tag="z2")
                    nc.scalar.activation(z2[:, :], awN[:128, :F], AF.Identity,
                                         scale=dinv_sh[:, t:t + 1])
                    z1s = workp.tile([128, F], f32, tag="z1s")
                    nc.scalar.activation(z1s[:, :], z1a[:, t, 0:F],
                                         AF.Identity,
                                         scale=diag_sh[:, t:t + 1])
                    nc.vector.tensor_sub(z1[:, :], z1s[:, :], z2[:, :])
                    ez = workp.tile([128, F], f32, tag="ez")
                    nc.scalar.activation(ez[:, :], z1[:, :], AF.Exp)
                    nc.vector.tensor_scalar(ez[:, :], ez[:, :], 1.0, -1.0,
                                            ALU.min, ALU.add)
                    rz = workp.tile([128, F], f32, tag="rz")
                    nc.scalar.activation(rz[:, :], z1[:, :], AF.Relu)
                    nc.vector.tensor_add(ez[:, :], ez[:, :], rz[:, :])
                    for i in range(FD):
                        blk = slice(i * HID, (i + 1) * HID)
                        cf = cfb[:, L * FD + i:L * FD + i + 1]
                        nc.vector.tensor_scalar(slabT[:, t, blk],
                                                slabT[:, t, blk], cf, None,
                                                ALU.mult)
                    nc.vector.tensor_sub(slabT[:, t, 0:F], slabT[:, t, 0:F],
                                         ez[:, :])
                    if L + 1 < NL:
                        xpT = tpose(slabT[:, t, 0:F], 128, F)
                        cp4 = ps2p.tile([128, 128], f32, tag="m2")
                        nc.tensor.matmul(cp4[:4, :128],
                                         w4sb[:, (L + 1) * 4:(L + 2) * 4],
                                         xpT[:F, :128], start=True, stop=True)
                        c4s = workp.tile([4, 128], f32, tag="c4s")
                        nc.scalar.copy(c4s[:, :], cp4[:4, :128])
                        cN = ps1p.tile([128, 128], f32, tag="tp")
                        nc.tensor.transpose(cN[:128, :4], c4s[:, :128],
                                            ident[:4, :4])
                        nc.vector.tensor_copy(ctile[:, t, :], cN[:128, :4])
                        ywp = ps2p.tile([128, 128], f32, tag="m2")
                        nc.tensor.matmul(ywp[:F, :128],
                                         wlit[:, (L + 1) * F:(L + 2) * F],
                                         xpT[:F, :128], start=True, stop=True)
                        ywT = workp.tile([F, 128], f32, tag="tsum")
                        nc.scalar.copy(ywT[:, :], ywp[:F, :128])
                        ywN = ps1p.tile([128, 128], f32, tag="tp")
                        nc.tensor.transpose(ywN[:128, :F], ywT[:, :128],
                                            ident[:F, :F])
                        nc.scalar.copy(ywl[:, t, 0:F], ywN[:128, :F])
                        ymp = ps2p.tile([128, 128], f32, tag="m2")
                        nc.tensor.matmul(ymp[:F, :128],
                                         wmt[:, (L + 1) * F:(L + 2) * F],
                                         xpT[:F, :128], start=True, stop=True)
                        ymT = workp.tile([F, 128], f32, tag="tsum")
                        nc.scalar.copy(ymT[:, :], ymp[:F, :128])
                        ymN = ps1p.tile([128, 128], f32, tag="tp")
                        nc.tensor.transpose(ymN[:128, :F], ymT[:, :128],
                                            ident[:F, :F])
                        nc.vector.tensor_copy(z1a[:, t, 0:F], ymN[:128, :F])
                def lin2_win(t):
                    xT = tpose(slabT[:, t, 0:F], 128, F)
                    op = ps1p.tile([128, 128], f32, tag="tp")
                    nc.tensor.matmul(op[:OUT_CH, :128], w2t[:, :],
                                     xT[:F, :128], start=True, stop=True)
                    ob = workp.tile([OUT_CH, 128], f32, tag="l2ob")
                    nc.scalar.activation(ob[:, :], op[:OUT_CH, :128],
                                         AF.Identity, bias=b2sb[:, :])
                    oN = ps1p.tile([128, 128], f32, tag="tp")
                    nc.tensor.transpose(oN[:128, :OUT_CH], ob[:, :128],
                                        ident[:OUT_CH, :OUT_CH])
                    os_ = workp.tile([128, OUT_CH], f32, tag="l2os")
                    nc.vector.tensor_copy(os_[:, :], oN[:128, :OUT_CH])
                    nc.sync.dma_start(out_d[t * 128:(t + 1) * 128, :],
                                      os_[:, :])


                # ---- messages: gather + rotate + aggregate ----
                aggP = None
                cur = None
                for ci in range(ncalls):
                    k0 = ci * CCH
                    reg0 = 0 if k0 < nlo_p else 1
                    src = (XFA[:, 0:F] if reg0 == 0 else XFB[:, 0:F])
                    gf = feap.tile([128, CCH, F], bf16, tag="gf", name="gf")
                    raw_gather(
                        nc.gpsimd, gf[:, :, :], src,
                        idxf_sb[:, ci * 64:(ci + 1) * 64], CALL, F, 128,
                        queue_num=ci % 4)
                    msg = msgp.tile([128, CCH, F], bf16, tag="msg", name="msg")
                    ksl = slice(k0, k0 + CCH)
                    c2r = c2b[:, ksl].unsqueeze(2).broadcast_to([128, CCH, HID])
                    s2r = s2b[:, ksl].unsqueeze(2).broadcast_to([128, CCH, HID])
                    w2r = w2b[:, ksl].unsqueeze(2).broadcast_to([128, CCH, HID])
                    g0b = gf[:, :, 0:HID]
                    g1b = gf[:, :, HID:2 * HID]
                    g2b = gf[:, :, 2 * HID:3 * HID]
                    tA = msgp.tile([128, CCH, HID], bf16, tag="tA", name="tA")
                    tB = msgp.tile([128, CCH, HID], bf16, tag="tB", name="tB")
                    nc.vector.tensor_mul(tA[:, :, :], g0b, c2r)
                    nc.vector.tensor_mul(tB[:, :, :], g1b, s2r)
                    nc.vector.tensor_sub(msg[:, :, 0:HID], tA[:, :, :],
                                         tB[:, :, :])
                    nc.vector.tensor_mul(tA[:, :, :], g0b, s2r)
                    nc.vector.tensor_mul(tB[:, :, :], g1b, c2r)
                    nc.vector.tensor_add(msg[:, :, HID:2 * HID], tA[:, :, :],
                                         tB[:, :, :])
                    nc.vector.tensor_mul(msg[:, :, 2 * HID:3 * HID], g2b, w2r)
                    selw = selp.tile([128, CCH, 128], bf16, tag="selw",
                                     name="selw")
                    nc.vector.tensor_tensor(
                        selw[:, :, :],
                        iota_b[:, :].unsqueeze(1).broadcast_to(
                            [128, CCH, 128]),
                        rloc_b[:, k0:k0 + CCH].unsqueeze(2).broadcast_to(
                            [128, CCH, 128]),
                        ALU.is_equal)
                    for j in range(CCH):
                        (k, reg, w, st, sp) = winchunks[k0 + j]
                        if st:
                            aggPt = psp.tile([128, 128], f32, tag="mm")
                            aggP = aggPt[:, 0:F]
                            cur = (reg, w)
                        assert cur == (reg, w)
                        nc.tensor.matmul(aggP, selw[:, j, :], msg[:, j, :],
                                         start=st, stop=sp)
                        if sp:
                            if reg == 0:
                                nc.scalar.copy(aggsh[:, w, :], aggP)
                            else:
                                nc.vector.tensor_add(aggsh[:, w, :],
                                                     aggsh[:, w, :],
                                                     aggP)
                                phase4_win(w)
                                if L + 1 == NL:
                                    lin2_win(w)

                if L + 1 < NL:
                    contrib_write(L + 1)

    nc.compile()
    return nc


def kernel(x, edge_index, W1, b1, W2, b2, W_left, W_right, eps,
           W_sheaf, W_wt):
    from concourse.bass_utils import run_bass_kernel_spmd
    in_maps, pad_id = _host_prep(x, edge_index, W1, b1, W2, b2, W_left,
                                 W_right, eps, W_sheaf, W_wt)
    plan = _CACHE['plan']
    key = (plan['QL'], plan['QH'])
    if _CACHE.get('key') != key:
        _CACHE['nc'] = _build_program(plan)
        _CACHE['key'] = key
    nc = _CACHE['nc']
    res = run_bass_kernel_spmd(nc, in_maps, list(range(NCORES)))
    full = np.concatenate([res.results[c]["out"] for c in range(NCORES)],
                          axis=0)
    return full[pad_id].astype(np.float32)



# revision 28
# speedup vs baseline: 3.8999x; 1.7591x over previous
"""Trainium2 Bass kernel for DiscreteBundleSheafDiffusion (D=2, FD=3, HID=32).

Redesign vs baseline: all per-edge gathers go through batched dma_gather
(1024 indices per call, int16 wrapped+replicated index tables) instead of
canonical [128,1] indirect DMAs; one-hot row-selection matrices for the
segment-sum matmuls are generated on-chip per chunk via tensor_scalar
is_equal (4x DVE mode) instead of streamed from HBM; the gathered feature
table holds dinv[v] * (Wl (x) I) xc[v] in bf16 (so per-edge work is only a
2D rotation + w2 scale, and the dinv AllGather disappears); aggregation
matmuls run in bf16 with f32 PSUM accumulate.

Tables: contribs (4 sheaf/weight projections per node) live 2-node-packed
in XC2 [NPAD/2, 64] f32 (256B rows, parity-selected after gather, index =
node>>1 fits int16); features live in XF [NPAD, 128] bf16 (256B rows) with
chunks class-sorted by col < 32768 (LO) vs >= 32768 (HI) so gathers address
XF[0:HALF] / XF[HALF:] with int16 indices. Per-window chunk capacities
QL/QH are data-derived maxima, uniform across cores (single SPMD program).
"""
import sys
sys.path.insert(0, '/opt/trn_rl_repo')
import numpy as np

N_NODES = 50000
E0 = 200000
IN_CH = 128
OUT_CH = 32
N_LAYERS = 2
FD, HID = 3, 32
F = FD * HID
NCORES = 8
SHR = 6250
SH = 6272
NW = SH // 128
NPAD = NCORES * SH
SPLA = 3200          # class-A rows per core slice (25 windows)
CALL = 1024           # indices per dma_gather call
CCH = CALL // 128     # chunks per feature/contrib call (8)

_CACHE = {}


def _spectral_normalize_np(W, iters=20):
    W = np.asarray(W, np.float32)
    u = np.full((W.shape[0],), 1.0 / np.sqrt(W.shape[0]), np.float32)
    for _ in range(iters):
        v = W.T @ u
        v = v / (np.linalg.norm(v) + np.float32(1e-12))
        u2 = W @ v
        u = u2 / (np.linalg.norm(u2) + np.float32(1e-12))
    v = W.T @ u
    v = v / (np.linalg.norm(v) + np.float32(1e-12))
    sigma = u @ W @ v
    return W / sigma


def _wrap_calls(seq2d):
    """seq2d: [ncalls, 1024] int -> [128, ncalls*64] int16 wrapped+replicated."""
    ncalls = seq2d.shape[0]
    out = np.zeros((128, ncalls * 64), np.int16)
    for j in range(ncalls):
        w = seq2d[j].reshape(64, 16).T.astype(np.int16)   # [16, 64]
        out[:, j * 64:(j + 1) * 64] = np.tile(w, (8, 1))
    return out


def _plan_chunks(edge_index):
    """Class-sorted chunk packing; returns plan dict + per-core edge arrays."""
    ei = np.asarray(edge_index)
    row = ei[0].astype(np.int64)
    col = ei[1].astype(np.int64)
    n_ids = np.arange(N_NODES)
    pad_id = (n_ids // SHR) * SH + (n_ids % SHR)
    rowp = pad_id[row]
    colp = pad_id[col]

    cores = []
    ql = qh = 0
    for c in range(NCORES):
        m = (rowp // SH) == c
        r = (rowp[m] - c * SH).astype(np.int64)
        cl = colp[m].astype(np.int64)
        hi = ((cl % SH) >= SPLA).astype(np.int64)
        order = np.lexsort((cl, r, hi, r // 128))
        r, cl, hi = r[order], cl[order], hi[order]
        w = r // 128
        for ww in range(NW):
            nlo = int(((w == ww) & (hi == 0)).sum())
            nhi = int(((w == ww) & (hi == 1)).sum())
            ql = max(ql, (nlo + 127) // 128)
            qh = max(qh, (nhi + 127) // 128)
        cores.append((r, cl, hi))

    nlo_p = -(-NW * ql // CCH) * CCH
    nhi_p = -(-NW * qh // CCH) * CCH
    nca = nlo_p + nhi_p
    plan = dict(QL=ql, QH=qh, NLOP=nlo_p, NHIP=nhi_p, NCA=nca)

    # chunk k -> (region, window, start, stop) in k order
    winchunks = []
    for k in range(nca):
        if k < nlo_p:
            reg, q, base = 0, ql, 0
            w = min(k // ql, NW - 1)
            k0 = w * ql
            k1 = nlo_p if w == NW - 1 else (w + 1) * ql
        else:
            reg, q, base = 1, qh, nlo_p
            kk = k - nlo_p
            w = min(kk // qh, NW - 1)
            k0 = base + w * qh
            k1 = nca if w == NW - 1 else base + (w + 1) * qh
        winchunks.append((k, reg, w, k == k0, k == k1 - 1))
    plan['winchunks'] = winchunks
    return plan, cores, pad_id


def _host_prep(x, edge_index, W1, b1, W2, b2, W_left, W_right, eps,
               W_sheaf, W_wt):
    plan, cores, pad_id = _plan_chunks(edge_index)
    _CACHE['plan'] = plan
    ql, qh = plan['QL'], plan['QH']
    nlo_p, nca = plan['NLOP'], plan['NCA']
    x = np.asarray(x, np.float32)

    in_maps = []
    for c in range(NCORES):
        r, cl, hi = cores[c]
        colp_arr = np.zeros((nca, 128), np.int64)      # global padded col id
        rloc = np.full((nca, 128), -1, np.int64)
        rglob = np.zeros((nca, 128), np.int64)         # global padded row id
        valid = np.zeros((nca, 128), bool)
        w = r // 128
        for ww in range(NW):
            for reg in range(2):
                msel = (w == ww) & (hi == reg)
                rw, cw = r[msel], cl[msel]
                cnt = rw.shape[0]
                cap = (ql if reg == 0 else qh) * 128
                assert cnt <= cap, f"window overflow {cnt} > {cap}"
                base = ww * ql if reg == 0 else nlo_p + ww * qh
                for qq in range((cnt + 127) // 128):
                    a, b = qq * 128, min(qq * 128 + 128, cnt)
                    k = base + qq
                    colp_arr[k, :b - a] = cw[a:b]
                    rloc[k, :b - a] = rw[a:b] - ww * 128
                    rglob[k, :b - a] = c * SH + rw[a:b]
                    valid[k, :b - a] = True

        # gather index sequences per call of CCH chunks
        ncalls = nca // CCH
        iC = (colp_arr >> 1).reshape(ncalls, CALL)
        core_of = colp_arr // SH
        rloc_of = colp_arr % SH
        iF = np.where(rloc_of < SPLA,
                      core_of * SPLA + rloc_of,
                      core_of * (SH - SPLA) + rloc_of - SPLA)
        iF[~valid] = 0
        iF = iF.reshape(ncalls, CALL)
        idxc_w = _wrap_calls(iC)
        idxf_w = _wrap_calls(iF)
        import ml_dtypes as _mldt
        mcolT = (colp_arr & 1).T.astype(np.float32).copy()   # [128, NCA]
        rloc_b = rloc.T.astype(_mldt.bfloat16).copy()         # [128, NCA]
        # transposed one-hots for on-chip row-contrib expansion:
        # selnT[n, k*128+j] = 1 iff rloc[k, j] == n  (layer-independent)
        selnT = np.equal.outer(
            np.arange(128, dtype=np.int64), rloc).astype(
                _mldt.bfloat16).reshape(128, nca * 128)
        in_maps.append({
            "idxc_w": idxc_w, "idxf_w": idxf_w,
            "mcolT": mcolT, "rloc_b": rloc_b, "selnT": selnT,
        })

    import ml_dtypes
    W1 = np.asarray(W1, np.float32); b1 = np.asarray(b1, np.float32)
    W2 = np.asarray(W2, np.float32); b2 = np.asarray(b2, np.float32)
    NL = N_LAYERS
    w4 = np.zeros((F, NL * 4), np.float32)
    wmt = np.zeros((F, NL * F), np.float32)
    wrkt = np.zeros((F, NL * F), np.float32)
    wlit = np.zeros((F, NL * F), np.float32)
    cfb = np.zeros((128, NL * FD), np.float32)
    for l in range(NL):
        sh_row = np.asarray(W_sheaf[l][1], np.float32)
        wt_row = np.asarray(W_wt[l][0], np.float32)
        w4[:, l * 4 + 0] = sh_row[:F]
        w4[:, l * 4 + 1] = sh_row[F:]
        w4[:, l * 4 + 2] = wt_row[:F]
        w4[:, l * 4 + 3] = wt_row[F:]
        Wl = _spectral_normalize_np(np.asarray(W_left[l], np.float32))
        Wr = _spectral_normalize_np(np.asarray(W_right[l], np.float32))
        wmt[:, l * F:(l + 1) * F] = np.kron(Wl, Wr).astype(np.float32).T
        wrkt[:, l * F:(l + 1) * F] = \
            np.kron(np.eye(FD, dtype=np.float32), Wr).astype(np.float32).T
        wlit[:, l * F:(l + 1) * F] = \
            np.kron(Wl, np.eye(HID, dtype=np.float32)).astype(np.float32).T
        cfb[:, l * FD:(l + 1) * FD] = \
            (1.0 + np.tanh(np.asarray(eps[l], np.float32))).reshape(1, FD)

    xp = np.zeros((NPAD, IN_CH), np.float32)
    xp[pad_id] = x
    iota_b = np.tile(np.arange(128, dtype=np.float32)[None, :],
                     (128, 1)).astype(ml_dtypes.bfloat16)
    shared = {
        "w1t": W1.T.copy(), "b1f": b1.reshape(F, 1).copy(),
        "w2t": W2.T.copy(), "b2": b2.reshape(OUT_CH, 1).copy(),
        "w4": w4, "wmt": wmt, "wrkt": wrkt, "wlit": wlit, "cfb": cfb,
        "iota_b": iota_b, "ident": np.eye(128, dtype=np.float32),
    }
    for c in range(NCORES):
        in_maps[c]["x_sh"] = xp[c * SH:(c + 1) * SH].T.copy()
        in_maps[c].update(shared)
    return in_maps, pad_id


# =================== bass program ===================
def _build_program(plan):
    import concourse.bacc as bacc
    import concourse.bass as bass
    import concourse.mybir as mybir
    from concourse import tile

    ql, qh = plan['QL'], plan['QH']
    nlo_p, nca = plan['NLOP'], plan['NCA']
    winchunks = plan['winchunks']
    ncalls = nca // CCH
    NL = N_LAYERS
    f32 = mybir.dt.float32
    bf16 = mybir.dt.bfloat16
    i16 = mybir.dt.int16
    AF = mybir.ActivationFunctionType
    ALU = mybir.AluOpType

    nc = bacc.Bacc("TRN2", target_bir_lowering=False, debug=False,
                   num_swdge_queues=4)

    def raw_gather(gps, out_ap, in_ap, idxs_ap, num_idxs, elem_size,
                   elem_step, queue_num=0):
        """dma_gather allowing elem_size < 256B (row stride must be %256B)."""
        stride_bytes = elem_step * mybir.dt.size(in_ap.dtype)
        assert stride_bytes % 256 == 0
        assert in_ap.ap[0][0] == elem_step
        assert in_ap.ap[-1][1] == out_ap.ap[-1][1] == elem_size
        _in_ap = gps.lower_ap_dma(in_ap, for_custom_bir_dma=True)
        _idxs_ap = gps.lower_ap(idxs_ap)
        _out_ap = gps.lower_ap(out_ap)
        return gps.add_instruction(
            mybir.InstDMAGatherAnt(
                name=gps.bass.get_next_instruction_name(),
                ins=[*_in_ap, _idxs_ap,
                     gps.lower_val_access(gps.to_reg(num_idxs))],
                outs=[_out_ap],
                transpose=False,
                num_idxs=num_idxs,
                elem_size=elem_size,
                stride_bytes_256=stride_bytes // 256,
                gen_mode=0,
                single_packet=True,
                queue_num=queue_num,
                sbuf_tokens_per_rank=0,
                sbuf_free_dim_per_rank=0,
                sbuf_free_dim_pad_per_rank=0,
                sbuf_byte_offset=0,
            ))


    x_sh = nc.dram_tensor("x_sh", [IN_CH, SH], f32, kind="ExternalInput").ap()
    idxc_d = nc.dram_tensor("idxc_w", [128, ncalls * 64], i16, kind="ExternalInput").ap()
    selnT_d = nc.dram_tensor("selnT", [128, nca * 128], bf16, kind="ExternalInput").ap()
    idxf_d = nc.dram_tensor("idxf_w", [128, ncalls * 64], i16, kind="ExternalInput").ap()
    mcol_d = nc.dram_tensor("mcolT", [128, nca], f32, kind="ExternalInput").ap()
    rloc_d = nc.dram_tensor("rloc_b", [128, nca], bf16, kind="ExternalInput").ap()
    iota_d = nc.dram_tensor("iota_b", [128, 128], bf16, kind="ExternalInput").ap()
    w1t_d = nc.dram_tensor("w1t", [IN_CH, F], f32, kind="ExternalInput").ap()
    b1f_d = nc.dram_tensor("b1f", [F, 1], f32, kind="ExternalInput").ap()
    w2t_d = nc.dram_tensor("w2t", [F, OUT_CH], f32, kind="ExternalInput").ap()
    b2_d = nc.dram_tensor("b2", [OUT_CH, 1], f32, kind="ExternalInput").ap()
    w4_d = nc.dram_tensor("w4", [F, NL * 4], f32, kind="ExternalInput").ap()
    wmt_d = nc.dram_tensor("wmt", [F, NL * F], f32, kind="ExternalInput").ap()
    wrkt_d = nc.dram_tensor("wrkt", [F, NL * F], f32, kind="ExternalInput").ap()
    wlit_d = nc.dram_tensor("wlit", [F, NL * F], f32, kind="ExternalInput").ap()
    cfb_d = nc.dram_tensor("cfb", [128, NL * FD], f32, kind="ExternalInput").ap()
    ident_d = nc.dram_tensor("ident", [128, 128], f32, kind="ExternalInput").ap()
    out_d = nc.dram_tensor("out", [SH, OUT_CH], f32, kind="ExternalOutput").ap()

    xcs = nc.dram_tensor("xcs", [SH, 4], f32)
    XCFULL = nc.dram_tensor("XCFULL", [NPAD, 4], f32, addr_space="Shared")
    XC2 = nc.dram_tensor("XC2", [NPAD // 2, 64], f32)
    xfs = nc.dram_tensor("xfs", [SH, 128], bf16)
    XFA = nc.dram_tensor("XFA", [NCORES * SPLA, 128], bf16, addr_space="Shared")
    XFB = nc.dram_tensor("XFB", [NCORES * (SH - SPLA), 128], bf16, addr_space="Shared")
    RG = [list(range(NCORES))]

    with tile.TileContext(nc) as tc:
        with tc.tile_pool(name="const", bufs=1) as constp, \
             tc.tile_pool(name="big", bufs=1) as bigp, \
             tc.tile_pool(name="wide", bufs=1) as widep, \
             tc.tile_pool(name="gath", bufs=3) as gathp, \
             tc.tile_pool(name="fea", bufs=4) as feap, \
             tc.tile_pool(name="selp", bufs=6) as selp, \
             tc.tile_pool(name="seln", bufs=3) as selnp, \
             tc.tile_pool(name="work", bufs=3) as workp, \
             tc.tile_pool(name="msgp", bufs=4) as msgp, \
             tc.tile_pool(name="ps", bufs=2, space="PSUM") as psp, \
             tc.tile_pool(name="ps2", bufs=2, space="PSUM") as ps2p, \
             tc.tile_pool(name="ps1", bufs=4, space="PSUM") as ps1p:

            def C(name, shape, src, dt=f32):
                t = constp.tile(shape, dt, tag=name, name=name)
                nc.sync.dma_start(t[:], src)
                return t

            ident = C("ident", [128, 128], ident_d[:])
            iota_b = C("iota", [128, 128], iota_d[:], dt=bf16)
            w1t = C("w1t", [IN_CH, F], w1t_d[:])
            b1f = C("b1f", [F, 1], b1f_d[:])
            w2t = C("w2t", [F, OUT_CH], w2t_d[:])
            b2sb = C("b2", [OUT_CH, 1], b2_d[:])
            w4sb = C("w4", [F, NL * 4], w4_d[:])
            wmt = C("wmt", [F, NL * F], wmt_d[:])
            wrkt = C("wrkt", [F, NL * F], wrkt_d[:])
            wlit = C("wlit", [F, NL * F], wlit_d[:])
            cfb = C("cfb", [128, NL * FD], cfb_d[:])
            mcol = C("mcol", [128, nca], mcol_d[:])
            rloc_b = C("rloc", [128, nca], rloc_d[:], dt=bf16)
            idxc_sb = C("idxc", [128, ncalls * 64], idxc_d[:], dt=i16)
            idxf_sb = C("idxf", [128, ncalls * 64], idxf_d[:], dt=i16)

            slabT = bigp.tile([128, NW, F], f32, tag="slabT")
            ctile = bigp.tile([128, NW, 4], f32, tag="ctile")
            ctb = bigp.tile([128, NW, 4], bf16, tag="ctb")
            aggsh = bigp.tile([128, NW, F], f32, tag="aggsh")
            xfN = bigp.tile([128, NW, F], bf16, tag="xfN")
            ywl = bigp.tile([128, NW, F], bf16, tag="ywl")
            z1a = bigp.tile([128, NW, F], f32, tag="z1a")
            dinv_sh = bigp.tile([128, NW], f32, tag="dinvsh")
            diag_sh = bigp.tile([128, NW], f32, tag="diagsh")
            ccall = bigp.tile([128, nca, 4], f32, tag="ccall")
            rcall = bigp.tile([128, nca, 4], f32, tag="rcall")
            c2b = bigp.tile([128, nca], bf16, tag="c2b")
            s2b = bigp.tile([128, nca], bf16, tag="s2b")
            w2b = bigp.tile([128, nca], bf16, tag="w2b")

            def tpose(src_ap, pdim, fdim, tag="tx"):
                pt = ps1p.tile([128, 128], f32, tag="tp", name="tp")
                nc.tensor.transpose(pt[:fdim, :pdim], src_ap,
                                    ident[:pdim, :pdim])
                dst = workp.tile([128, 128], f32, tag=tag, name=tag)
                nc.scalar.copy(dst[:fdim, :pdim], pt[:fdim, :pdim])
                return dst

            def contrib_write(L):
                """ctile -> xcs -> AllGather -> XCFULL -> expand into XC2."""
                nc.sync.dma_start(
                    xcs[:].rearrange("(w p) f -> p w f", p=128), ctile[:, :, :])
                nc.gpsimd.collective_compute(
                    "AllGather", ALU.bypass, replica_groups=RG,
                    ins=[xcs[:]], outs=[XCFULL[:]])
                ct2 = widep.tile([128, NPAD // 256, 8], f32, tag="ct2",
                                 name="ct2")
                nc.sync.dma_start(
                    ct2[:, :, :],
                    XCFULL[:].rearrange("(c p two) f -> p c (two f)",
                                        p=128, two=2))
                nc.sync.dma_start(
                    XC2[:, 0:8].rearrange("(c p) f8 -> p c f8", p=128),
                    ct2[:, :, :])

            # ---------------- lin1 + layer-0 contribs ----------------
            for t in range(NW):
                xT = workp.tile([128, 128], f32, tag="xt")
                nc.sync.dma_start(xT[:], x_sh[:, t * 128:(t + 1) * 128])
                hp = psp.tile([128, 128], f32, tag="mm")
                nc.tensor.matmul(hp[:F, :128], w1t[:], xT[:IN_CH, :128],
                                 start=True, stop=True)
                e1 = workp.tile([F, 128], f32, tag="e1")
                nc.scalar.activation(e1[:, :], hp[:F, :128], AF.Exp,
                                     bias=b1f[:, :])
                nc.vector.tensor_scalar(e1[:, :], e1[:, :], 1.0, -1.0,
                                        ALU.min, ALU.add)
                r1 = workp.tile([F, 128], f32, tag="r1")
                nc.scalar.activation(r1[:, :], hp[:F, :128], AF.Relu,
                                     bias=b1f[:, :])
                hF = workp.tile([F, 128], f32, tag="hF")
                nc.vector.tensor_add(hF[:, :], e1[:, :], r1[:, :])
                cp4 = ps1p.tile([128, 128], f32, tag="tp")
                nc.tensor.matmul(cp4[:4, :128], w4sb[:, 0:4], hF[:, :128],
                                 start=True, stop=True)
                c4s = workp.tile([4, 128], f32, tag="c4s")
                nc.scalar.copy(c4s[:, :], cp4[:4, :128])
                hN = ps1p.tile([128, 128], f32, tag="tp")
                nc.tensor.transpose(hN[:128, :F], hF[:, :128], ident[:F, :F])
                nc.vector.tensor_copy(slabT[:, t, 0:F], hN[:128, :F])
                cN = ps1p.tile([128, 128], f32, tag="tp")
                nc.tensor.transpose(cN[:128, :4], c4s[:, :128], ident[:4, :4])
                nc.vector.tensor_copy(ctile[:, t, :], cN[:128, :4])
                nc.vector.tensor_copy(ctb[:, t, :], cN[:128, :4])
                ywp = psp.tile([128, 128], f32, tag="mm")
                nc.tensor.matmul(ywp[:F, :128], wlit[:, 0:F], hF[:, :128],
                                 start=True, stop=True)
                ywT = workp.tile([F, 128], f32, tag="tsum")
                nc.vector.tensor_copy(ywT[:, :], ywp[:F, :128])
                ywN = ps1p.tile([128, 128], f32, tag="tp")
                nc.tensor.transpose(ywN[:128, :F], ywT[:, :128], ident[:F, :F])
                nc.vector.tensor_copy(ywl[:, t, 0:F], ywN[:128, :F])
                ymp = ps2p.tile([128, 128], f32, tag="m2")
                nc.tensor.matmul(ymp[:F, :128], wmt[:, 0:F], hF[:, :128],
                                 start=True, stop=True)
                ymT = workp.tile([F, 128], f32, tag="tsum")
                nc.scalar.copy(ymT[:, :], ymp[:F, :128])
                ymN = ps1p.tile([128, 128], f32, tag="tp")
                nc.tensor.transpose(ymN[:128, :F], ymT[:, :128], ident[:F, :F])
                nc.vector.tensor_copy(z1a[:, t, 0:F], ymN[:128, :F])
            contrib_write(0)

            # =================== layers ===================
            for L in range(NL):
                # ---- phase 1: contribs ----
                # row side: expand per-window ctile to edge slots via
                # streamed transposed one-hots (selnT) on the PE array —
                # depends only on local ctile, so it runs under the XC
                # AllGather + expand; col side gathers wait on XC2.
                GB = 4  # calls per select batch (32 chunks)

                def psel(gt, mt, dst, ks, nk):
                    d = msgp.tile([128, GB * CCH, 4], f32, tag="d4", name="d4")
                    nc.vector.tensor_sub(d[:, :nk, :], gt[:, :nk, 4:8],
                                         gt[:, :nk, 0:4])
                    nc.vector.tensor_mul(
                        d[:, :nk, :], d[:, :nk, :],
                        mt[:, ks].unsqueeze(2).broadcast_to([128, nk, 4]))
                    nc.vector.tensor_add(dst[:, ks, :], gt[:, :nk, 0:4],
                                         d[:, :nk, :])

                for g in range(ncalls):
                    selnw = selnp.tile([128, CALL], bf16, tag="selnw",
                                       name="selnw")
                    nc.sync.dma_start(selnw[:, :],
                                      selnT_d[:, g * CALL:(g + 1) * CALL])
                    rcp = ps1p.tile([128, 128], f32, tag="tp", name="rcp")
                    for j in range(CCH):
                        k = g * CCH + j
                        w = winchunks[k][2]
                        nc.tensor.matmul(rcp[:, j * 4:(j + 1) * 4],
                                         selnw[:, j * 128:(j + 1) * 128],
                                         ctb[:, w, :], start=True, stop=True)
                    nc.scalar.copy(rcall[:, g * CCH:(g + 1) * CCH, :],
                                   rcp[:, 0:32])
                for g0 in range(0, ncalls, GB):
                    gb = min(GB, ncalls - g0)
                    gt = gathp.tile([128, GB * CCH, 8], f32, tag="gc",
                                    name="gc")
                    for j in range(gb):
                        g = g0 + j
                        sl = slice(g * 64, (g + 1) * 64)
                        raw_gather(
                            nc.gpsimd, gt[:, j * CCH:(j + 1) * CCH, :],
                            XC2[:, 0:8], idxc_sb[:, sl], CALL, 8, 64,
                            queue_num=g % 4)
                    psel(gt, mcol, ccall,
                         slice(g0 * CCH, (g0 + gb) * CCH), gb * CCH)

                # ---- learner algebra (f32, [128, NCA]) ----
                def wt(tag):
                    return widep.tile([128, nca], f32, tag=tag, name=tag)
                rc, cc = rcall, ccall
                ta, tb = wt("ta"), wt("tb")
                nc.vector.tensor_add(ta[:, :], rc[:, :, 0], cc[:, :, 1])
                nc.vector.tensor_add(tb[:, :], cc[:, :, 0], rc[:, :, 1])
                af, ab = wt("af"), wt("ab")
                nc.scalar.activation(af[:, :], ta[:, :], AF.Tanh)
                nc.scalar.activation(ab[:, :], tb[:, :], AF.Tanh)
                nc.vector.tensor_add(ta[:, :], rc[:, :, 2], cc[:, :, 3])
                nc.vector.tensor_add(tb[:, :], cc[:, :, 2], rc[:, :, 3])
                u1, u2 = wt("u1"), wt("u2")
                nc.scalar.activation(u1[:, :], ta[:, :], AF.Tanh, scale=0.5)
                nc.scalar.activation(u2[:, :], tb[:, :], AF.Tanh, scale=0.5)
                w2e, t1, t2 = wt("w2e"), wt("t1"), wt("t2")
                nc.vector.tensor_mul(t1[:, :], u1[:, :], u2[:, :])
                nc.vector.tensor_add(t2[:, :], u1[:, :], u2[:, :])
                nc.vector.tensor_add(t1[:, :], t1[:, :], t2[:, :])
                nc.vector.tensor_scalar(w2e[:, :], t1[:, :], 0.25, 0.25,
                                        ALU.mult, ALU.add)
                nc.vector.tensor_mul(w2e[:, :], w2e[:, :], w2e[:, :])
                A2, R2 = wt("A2"), wt("R2")
                nc.vector.tensor_mul(A2[:, :], af[:, :], af[:, :])
                nc.vector.tensor_mul(R2[:, :], ab[:, :], ab[:, :])
                de, dr = wt("de"), wt("dr")
                nc.vector.tensor_scalar(de[:, :], A2[:, :], 1.0, None, ALU.add)
                nc.vector.reciprocal(de[:, :], de[:, :])
                nc.vector.tensor_scalar(dr[:, :], R2[:, :], 1.0, None, ALU.add)
                nc.vector.reciprocal(dr[:, :], dr[:, :])
                ce, se, cr, sr = wt("ta"), wt("tb"), wt("u1"), wt("u2")
                nc.vector.tensor_scalar(t1[:, :], A2[:, :], -1.0, 1.0,
                                        ALU.mult, ALU.add)
                nc.vector.tensor_mul(ce[:, :], t1[:, :], de[:, :])
                nc.vector.tensor_scalar(t1[:, :], af[:, :], 2.0, None, ALU.mult)
                nc.vector.tensor_mul(se[:, :], t1[:, :], de[:, :])
                nc.vector.tensor_scalar(t1[:, :], R2[:, :], -1.0, 1.0,
                                        ALU.mult, ALU.add)
                nc.vector.tensor_mul(cr[:, :], t1[:, :], dr[:, :])
                nc.vector.tensor_scalar(t1[:, :], ab[:, :], 2.0, None, ALU.mult)
                nc.vector.tensor_mul(sr[:, :], t1[:, :], dr[:, :])
                c_e, s_e = wt("A2"), wt("R2")
                nc.vector.tensor_mul(t1[:, :], ce[:, :], cr[:, :])
                nc.vector.tensor_mul(t2[:, :], se[:, :], sr[:, :])
                nc.vector.tensor_add(c_e[:, :], t1[:, :], t2[:, :])
                nc.vector.tensor_mul(t1[:, :], sr[:, :], ce[:, :])
                nc.vector.tensor_mul(t2[:, :], se[:, :], cr[:, :])
                nc.vector.tensor_sub(s_e[:, :], t1[:, :], t2[:, :])
                # rotation coefs in bf16 (w2 folded in)
                nc.vector.tensor_mul(c2b[:, :], c_e[:, :], w2e[:, :])
                nc.vector.tensor_mul(s2b[:, :], s_e[:, :], w2e[:, :])
                nc.vector.tensor_copy(w2b[:, :], w2e[:, :])

                # ---- deg (on-chip one-hot matmuls, LO + HI PSUM passes) ----
                degLt = psp.tile([128, 128], f32, tag="mm", name="degL")
                degHt = psp.tile([128, 128], f32, tag="mm", name="degH")
                degL = degLt[:, 0:NW]
                degH = degHt[:, 0:NW]
                for g0 in range(0, nca, CCH):
                    selw = selp.tile([128, CCH, 128], bf16, tag="selw",
                                     name="selw")
                    nc.vector.tensor_tensor(
                        selw[:, :, :],
                        iota_b[:, :].unsqueeze(1).broadcast_to(
                            [128, CCH, 128]),
                        rloc_b[:, g0:g0 + CCH].unsqueeze(2).broadcast_to(
                            [128, CCH, 128]),
                        ALU.is_equal)
                    for j in range(CCH):
                        (k, reg, w, st, sp) = winchunks[g0 + j]
                        degP = degL if reg == 0 else degH
                        nc.tensor.matmul(degP[:, w:w + 1], selw[:, j, :],
                                         w2b[:, k:k + 1], start=st, stop=sp)
                deg = wt("ta")
                nc.vector.tensor_copy(deg[:, 0:NW], degL)
                nc.vector.tensor_add(deg[:, 0:NW], deg[:, 0:NW], degH)
                nc.vector.tensor_scalar(diag_sh[:, :], deg[:, 0:NW], 1e30, 1.0,
                                        ALU.mult, ALU.min)
                nc.vector.tensor_scalar(deg[:, 0:NW], deg[:, 0:NW], 1e-30,
                                        None, ALU.max)
                rrec = wt("tb")
                nc.vector.reciprocal(rrec[:, 0:NW], deg[:, 0:NW])
                nc.scalar.activation(dinv_sh[:, :], rrec[:, 0:NW], AF.Sqrt)
                ny = wt("u1")
                nc.vector.tensor_mul(ny[:, 0:NW], dinv_sh[:, :], dinv_sh[:, :])
                nc.vector.tensor_mul(ny[:, 0:NW], ny[:, 0:NW], deg[:, 0:NW])
                nc.vector.tensor_scalar(ny[:, 0:NW], ny[:, 0:NW], -0.5, 1.5,
                                        ALU.mult, ALU.add)
                nc.vector.tensor_mul(dinv_sh[:, :], dinv_sh[:, :], ny[:, 0:NW])
                nc.vector.tensor_mul(dinv_sh[:, :], dinv_sh[:, :],
                                     diag_sh[:, :])

                # ---- feature table: dinv * (Wl (x) I) xc (ywl precomputed) ----
                for t in range(NW):
                    nc.scalar.activation(xfN[:, t, 0:F], ywl[:, t, 0:F],
                                         AF.Identity,
                                         scale=dinv_sh[:, t:t + 1])
                xfsv = xfs[:].rearrange("(w p) f -> p w f", p=128)[:, :, 0:F]
                nc.sync.dma_start(xfsv[:, 0:25, :], xfN[:, 0:25, :])
                nc.sync.dma_start(xfsv[:, 25:NW, :], xfN[:, 25:NW, :])

                nc.gpsimd.collective_compute(
                    "AllGather", ALU.bypass, replica_groups=RG,
                    ins=[xfs[0:SPLA, :]], outs=[XFA[:]])
                nc.gpsimd.collective_compute(
                    "AllGather", ALU.bypass, replica_groups=RG,
                    ins=[xfs[SPLA:SH, :]], outs=[XFB[:]])

                # ---- phase 4: x-update per window (fused into messages) ----
                def phase4_win(t):
                    aT = tpose(aggsh[:, t, :], 128, F)
                    awp = ps2p.tile([128, 128], f32, tag="m2")
                    nc.tensor.matmul(awp[:F, :128],
                                     wrkt[:, L * F:(L + 1) * F],
                                     aT[:F, :128], start=True, stop=True)
                    awT = workp.tile([F, 128], f32, tag="tsum")
                    nc.scalar.copy(awT[:, :], awp[:F, :128])
                    awN = ps1p.tile([128, 128], f32, tag="tp")
                    nc.tensor.transpose(awN[:128, :F], awT[:, :128],
                                        ident[:F, :F])
                    z1 = workp.tile([128, F], f32, tag="z1")
                    z2 = workp.tile([128, F], f32, tag="z2")
                    nc.scalar.activation(z2[:, :], awN[:128, :F], AF.Identity,
                                         scale=dinv_sh[:, t:t + 1])
                    z1s = workp.tile([128, F], f32, tag="z1s")
                    nc.scalar.activation(z1s[:, :], z1a[:, t, 0:F],
                                         AF.Identity,
                                         scale=diag_sh[:, t:t + 1])
                    nc.vector.tensor_sub(z1[:, :], z1s[:, :], z2[:, :])
                    ez = workp.tile([128, F], f32, tag="ez")
                    nc.scalar.activation(ez[:, :], z1[:, :], AF.Exp)
                    nc.vector.tensor_scalar(ez[:, :], ez[:, :], 1.0, -1.0,
                                            ALU.min, ALU.add)
                    rz = workp.tile([128, F], f32, tag="rz")
                    nc.scalar.activation(rz[:, :], z1[:, :], AF.Relu)
                    nc.vector.tensor_add(ez[:, :], ez[:, :], rz[:, :])
                    for i in range(FD):
                        blk = slice(i * HID, (i + 1) * HID)
                        cf = cfb[:, L * FD + i:L * FD + i + 1]
                        nc.vector.tensor_scalar(slabT[:, t, blk],
                                                slabT[:, t, blk], cf, None,
                                                ALU.mult)
                    nc.vector.tensor_sub(slabT[:, t, 0:F], slabT[:, t, 0:F],
                                         ez[:, :])
                    if L + 1 < NL:
                        xpT = tpose(slabT[:, t, 0:F], 128, F)
                        cp4 = ps2p.tile([128, 128], f32, tag="m2")
                        nc.tensor.matmul(cp4[:4, :128],
                                         w4sb[:, (L + 1) * 4:(L + 2) * 4],
                                         xpT[:F, :128], start=True, stop=True)
                        c4s = workp.tile([4, 128], f32, tag="c4s")
                        nc.scalar.copy(c4s[:, :], cp4[:4, :128])
                        cN = ps1p.tile([128, 128], f32, tag="tp")
                        nc.tensor.transpose(cN[:128, :4], c4s[:, :128],
                                            ident[:4, :4])
                        nc.vector.tensor_copy(ctile[:, t, :], cN[:128, :4])
                        nc.vector.tensor_copy(ctb[:, t, :], cN[:128, :4])
                        ywp = ps2p.tile([128, 128], f32, tag="m2")
                        nc.tensor.matmul(ywp[:F, :128],
                                         wlit[:, (L + 1) * F:(L + 2) * F],
                                         xpT[:F, :128], start=True, stop=True)
                        ywT = workp.tile([F, 128], f32, tag="tsum")
                        nc.scalar.copy(ywT[:, :], ywp[:F, :128])
                        ywN = ps1p.tile([128, 128], f32, tag="tp")
                        nc.tensor.transpose(ywN[:128, :F], ywT[:, :128],
                                            ident[:F, :F])
                        nc.scalar.copy(ywl[:, t, 0:F], ywN[:128, :F])
                        ymp = ps2p.tile([128, 128], f32, tag="m2")
                        nc.tensor.matmul(ymp[:F, :128],
                                         wmt[:, (L + 1) * F:(L + 2) * F],
                                         xpT[:F, :128], start=True, stop=True)
                        ymT = workp.tile([F, 128], f32, tag="tsum")
                        nc.scalar.copy(ymT[:, :], ymp[:F, :128])
                        ymN = ps1p.tile([128, 128], f32, tag="tp")
                        nc.tensor.transpose(ymN[:128, :F], ymT[:, :128],
                                            ident[:F, :F])
                        nc.vector.tensor_copy(z1a[:, t, 0:F], ymN[:128, :F])
                def lin2_win(t):
                    xT = tpose(slabT[:, t, 0:F], 128, F)
                    op = ps1p.tile([128, 128], f32, tag="tp")
                    nc.tensor.matmul(op[:OUT_CH, :128], w2t[:, :],
                                     xT[:F, :128], start=True, stop=True)
                    ob = workp.tile([OUT_CH, 128], f32, tag="l2ob")
                    nc.scalar.activation(ob[:, :], op[:OUT_CH, :128],
                                         AF.Identity, bias=b2sb[:, :])
                    oN = ps1p.tile([128, 128], f32, tag="tp")
                    nc.tensor.transpose(oN[:128, :OUT_CH], ob[:, :128],
                                        ident[:OUT_CH, :OUT_CH])
                    os_ = workp.tile([128, OUT_CH], f32, tag="l2os")
                    nc.vector.tensor_copy(os_[:, :], oN[:128, :OUT_CH])
                    nc.sync.dma_start(out_d[t * 128:(t + 1) * 128, :],
                                      os_[:, :])


                # ---- messages: gather + rotate + aggregate ----
                aggP = None
                cur = None
                for ci in range(ncalls):
                    k0 = ci * CCH
                    reg0 = 0 if k0 < nlo_p else 1
                    src = (XFA[:, 0:F] if reg0 == 0 else XFB[:, 0:F])
                    gf = feap.tile([128, CCH, F], bf16, tag="gf", name="gf")
                    raw_gather(
                        nc.gpsimd, gf[:, :, :], src,
                        idxf_sb[:, ci * 64:(ci + 1) * 64], CALL, F, 128,
                        queue_num=ci % 4)
                    msg = msgp.tile([128, CCH, F], bf16, tag="msg", name="msg")
                    ksl = slice(k0, k0 + CCH)
                    c2r = c2b[:, ksl].unsqueeze(2).broadcast_to([128, CCH, HID])
                    s2r = s2b[:, ksl].unsqueeze(2).broadcast_to([128, CCH, HID])
                    w2r = w2b[:, ksl].unsqueeze(2).broadcast_to([128, CCH, HID])
                    g0b = gf[:, :, 0:HID]
                    g1b = gf[:, :, HID:2 * HID]
                    g2b = gf[:, :, 2 * HID:3 * HID]
                    tA = msgp.tile([128, CCH, HID], bf16, tag="tA", name="tA")
                    tB = msgp.tile([128, CCH, HID], bf16, tag="tB", name="tB")
                    nc.vector.tensor_mul(tA[:, :, :], g0b, c2r)
                    nc.vector.tensor_mul(tB[:, :, :], g1b, s2r)
                    nc.vector.tensor_sub(msg[:, :, 0:HID], tA[:, :, :],
                                         tB[:, :, :])
                    nc.vector.tensor_mul(tA[:, :, :], g0b, s2r)
                    nc.vector.tensor_mul(tB[:, :, :], g1b, c2r)
                    nc.vector.tensor_add(msg[:, :, HID:2 * HID], tA[:, :, :],
                                         tB[:, :, :])
                    nc.vector.tensor_mul(msg[:, :, 2 * HID:3 * HID], g2b, w2r)
                    selw = selp.tile([128, CCH, 128], bf16, tag="selw",
                                     name="selw")
                    nc.vector.tensor_tensor(
                        selw[:, :, :],
                        iota_b[:, :].unsqueeze(1).broadcast_to(
                            [128, CCH, 128]),
                        rloc_b[:, k0:k0 + CCH].unsqueeze(2).broadcast_to(
                            [128, CCH, 128]),
                        ALU.is_equal)
                    for j in range(CCH):
                        (k, reg, w, st, sp) = winchunks[k0 + j]
                        if st:
                            aggPt = psp.tile([128, 128], f32, tag="mm")
                            aggP = aggPt[:, 0:F]
                            cur = (reg, w)
                        assert cur == (reg, w)
                        nc.tensor.matmul(aggP, selw[:, j, :], msg[:, j, :],
                                         start=st, stop=sp)
                        if sp:
                            if reg == 0:
                                nc.scalar.copy(aggsh[:, w, :], aggP)
                            else:
                                nc.vector.tensor_add(aggsh[:, w, :],
                                                     aggsh[:, w, :],
                                                     aggP)
                                phase4_win(w)
                                if L + 1 == NL:
                                    lin2_win(w)

                if L + 1 < NL:
                    contrib_write(L + 1)

    nc.compile()
    return nc


def kernel(x, edge_index, W1, b1, W2, b2, W_left, W_right, eps,
           W_sheaf, W_wt):
    from concourse.bass_utils import run_bass_kernel_spmd
    in_maps, pad_id = _host_prep(x, edge_index, W1, b1, W2, b2, W_left,
                                 W_right, eps, W_sheaf, W_wt)
    plan = _CACHE['plan']
    key = (plan['QL'], plan['QH'])
    if _CACHE.get('key') != key:
        _CACHE['nc'] = _build_program(plan)
        _CACHE['key'] = key
    nc = _CACHE['nc']
    res = run_bass_kernel_spmd(nc, in_maps, list(range(NCORES)))
    full = np.concatenate([res.results[c]["out"] for c in range(NCORES)],
                          axis=0)
    return full[pad_id].astype(np.float32)



# revision 60
# speedup vs baseline: 5.2366x; 1.3427x over previous
"""Trainium2 Bass kernel for DiscreteBundleSheafDiffusion (D=2, FD=3, HID=32).

Redesign vs baseline: all per-edge gathers go through batched dma_gather
(1024 indices per call, int16 wrapped+replicated index tables) instead of
canonical [128,1] indirect DMAs; one-hot row-selection matrices for the
segment-sum matmuls are generated on-chip per chunk via tensor_scalar
is_equal (4x DVE mode) instead of streamed from HBM; the gathered feature
table holds dinv[v] * (Wl (x) I) xc[v] in bf16 (so per-edge work is only a
2D rotation + w2 scale, and the dinv AllGather disappears); aggregation
matmuls run in bf16 with f32 PSUM accumulate.

Tables: contribs (4 sheaf/weight projections per node) live 2-node-packed
in XC2 [NPAD/2, 64] f32 (256B rows, parity-selected after gather, index =
node>>1 fits int16); features live in XF [NPAD, 128] bf16 (256B rows) with
chunks class-sorted by col < 32768 (LO) vs >= 32768 (HI) so gathers address
XF[0:HALF] / XF[HALF:] with int16 indices. Per-window chunk capacities
QL/QH are data-derived maxima, uniform across cores (single SPMD program).
"""
import sys
sys.path.insert(0, '/opt/trn_rl_repo')
import numpy as np

N_NODES = 50000
E0 = 200000
IN_CH = 128
OUT_CH = 32
N_LAYERS = 2
FD, HID = 3, 32
F = FD * HID
NCORES = 8
SHR = 6250
SH = 6272
NW = SH // 128
NPAD = NCORES * SH
CALL = 1024           # indices per dma_gather call
CCH = CALL // 128     # chunks per feature/contrib call (8)

_CACHE = {}


def _spectral_normalize_np(W, iters=20):
    W = np.asarray(W, np.float32)
    u = np.full((W.shape[0],), 1.0 / np.sqrt(W.shape[0]), np.float32)
    for _ in range(iters):
        v = W.T @ u
        v = v / (np.linalg.norm(v) + np.float32(1e-12))
        u2 = W @ v
        u = u2 / (np.linalg.norm(u2) + np.float32(1e-12))
    v = W.T @ u
    v = v / (np.linalg.norm(v) + np.float32(1e-12))
    sigma = u @ W @ v
    return W / sigma


def _wrap_calls(seq2d):
    """seq2d: [ncalls, 1024] int -> [128, ncalls*64] int16 wrapped+replicated."""
    ncalls = seq2d.shape[0]
    out = np.zeros((128, ncalls * 64), np.int16)
    for j in range(ncalls):
        w = seq2d[j].reshape(64, 16).T.astype(np.int16)   # [16, 64]
        out[:, j * 64:(j + 1) * 64] = np.tile(w, (8, 1))
    return out


def _plan_chunks(edge_index):
    """Window-sorted single-class chunk packing; returns plan + per-core
    edge arrays. Capacity Q per window is the max over cores/windows so a
    single SPMD program covers every core (short windows pad with invalid
    slots: rloc = -1 -> all-zero sel columns)."""
    ei = np.asarray(edge_index)
    row = ei[0].astype(np.int64)
    col = ei[1].astype(np.int64)
    n_ids = np.arange(N_NODES)
    pad_id = (n_ids // SHR) * SH + (n_ids % SHR)
    rowp = pad_id[row]
    colp = pad_id[col]

    cores = []
    q = 0
    for c in range(NCORES):
        m = (rowp // SH) == c
        r = (rowp[m] - c * SH).astype(np.int64)
        cl = colp[m].astype(np.int64)
        order = np.lexsort((cl, r))
        r, cl = r[order], cl[order]
        w = r // 128
        for ww in range(NW):
            cnt = int((w == ww).sum())
            q = max(q, (cnt + 127) // 128)
        cores.append((r, cl))

    nca = -(-NW * q // CCH) * CCH
    plan = dict(Q=q, NCA=nca)

    # chunk k -> (window, start, stop) in k order; padding chunks land in
    # the last window (all-invalid, contribute zeros).
    winchunks = []
    for k in range(nca):
        w = min(k // q, NW - 1)
        k0 = w * q
        k1 = nca if w == NW - 1 else (w + 1) * q
        winchunks.append((k, 0, w, k == k0, k == k1 - 1))
    plan['winchunks'] = winchunks
    return plan, cores, pad_id


def _host_prep(x, edge_index, W1, b1, W2, b2, W_left, W_right, eps,
               W_sheaf, W_wt):
    plan, cores, pad_id = _plan_chunks(edge_index)
    _CACHE['plan'] = plan
    q, nca = plan['Q'], plan['NCA']
    x = np.asarray(x, np.float32)

    in_maps = []
    for c in range(NCORES):
        r, cl = cores[c]
        colp_arr = np.zeros((nca, 128), np.int64)      # global padded col id
        rloc = np.full((nca, 128), -1, np.int64)
        valid = np.zeros((nca, 128), bool)
        w = r // 128
        for ww in range(NW):
            msel = (w == ww)
            rw, cw = r[msel], cl[msel]
            cnt = rw.shape[0]
            assert cnt <= q * 128, f"window overflow {cnt} > {q * 128}"
            base = ww * q
            for qq in range((cnt + 127) // 128):
                a, b = qq * 128, min(qq * 128 + 128, cnt)
                k = base + qq
                colp_arr[k, :b - a] = cw[a:b]
                rloc[k, :b - a] = rw[a:b] - ww * 128
                valid[k, :b - a] = True

        # contrib gather: 2-node XC2 rows keyed by col>>1 (pair 2k,2k+1);
        # feature gather: 2-node XF2 rows keyed by (core, win, p%64)
        # (pair p, p+64 within a window - DMA-expressible write pattern)
        ncalls = nca // CCH
        iC = (colp_arr >> 1).reshape(ncalls, CALL)
        idxc_w = _wrap_calls(iC)
        core_of = colp_arr // SH
        lr_of = colp_arr % SH
        w_of = lr_of // 128
        p_of = lr_of % 128
        iF = core_of * (SH // 2) + w_of * 64 + (p_of % 64)
        iF[~valid] = 0
        idxf_w = _wrap_calls(iF.reshape(ncalls, CALL))
        import ml_dtypes as _mldt
        mcolT = (colp_arr & 1).T.astype(np.float32).copy()   # [128, NCA]
        mcolb = (p_of >= 64).T.astype(_mldt.bfloat16)        # feature parity
        mcolbi = (p_of < 64).T.astype(_mldt.bfloat16)        # 1 - parity
        rloc_b = rloc.T.astype(_mldt.bfloat16).copy()         # [128, NCA]
        # transposed one-hots for on-chip row-contrib expansion:
        # selnT[n, k*128+j] = 1 iff rloc[k, j] == n  (layer-independent)
        selnT = np.equal.outer(
            np.arange(128, dtype=np.int64), rloc).astype(
                _mldt.bfloat16).reshape(128, nca * 128)
        in_maps.append({
            "idxc_w": idxc_w, "idxf_w": idxf_w, "mcolT": mcolT,
            "mcolb": mcolb, "mcolbi": mcolbi, "rloc_b": rloc_b,
            "selnT": selnT,
        })

    import ml_dtypes
    W1 = np.asarray(W1, np.float32); b1 = np.asarray(b1, np.float32)
    W2 = np.asarray(W2, np.float32); b2 = np.asarray(b2, np.float32)
    NL = N_LAYERS
    QC = 4 + F   # contrib (4) + q = (Wl (x) Wr) x (F) per layer
    wqm = np.zeros((F, NL * QC), np.float32)
    cfb = np.zeros((128, NL * FD), np.float32)
    for l in range(NL):
        sh_row = np.asarray(W_sheaf[l][1], np.float32)
        wt_row = np.asarray(W_wt[l][0], np.float32)
        wqm[:, l * QC + 0] = sh_row[:F]
        wqm[:, l * QC + 1] = sh_row[F:]
        wqm[:, l * QC + 2] = wt_row[:F]
        wqm[:, l * QC + 3] = wt_row[F:]
        Wl = _spectral_normalize_np(np.asarray(W_left[l], np.float32))
        Wr = _spectral_normalize_np(np.asarray(W_right[l], np.float32))
        wqm[:, l * QC + 4:(l + 1) * QC] = \
            np.kron(Wl, Wr).astype(np.float32).T
        cfb[:, l * FD:(l + 1) * FD] = \
            (1.0 + np.tanh(np.asarray(eps[l], np.float32))).reshape(1, FD)

    xp = np.zeros((NPAD, IN_CH), np.float32)
    xp[pad_id] = x
    iota_b = np.tile(np.arange(128, dtype=np.float32)[None, :],
                     (128, 1)).astype(ml_dtypes.bfloat16)
    shared = {
        "w1t": W1.T.copy(), "b1f": b1.reshape(F, 1).copy(),
        "w2t": W2.T.copy(), "b2": b2.reshape(OUT_CH, 1).copy(),
        "wqm": wqm, "cfb": cfb,
        "iota_b": iota_b, "ident": np.eye(128, dtype=np.float32),
    }
    for c in range(NCORES):
        in_maps[c]["x_sh"] = xp[c * SH:(c + 1) * SH].T.copy()
        in_maps[c].update(shared)
    return in_maps, pad_id


# =================== bass program ===================
def _build_program(plan):
    import concourse.bacc as bacc
    import concourse.bass as bass
    import concourse.mybir as mybir
    from concourse import tile

    nca = plan['NCA']
    winchunks = plan['winchunks']
    ncalls = nca // CCH
    NL = N_LAYERS
    f32 = mybir.dt.float32
    bf16 = mybir.dt.bfloat16
    i16 = mybir.dt.int16
    AF = mybir.ActivationFunctionType
    ALU = mybir.AluOpType

    nc = bacc.Bacc("TRN2", target_bir_lowering=False, debug=False,
                   num_swdge_queues=4)

    def raw_gather(gps, out_ap, in_ap, idxs_ap, num_idxs, elem_size,
                   elem_step, queue_num=0):
        """dma_gather allowing elem_size < 256B (row stride must be %256B)."""
        stride_bytes = elem_step * mybir.dt.size(in_ap.dtype)
        assert stride_bytes % 256 == 0
        assert in_ap.ap[0][0] == elem_step
        assert in_ap.ap[-1][1] == out_ap.ap[-1][1] == elem_size
        _in_ap = gps.lower_ap_dma(in_ap, for_custom_bir_dma=True)
        _idxs_ap = gps.lower_ap(idxs_ap)
        _out_ap = gps.lower_ap(out_ap)
        return gps.add_instruction(
            mybir.InstDMAGatherAnt(
                name=gps.bass.get_next_instruction_name(),
                ins=[*_in_ap, _idxs_ap,
                     gps.lower_val_access(gps.to_reg(num_idxs))],
                outs=[_out_ap],
                transpose=False,
                num_idxs=num_idxs,
                elem_size=elem_size,
                stride_bytes_256=stride_bytes // 256,
                gen_mode=0,
                single_packet=True,
                queue_num=queue_num,
                sbuf_tokens_per_rank=0,
                sbuf_free_dim_per_rank=0,
                sbuf_free_dim_pad_per_rank=0,
                sbuf_byte_offset=0,
            ))


    x_sh = nc.dram_tensor("x_sh", [IN_CH, SH], f32, kind="ExternalInput").ap()
    idxc_d = nc.dram_tensor("idxc_w", [128, ncalls * 64], i16, kind="ExternalInput").ap()
    idxf_d = nc.dram_tensor("idxf_w", [128, ncalls * 64], i16, kind="ExternalInput").ap()
    selnT_d = nc.dram_tensor("selnT", [128, nca * 128], bf16, kind="ExternalInput").ap()
    mcol_d = nc.dram_tensor("mcolT", [128, nca], f32, kind="ExternalInput").ap()
    mcolb_d = nc.dram_tensor("mcolb", [128, nca], bf16, kind="ExternalInput").ap()
    mcolbi_d = nc.dram_tensor("mcolbi", [128, nca], bf16, kind="ExternalInput").ap()
    rloc_d = nc.dram_tensor("rloc_b", [128, nca], bf16, kind="ExternalInput").ap()
    iota_d = nc.dram_tensor("iota_b", [128, 128], bf16, kind="ExternalInput").ap()
    w1t_d = nc.dram_tensor("w1t", [IN_CH, F], f32, kind="ExternalInput").ap()
    b1f_d = nc.dram_tensor("b1f", [F, 1], f32, kind="ExternalInput").ap()
    w2t_d = nc.dram_tensor("w2t", [F, OUT_CH], f32, kind="ExternalInput").ap()
    b2_d = nc.dram_tensor("b2", [OUT_CH, 1], f32, kind="ExternalInput").ap()
    QC = 4 + F
    wqm_d = nc.dram_tensor("wqm", [F, NL * QC], f32, kind="ExternalInput").ap()
    cfb_d = nc.dram_tensor("cfb", [128, NL * FD], f32, kind="ExternalInput").ap()
    ident_d = nc.dram_tensor("ident", [128, 128], f32, kind="ExternalInput").ap()
    out_d = nc.dram_tensor("out", [SH, OUT_CH], f32, kind="ExternalOutput").ap()

    xcs = nc.dram_tensor("xcs", [SH, 4], f32)
    XCFULL = nc.dram_tensor("XCFULL", [NPAD, 4], f32, addr_space="Shared")
    XC2 = nc.dram_tensor("XC2", [NPAD // 2, 64], f32)
    xfs2 = nc.dram_tensor("xfs2", [SH // 2, 256], bf16)
    XF2 = nc.dram_tensor("XF2", [NPAD // 2, 256], bf16, addr_space="Shared")
    RG = [list(range(NCORES))]

    with tile.TileContext(nc) as tc:
        with tc.tile_pool(name="const", bufs=1) as constp, \
             tc.tile_pool(name="big", bufs=1) as bigp, \
             tc.tile_pool(name="wide", bufs=1) as widep, \
             tc.tile_pool(name="gath", bufs=3) as gathp, \
             tc.tile_pool(name="fea", bufs=4) as feap, \
             tc.tile_pool(name="selp", bufs=6) as selp, \
             tc.tile_pool(name="seln", bufs=3) as selnp, \
             tc.tile_pool(name="work", bufs=3) as workp, \
             tc.tile_pool(name="msgp", bufs=4) as msgp, \
             tc.tile_pool(name="ps", bufs=2, space="PSUM") as psp, \
             tc.tile_pool(name="ps2", bufs=2, space="PSUM") as ps2p, \
             tc.tile_pool(name="ps1", bufs=4, space="PSUM") as ps1p:

            def C(name, shape, src, dt=f32):
                t = constp.tile(shape, dt, tag=name, name=name)
                nc.sync.dma_start(t[:], src)
                return t

            ident = C("ident", [128, 128], ident_d[:])
            iota_b = C("iota", [128, 128], iota_d[:], dt=bf16)
            w1t = C("w1t", [IN_CH, F], w1t_d[:])
            b1f = C("b1f", [F, 1], b1f_d[:])
            w2t = C("w2t", [F, OUT_CH], w2t_d[:])
            b2sb = C("b2", [OUT_CH, 1], b2_d[:])
            wqm = C("wqm", [F, NL * QC], wqm_d[:])
            cfb = C("cfb", [128, NL * FD], cfb_d[:])
            mcol = C("mcol", [128, nca], mcol_d[:])
            mcolb = C("mcolb", [128, nca], mcolb_d[:], dt=bf16)
            mcolbi = C("mcolbi", [128, nca], mcolbi_d[:], dt=bf16)
            rloc_b = C("rloc", [128, nca], rloc_d[:], dt=bf16)
            idxc_sb = C("idxc", [128, ncalls * 64], idxc_d[:], dt=i16)
            idxf_sb = C("idxf", [128, ncalls * 64], idxf_d[:], dt=i16)

            slabT = bigp.tile([128, NW, F], f32, tag="slabT")
            ctile = bigp.tile([128, NW, 4], f32, tag="ctile")
            ctb = bigp.tile([128, NW, 4], bf16, tag="ctb")
            xfN = bigp.tile([128, NW, F], bf16, tag="xfN")
            z1a = bigp.tile([128, NW, F], f32, tag="z1a")
            dinv_sh = bigp.tile([128, NW], f32, tag="dinvsh")
            diag_sh = bigp.tile([128, NW], f32, tag="diagsh")
            ccall = bigp.tile([128, nca, 4], f32, tag="ccall")
            rcall = bigp.tile([128, nca, 4], f32, tag="rcall")
            c2b = bigp.tile([128, nca], bf16, tag="c2b")
            s2b = bigp.tile([128, nca], bf16, tag="s2b")
            w2b = bigp.tile([128, nca], bf16, tag="w2b")

            def tpose(src_ap, pdim, fdim, tag="tx"):
                pt = ps1p.tile([128, 128], f32, tag="tp", name="tp")
                nc.tensor.transpose(pt[:fdim, :pdim], src_ap,
                                    ident[:pdim, :pdim])
                dst = workp.tile([128, 128], f32, tag=tag, name=tag)
                nc.scalar.copy(dst[:fdim, :pdim], pt[:fdim, :pdim])
                return dst

            def contrib_write(L):
                """ctile -> xcs -> AllGather -> XCFULL -> expand into XC2."""
                nc.sync.dma_start(
                    xcs[:].rearrange("(w p) f -> p w f", p=128), ctile[:, :, :])
                nc.gpsimd.collective_compute(
                    "AllGather", ALU.bypass, replica_groups=RG,
                    ins=[xcs[:]], outs=[XCFULL[:]])
                ct2 = widep.tile([128, NPAD // 256, 8], f32, tag="ct2",
                                 name="ct2")
                nc.sync.dma_start(
                    ct2[:, :, :],
                    XCFULL[:].rearrange("(c p two) f -> p c (two f)",
                                        p=128, two=2))
                nc.sync.dma_start(
                    XC2[:, 0:8].rearrange("(c p) f8 -> p c f8", p=128),
                    ct2[:, :, :])

            # ---------------- lin1 + layer-0 contribs ----------------
            for t in range(NW):
                xT = workp.tile([128, 128], f32, tag="xt")
                nc.sync.dma_start(xT[:], x_sh[:, t * 128:(t + 1) * 128])
                hp = psp.tile([128, 128], f32, tag="mm")
                nc.tensor.matmul(hp[:F, :128], w1t[:], xT[:IN_CH, :128],
                                 start=True, stop=True)
                e1 = workp.tile([F, 128], f32, tag="e1")
                nc.scalar.activation(e1[:, :], hp[:F, :128], AF.Exp,
                                     bias=b1f[:, :])
                nc.vector.tensor_scalar(e1[:, :], e1[:, :], 1.0, -1.0,
                                        ALU.min, ALU.add)
                r1 = workp.tile([F, 128], f32, tag="r1")
                nc.scalar.activation(r1[:, :], hp[:F, :128], AF.Relu,
                                     bias=b1f[:, :])
                hF = workp.tile([F, 128], f32, tag="hF")
                nc.vector.tensor_add(hF[:, :], e1[:, :], r1[:, :])
                hN = ps1p.tile([128, 128], f32, tag="tp")
                nc.tensor.transpose(hN[:128, :F], hF[:, :128], ident[:F, :F])
                nc.vector.tensor_copy(slabT[:, t, 0:F], hN[:128, :F])
                # contribs + q = (Wl (x) Wr) x in one node-major matmul
                # (stationary = hF, contraction over F)
                qcp = ps2p.tile([128, 128], f32, tag="m2")
                nc.tensor.matmul(qcp[:, 0:QC], hF[:, :128], wqm[:, 0:QC],
                                 start=True, stop=True)
                nc.vector.tensor_copy(ctile[:, t, :], qcp[:, 0:4])
                nc.vector.tensor_copy(ctb[:, t, :], qcp[:, 0:4])
                nc.vector.tensor_copy(z1a[:, t, 0:F], qcp[:, 4:QC])
            contrib_write(0)

            # =================== layers ===================
            for L in range(NL):
                # ---- phase 1: contribs ----
                # row side: expand per-window ctile to edge slots via
                # streamed transposed one-hots (selnT) on the PE array —
                # depends only on local ctile, so it runs under the XC
                # AllGather + expand; col side gathers wait on XC2.
                GB = 4  # calls per select batch (32 chunks)

                def psel(gt, mt, dst, ks, nk):
                    d = msgp.tile([128, GB * CCH, 4], f32, tag="d4", name="d4")
                    nc.vector.tensor_sub(d[:, :nk, :], gt[:, :nk, 4:8],
                                         gt[:, :nk, 0:4])
                    nc.vector.tensor_mul(
                        d[:, :nk, :], d[:, :nk, :],
                        mt[:, ks].unsqueeze(2).broadcast_to([128, nk, 4]))
                    nc.vector.tensor_add(dst[:, ks, :], gt[:, :nk, 0:4],
                                         d[:, :nk, :])

                for g in range(ncalls):
                    selnw = selnp.tile([128, CALL], bf16, tag="selnw",
                                       name="selnw")
                    nc.sync.dma_start(selnw[:, :],
                                      selnT_d[:, g * CALL:(g + 1) * CALL])
                    rcp = ps1p.tile([128, 128], f32, tag="tp", name="rcp")
                    for j in range(CCH):
                        k = g * CCH + j
                        w = winchunks[k][2]
                        nc.tensor.matmul(rcp[:, j * 4:(j + 1) * 4],
                                         selnw[:, j * 128:(j + 1) * 128],
                                         ctb[:, w, :], start=True, stop=True)
                    nc.scalar.copy(rcall[:, g * CCH:(g + 1) * CCH, :],
                                   rcp[:, 0:32])
                for g0 in range(0, ncalls, GB):
                    gb = min(GB, ncalls - g0)
                    gt = gathp.tile([128, GB * CCH, 8], f32, tag="gc",
                                    name="gc")
                    for j in range(gb):
                        g = g0 + j
                        sl = slice(g * 64, (g + 1) * 64)
                        raw_gather(
                            nc.gpsimd, gt[:, j * CCH:(j + 1) * CCH, :],
                            XC2[:, 0:8], idxc_sb[:, sl], CALL, 8, 64,
                            queue_num=g % 4)
                    psel(gt, mcol, ccall,
                         slice(g0 * CCH, (g0 + gb) * CCH), gb * CCH)

                # ---- learner algebra (f32, [128, NCA]) ----
                def wt(tag):
                    return widep.tile([128, nca], f32, tag=tag, name=tag)
                rc, cc = rcall, ccall
                ta, tb = wt("ta"), wt("tb")
                nc.vector.tensor_add(ta[:, :], rc[:, :, 0], cc[:, :, 1])
                nc.vector.tensor_add(tb[:, :], cc[:, :, 0], rc[:, :, 1])
                af, ab = wt("af"), wt("ab")
                nc.scalar.activation(af[:, :], ta[:, :], AF.Tanh)
                nc.scalar.activation(ab[:, :], tb[:, :], AF.Tanh)
                nc.vector.tensor_add(ta[:, :], rc[:, :, 2], cc[:, :, 3])
                nc.vector.tensor_add(tb[:, :], cc[:, :, 2], rc[:, :, 3])
                u1, u2 = wt("u1"), wt("u2")
                nc.scalar.activation(u1[:, :], ta[:, :], AF.Tanh, scale=0.5)
                nc.scalar.activation(u2[:, :], tb[:, :], AF.Tanh, scale=0.5)
                w2e, t1, t2 = wt("w2e"), wt("t1"), wt("t2")
                nc.vector.tensor_mul(t1[:, :], u1[:, :], u2[:, :])
                nc.vector.tensor_add(t2[:, :], u1[:, :], u2[:, :])
                nc.vector.tensor_add(t1[:, :], t1[:, :], t2[:, :])
                nc.vector.tensor_scalar(w2e[:, :], t1[:, :], 0.25, 0.25,
                                        ALU.mult, ALU.add)
                nc.vector.tensor_mul(w2e[:, :], w2e[:, :], w2e[:, :])
                A2, R2 = wt("A2"), wt("R2")
                nc.vector.tensor_mul(A2[:, :], af[:, :], af[:, :])
                nc.vector.tensor_mul(R2[:, :], ab[:, :], ab[:, :])
                de, dr = wt("de"), wt("dr")
                nc.vector.tensor_scalar(de[:, :], A2[:, :], 1.0, None, ALU.add)
                nc.vector.reciprocal(de[:, :], de[:, :])
                nc.vector.tensor_scalar(dr[:, :], R2[:, :], 1.0, None, ALU.add)
                nc.vector.reciprocal(dr[:, :], dr[:, :])
                ce, se, cr, sr = wt("ta"), wt("tb"), wt("u1"), wt("u2")
                nc.vector.tensor_scalar(t1[:, :], A2[:, :], -1.0, 1.0,
                                        ALU.mult, ALU.add)
                nc.vector.tensor_mul(ce[:, :], t1[:, :], de[:, :])
                nc.vector.tensor_scalar(t1[:, :], af[:, :], 2.0, None, ALU.mult)
                nc.vector.tensor_mul(se[:, :], t1[:, :], de[:, :])
                nc.vector.tensor_scalar(t1[:, :], R2[:, :], -1.0, 1.0,
                                        ALU.mult, ALU.add)
                nc.vector.tensor_mul(cr[:, :], t1[:, :], dr[:, :])
                nc.vector.tensor_scalar(t1[:, :], ab[:, :], 2.0, None, ALU.mult)
                nc.vector.tensor_mul(sr[:, :], t1[:, :], dr[:, :])
                c_e, s_e = wt("A2"), wt("R2")
                nc.vector.tensor_mul(t1[:, :], ce[:, :], cr[:, :])
                nc.vector.tensor_mul(t2[:, :], se[:, :], sr[:, :])
                nc.vector.tensor_add(c_e[:, :], t1[:, :], t2[:, :])
                nc.vector.tensor_mul(t1[:, :], sr[:, :], ce[:, :])
                nc.vector.tensor_mul(t2[:, :], se[:, :], cr[:, :])
                nc.vector.tensor_sub(s_e[:, :], t1[:, :], t2[:, :])
                # rotation coefs in bf16 (w2 folded in)
                nc.vector.tensor_mul(c2b[:, :], c_e[:, :], w2e[:, :])
                nc.vector.tensor_mul(s2b[:, :], s_e[:, :], w2e[:, :])
                nc.vector.tensor_copy(w2b[:, :], w2e[:, :])

                # ---- deg (on-chip one-hot matmuls, single PSUM pass) ----
                degt = psp.tile([128, 128], f32, tag="mm", name="deg")
                degP = degt[:, 0:NW]
                for g0 in range(0, nca, CCH):
                    selw = selp.tile([128, CCH, 128], bf16, tag="selw",
                                     name="selw")
                    nc.vector.tensor_tensor(
                        selw[:, :, :],
                        iota_b[:, :].unsqueeze(1).broadcast_to(
                            [128, CCH, 128]),
                        rloc_b[:, g0:g0 + CCH].unsqueeze(2).broadcast_to(
                            [128, CCH, 128]),
                        ALU.is_equal)
                    for j in range(CCH):
                        (k, reg, w, st, sp) = winchunks[g0 + j]
                        nc.tensor.matmul(degP[:, w:w + 1], selw[:, j, :],
                                         w2b[:, k:k + 1], start=st, stop=sp)
                deg = wt("ta")
                nc.vector.tensor_copy(deg[:, 0:NW], degP)
                nc.vector.tensor_scalar(diag_sh[:, :], deg[:, 0:NW], 1e30, 1.0,
                                        ALU.mult, ALU.min)
                nc.vector.tensor_scalar(deg[:, 0:NW], deg[:, 0:NW], 1e-30,
                                        None, ALU.max)
                rrec = wt("tb")
                nc.vector.reciprocal(rrec[:, 0:NW], deg[:, 0:NW])
                nc.scalar.activation(dinv_sh[:, :], rrec[:, 0:NW], AF.Sqrt)
                ny = wt("u1")
                nc.vector.tensor_mul(ny[:, 0:NW], dinv_sh[:, :], dinv_sh[:, :])
                nc.vector.tensor_mul(ny[:, 0:NW], ny[:, 0:NW], deg[:, 0:NW])
                nc.vector.tensor_scalar(ny[:, 0:NW], ny[:, 0:NW], -0.5, 1.5,
                                        ALU.mult, ALU.add)
                nc.vector.tensor_mul(dinv_sh[:, :], dinv_sh[:, :], ny[:, 0:NW])
                nc.vector.tensor_mul(dinv_sh[:, :], dinv_sh[:, :],
                                     diag_sh[:, :])

                # ---- feature table: dinv * (Wl (x) Wr) xc (q precomputed in
                # z1a; Wr folded in - the stalk rotation commutes with it),
                # written 2-node-packed into xfs2 rows (node pair per 512B) --
                for t in range(NW):
                    nc.scalar.activation(xfN[:, t, 0:F], z1a[:, t, 0:F],
                                         AF.Identity,
                                         scale=dinv_sh[:, t:t + 1])
                xf2lo = xfs2[:, 0:F].rearrange("(w p2) f -> p2 w f", p2=64)
                xf2hi = xfs2[:, F:2 * F].rearrange("(w p2) f -> p2 w f",
                                                   p2=64)
                nc.sync.dma_start(xf2lo[:, :, :], xfN[0:64, :, :])
                nc.sync.dma_start(xf2hi[:, :, :], xfN[64:128, :, :])

                nc.gpsimd.collective_compute(
                    "AllGather", ALU.bypass, replica_groups=RG,
                    ins=[xfs2[:]], outs=[XF2[:]])

                # ---- phase 4: x-update per window (fused into messages) ----
                def phase4_win(t, aggP):
                    z1 = workp.tile([128, F], f32, tag="z1")
                    z2 = workp.tile([128, F], f32, tag="z2")
                    nc.scalar.activation(z2[:, :], aggP, AF.Identity,
                                         scale=dinv_sh[:, t:t + 1])
                    z1s = workp.tile([128, F], f32, tag="z1s")
                    nc.scalar.activation(z1s[:, :], z1a[:, t, 0:F],
                                         AF.Identity,
                                         scale=diag_sh[:, t:t + 1])
                    nc.vector.tensor_sub(z1[:, :], z1s[:, :], z2[:, :])
                    ez = workp.tile([128, F], f32, tag="ez")
                    nc.scalar.activation(ez[:, :], z1[:, :], AF.Exp)
                    nc.vector.tensor_scalar(ez[:, :], ez[:, :], 1.0, -1.0,
                                            ALU.min, ALU.add)
                    rz = workp.tile([128, F], f32, tag="rz")
                    nc.scalar.activation(rz[:, :], z1[:, :], AF.Relu)
                    nc.vector.tensor_add(ez[:, :], ez[:, :], rz[:, :])
                    for i in range(FD):
                        blk = slice(i * HID, (i + 1) * HID)
                        cf = cfb[:, L * FD + i:L * FD + i + 1]
                        nc.vector.tensor_scalar(slabT[:, t, blk],
                                                slabT[:, t, blk], cf, None,
                                                ALU.mult)
                    nc.vector.tensor_sub(slabT[:, t, 0:F], slabT[:, t, 0:F],
                                         ez[:, :])
                    if L + 1 < NL:
                        xpT = tpose(slabT[:, t, 0:F], 128, F)
                        qcp = ps2p.tile([128, 128], f32, tag="m2")
                        nc.tensor.matmul(qcp[:, 0:QC], xpT[:F, :128],
                                         wqm[:, (L + 1) * QC:(L + 2) * QC],
                                         start=True, stop=True)
                        nc.vector.tensor_copy(ctile[:, t, :], qcp[:, 0:4])
                        nc.vector.tensor_copy(ctb[:, t, :], qcp[:, 0:4])
                        nc.vector.tensor_copy(z1a[:, t, 0:F], qcp[:, 4:QC])
                def lin2_win(t):
                    xT = tpose(slabT[:, t, 0:F], 128, F)
                    op = ps1p.tile([128, 128], f32, tag="tp")
                    nc.tensor.matmul(op[:OUT_CH, :128], w2t[:, :],
                                     xT[:F, :128], start=True, stop=True)
                    ob = workp.tile([OUT_CH, 128], f32, tag="l2ob")
                    nc.scalar.activation(ob[:, :], op[:OUT_CH, :128],
                                         AF.Identity, bias=b2sb[:, :])
                    oN = ps1p.tile([128, 128], f32, tag="tp")
                    nc.tensor.transpose(oN[:128, :OUT_CH], ob[:, :128],
                                        ident[:OUT_CH, :OUT_CH])
                    os_ = workp.tile([128, OUT_CH], f32, tag="l2os")
                    nc.vector.tensor_copy(os_[:, :], oN[:128, :OUT_CH])
                    nc.sync.dma_start(out_d[t * 128:(t + 1) * 128, :],
                                      os_[:, :])


                # ---- messages: gather + parity-select + rotate + aggregate --
                aggP = None
                cur = None
                for ci in range(ncalls):
                    k0 = ci * CCH
                    gf2 = feap.tile([128, CCH, 2 * F], bf16, tag="gf",
                                    name="gf")
                    raw_gather(
                        nc.gpsimd, gf2[:, :, :], XF2[:, 0:2 * F],
                        idxf_sb[:, ci * 64:(ci + 1) * 64], CALL, 2 * F, 256,
                        queue_num=ci % 4)
                    ksl = slice(k0, k0 + CCH)
                    # exact bf16 parity select: lo*(1-m) + hi*m (m in {0,1}
                    # so every product and the sum are exact)
                    gf = feap.tile([128, CCH, F], bf16, tag="gfs", name="gfs")
                    gft = feap.tile([128, CCH, F], bf16, tag="gft", name="gft")
                    nc.vector.tensor_mul(
                        gf[:, :, :], gf2[:, :, 0:F],
                        mcolbi[:, ksl].unsqueeze(2).broadcast_to([128, CCH, F]))
                    nc.vector.tensor_mul(
                        gft[:, :, :], gf2[:, :, F:2 * F],
                        mcolb[:, ksl].unsqueeze(2).broadcast_to([128, CCH, F]))
                    nc.vector.tensor_add(gf[:, :, :], gf[:, :, :],
                                         gft[:, :, :])
                    msg = msgp.tile([128, CCH, F], bf16, tag="msg", name="msg")
                    c2r = c2b[:, ksl].unsqueeze(2).broadcast_to([128, CCH, HID])
                    s2r = s2b[:, ksl].unsqueeze(2).broadcast_to([128, CCH, HID])
                    w2r = w2b[:, ksl].unsqueeze(2).broadcast_to([128, CCH, HID])
                    g0b = gf[:, :, 0:HID]
                    g1b = gf[:, :, HID:2 * HID]
                    g2b = gf[:, :, 2 * HID:3 * HID]
                    tA = msgp.tile([128, CCH, HID], bf16, tag="tA", name="tA")
                    tB = msgp.tile([128, CCH, HID], bf16, tag="tB", name="tB")
                    nc.vector.tensor_mul(tA[:, :, :], g0b, c2r)
                    nc.vector.tensor_mul(tB[:, :, :], g1b, s2r)
                    nc.vector.tensor_sub(msg[:, :, 0:HID], tA[:, :, :],
                                         tB[:, :, :])
                    nc.vector.tensor_mul(tA[:, :, :], g0b, s2r)
                    nc.vector.tensor_mul(tB[:, :, :], g1b, c2r)
                    nc.vector.tensor_add(msg[:, :, HID:2 * HID], tA[:, :, :],
                                         tB[:, :, :])
                    nc.vector.tensor_mul(msg[:, :, 2 * HID:3 * HID], g2b, w2r)
                    selw = selp.tile([128, CCH, 128], bf16, tag="selw",
                                     name="selw")
                    nc.vector.tensor_tensor(
                        selw[:, :, :],
                        iota_b[:, :].unsqueeze(1).broadcast_to(
                            [128, CCH, 128]),
                        rloc_b[:, k0:k0 + CCH].unsqueeze(2).broadcast_to(
                            [128, CCH, 128]),
                        ALU.is_equal)
                    for j in range(CCH):
                        (k, reg, w, st, sp) = winchunks[k0 + j]
                        if st:
                            aggPt = psp.tile([128, 128], f32, tag="mm")
                            aggP = aggPt[:, 0:F]
                            cur = (reg, w)
                        assert cur == (reg, w)
                        nc.tensor.matmul(aggP, selw[:, j, :], msg[:, j, :],
                                         start=st, stop=sp)
                        if sp:
                            phase4_win(w, aggP)
                            if L + 1 == NL:
                                lin2_win(w)

                if L + 1 < NL:
                    contrib_write(L + 1)

    nc.compile()
    return nc


def kernel(x, edge_index, W1, b1, W2, b2, W_left, W_right, eps,
           W_sheaf, W_wt):
    from concourse.bass_utils import run_bass_kernel_spmd
    in_maps, pad_id = _host_prep(x, edge_index, W1, b1, W2, b2, W_left,
                                 W_right, eps, W_sheaf, W_wt)
    plan = _CACHE['plan']
    key = (plan['Q'],)
    if _CACHE.get('key') != key:
        _CACHE['nc'] = _build_program(plan)
        _CACHE['key'] = key
    nc = _CACHE['nc']
    res = run_bass_kernel_spmd(nc, in_maps, list(range(NCORES)))
    full = np.concatenate([res.results[c]["out"] for c in range(NCORES)],
                          axis=0)
    return full[pad_id].astype(np.float32)



# revision 61
# speedup vs baseline: 5.2520x; 1.0030x over previous
"""Trainium2 Bass kernel for DiscreteBundleSheafDiffusion (D=2, FD=3, HID=32).

Redesign vs baseline: all per-edge gathers go through batched dma_gather
(1024 indices per call, int16 wrapped+replicated index tables) instead of
canonical [128,1] indirect DMAs; one-hot row-selection matrices for the
segment-sum matmuls are generated on-chip per chunk via tensor_scalar
is_equal (4x DVE mode) instead of streamed from HBM; the gathered feature
table holds dinv[v] * (Wl (x) I) xc[v] in bf16 (so per-edge work is only a
2D rotation + w2 scale, and the dinv AllGather disappears); aggregation
matmuls run in bf16 with f32 PSUM accumulate.

Tables: contribs (4 sheaf/weight projections per node) live 2-node-packed
in XC2 [NPAD/2, 64] f32 (256B rows, parity-selected after gather, index =
node>>1 fits int16); features live in XF [NPAD, 128] bf16 (256B rows) with
chunks class-sorted by col < 32768 (LO) vs >= 32768 (HI) so gathers address
XF[0:HALF] / XF[HALF:] with int16 indices. Per-window chunk capacities
QL/QH are data-derived maxima, uniform across cores (single SPMD program).
"""
import sys
sys.path.insert(0, '/opt/trn_rl_repo')
import numpy as np

N_NODES = 50000
E0 = 200000
IN_CH = 128
OUT_CH = 32
N_LAYERS = 2
FD, HID = 3, 32
F = FD * HID
NCORES = 8
SHR = 6250
SH = 6272
NW = SH // 128
NPAD = NCORES * SH
CALL = 1024           # indices per dma_gather call
CCH = CALL // 128     # chunks per feature/contrib call (8)

_CACHE = {}


def _spectral_normalize_np(W, iters=20):
    W = np.asarray(W, np.float32)
    u = np.full((W.shape[0],), 1.0 / np.sqrt(W.shape[0]), np.float32)
    for _ in range(iters):
        v = W.T @ u
        v = v / (np.linalg.norm(v) + np.float32(1e-12))
        u2 = W @ v
        u = u2 / (np.linalg.norm(u2) + np.float32(1e-12))
    v = W.T @ u
    v = v / (np.linalg.norm(v) + np.float32(1e-12))
    sigma = u @ W @ v
    return W / sigma


def _wrap_calls(seq2d):
    """seq2d: [ncalls, 1024] int -> [128, ncalls*64] int16 wrapped+replicated."""
    ncalls = seq2d.shape[0]
    out = np.zeros((128, ncalls * 64), np.int16)
    for j in range(ncalls):
        w = seq2d[j].reshape(64, 16).T.astype(np.int16)   # [16, 64]
        out[:, j * 64:(j + 1) * 64] = np.tile(w, (8, 1))
    return out


def _plan_chunks(edge_index):
    """Window-sorted single-class chunk packing; returns plan + per-core
    edge arrays. Capacity Q per window is the max over cores/windows so a
    single SPMD program covers every core (short windows pad with invalid
    slots: rloc = -1 -> all-zero sel columns)."""
    ei = np.asarray(edge_index)
    row = ei[0].astype(np.int64)
    col = ei[1].astype(np.int64)
    n_ids = np.arange(N_NODES)
    pad_id = (n_ids // SHR) * SH + (n_ids % SHR)
    rowp = pad_id[row]
    colp = pad_id[col]

    cores = []
    q = 0
    for c in range(NCORES):
        m = (rowp // SH) == c
        r = (rowp[m] - c * SH).astype(np.int64)
        cl = colp[m].astype(np.int64)
        order = np.lexsort((cl, r))
        r, cl = r[order], cl[order]
        w = r // 128
        for ww in range(NW):
            cnt = int((w == ww).sum())
            q = max(q, (cnt + 127) // 128)
        cores.append((r, cl))

    nca = -(-NW * q // CCH) * CCH
    plan = dict(Q=q, NCA=nca)

    # chunk k -> (window, start, stop) in k order; padding chunks land in
    # the last window (all-invalid, contribute zeros).
    winchunks = []
    for k in range(nca):
        w = min(k // q, NW - 1)
        k0 = w * q
        k1 = nca if w == NW - 1 else (w + 1) * q
        winchunks.append((k, 0, w, k == k0, k == k1 - 1))
    plan['winchunks'] = winchunks
    return plan, cores, pad_id


def _host_prep(x, edge_index, W1, b1, W2, b2, W_left, W_right, eps,
               W_sheaf, W_wt):
    plan, cores, pad_id = _plan_chunks(edge_index)
    _CACHE['plan'] = plan
    q, nca = plan['Q'], plan['NCA']
    x = np.asarray(x, np.float32)

    in_maps = []
    for c in range(NCORES):
        r, cl = cores[c]
        colp_arr = np.zeros((nca, 128), np.int64)      # global padded col id
        rloc = np.full((nca, 128), -1, np.int64)
        valid = np.zeros((nca, 128), bool)
        w = r // 128
        for ww in range(NW):
            msel = (w == ww)
            rw, cw = r[msel], cl[msel]
            cnt = rw.shape[0]
            assert cnt <= q * 128, f"window overflow {cnt} > {q * 128}"
            base = ww * q
            for qq in range((cnt + 127) // 128):
                a, b = qq * 128, min(qq * 128 + 128, cnt)
                k = base + qq
                colp_arr[k, :b - a] = cw[a:b]
                rloc[k, :b - a] = rw[a:b] - ww * 128
                valid[k, :b - a] = True

        # contrib gather: 2-node XC2 rows keyed by col>>1 (pair 2k,2k+1);
        # feature gather: 2-node XF2 rows keyed by (core, win, p%64)
        # (pair p, p+64 within a window - DMA-expressible write pattern)
        ncalls = nca // CCH
        iC = (colp_arr >> 1).reshape(ncalls, CALL)
        idxc_w = _wrap_calls(iC)
        core_of = colp_arr // SH
        lr_of = colp_arr % SH
        w_of = lr_of // 128
        p_of = lr_of % 128
        iF = core_of * (SH // 2) + w_of * 64 + (p_of % 64)
        iF[~valid] = 0
        idxf_w = _wrap_calls(iF.reshape(ncalls, CALL))
        import ml_dtypes as _mldt
        mcolT = (colp_arr & 1).T.astype(np.float32).copy()   # [128, NCA]
        mcolb = (p_of >= 64).T.astype(_mldt.bfloat16)        # feature parity
        mcolbi = (p_of < 64).T.astype(_mldt.bfloat16)        # 1 - parity
        rloc_b = rloc.T.astype(_mldt.bfloat16).copy()         # [128, NCA]
        # transposed one-hots for on-chip row-contrib expansion:
        # selnT[n, k*128+j] = 1 iff rloc[k, j] == n  (layer-independent)
        selnT = np.equal.outer(
            np.arange(128, dtype=np.int64), rloc).astype(
                _mldt.bfloat16).reshape(128, nca * 128)
        in_maps.append({
            "idxc_w": idxc_w, "idxf_w": idxf_w, "mcolT": mcolT,
            "mcolb": mcolb, "mcolbi": mcolbi, "rloc_b": rloc_b,
            "selnT": selnT,
        })

    import ml_dtypes
    W1 = np.asarray(W1, np.float32); b1 = np.asarray(b1, np.float32)
    W2 = np.asarray(W2, np.float32); b2 = np.asarray(b2, np.float32)
    NL = N_LAYERS
    QC = 4 + F   # contrib (4) + q = (Wl (x) Wr) x (F) per layer
    wqm = np.zeros((F, NL * QC), np.float32)
    cfb = np.zeros((128, NL * FD), np.float32)
    for l in range(NL):
        sh_row = np.asarray(W_sheaf[l][1], np.float32)
        wt_row = np.asarray(W_wt[l][0], np.float32)
        wqm[:, l * QC + 0] = sh_row[:F]
        wqm[:, l * QC + 1] = sh_row[F:]
        wqm[:, l * QC + 2] = wt_row[:F]
        wqm[:, l * QC + 3] = wt_row[F:]
        Wl = _spectral_normalize_np(np.asarray(W_left[l], np.float32))
        Wr = _spectral_normalize_np(np.asarray(W_right[l], np.float32))
        wqm[:, l * QC + 4:(l + 1) * QC] = \
            np.kron(Wl, Wr).astype(np.float32).T
        cfb[:, l * FD:(l + 1) * FD] = \
            (1.0 + np.tanh(np.asarray(eps[l], np.float32))).reshape(1, FD)

    xp = np.zeros((NPAD, IN_CH), np.float32)
    xp[pad_id] = x
    iota_b = np.tile(np.arange(128, dtype=np.float32)[None, :],
                     (128, 1)).astype(ml_dtypes.bfloat16)
    shared = {
        "w1t": W1.T.copy(), "b1f": b1.reshape(F, 1).copy(),
        "w2t": W2.T.copy(), "b2": b2.reshape(OUT_CH, 1).copy(),
        "wqm": wqm, "cfb": cfb,
        "iota_b": iota_b, "ident": np.eye(128, dtype=np.float32),
    }
    for c in range(NCORES):
        in_maps[c]["x_sh"] = xp[c * SH:(c + 1) * SH].T.copy()
        in_maps[c].update(shared)
    return in_maps, pad_id


# =================== bass program ===================
def _build_program(plan):
    import concourse.bacc as bacc
    import concourse.bass as bass
    import concourse.mybir as mybir
    from concourse import tile

    nca = plan['NCA']
    winchunks = plan['winchunks']
    ncalls = nca // CCH
    NL = N_LAYERS
    f32 = mybir.dt.float32
    bf16 = mybir.dt.bfloat16
    i16 = mybir.dt.int16
    AF = mybir.ActivationFunctionType
    ALU = mybir.AluOpType

    nc = bacc.Bacc("TRN2", target_bir_lowering=False, debug=False,
                   num_swdge_queues=4)

    def raw_gather(gps, out_ap, in_ap, idxs_ap, num_idxs, elem_size,
                   elem_step, queue_num=0):
        """dma_gather allowing elem_size < 256B (row stride must be %256B)."""
        stride_bytes = elem_step * mybir.dt.size(in_ap.dtype)
        assert stride_bytes % 256 == 0
        assert in_ap.ap[0][0] == elem_step
        assert in_ap.ap[-1][1] == out_ap.ap[-1][1] == elem_size
        _in_ap = gps.lower_ap_dma(in_ap, for_custom_bir_dma=True)
        _idxs_ap = gps.lower_ap(idxs_ap)
        _out_ap = gps.lower_ap(out_ap)
        return gps.add_instruction(
            mybir.InstDMAGatherAnt(
                name=gps.bass.get_next_instruction_name(),
                ins=[*_in_ap, _idxs_ap,
                     gps.lower_val_access(gps.to_reg(num_idxs))],
                outs=[_out_ap],
                transpose=False,
                num_idxs=num_idxs,
                elem_size=elem_size,
                stride_bytes_256=stride_bytes // 256,
                gen_mode=0,
                single_packet=True,
                queue_num=queue_num,
                sbuf_tokens_per_rank=0,
                sbuf_free_dim_per_rank=0,
                sbuf_free_dim_pad_per_rank=0,
                sbuf_byte_offset=0,
            ))


    x_sh = nc.dram_tensor("x_sh", [IN_CH, SH], f32, kind="ExternalInput").ap()
    idxc_d = nc.dram_tensor("idxc_w", [128, ncalls * 64], i16, kind="ExternalInput").ap()
    idxf_d = nc.dram_tensor("idxf_w", [128, ncalls * 64], i16, kind="ExternalInput").ap()
    selnT_d = nc.dram_tensor("selnT", [128, nca * 128], bf16, kind="ExternalInput").ap()
    mcol_d = nc.dram_tensor("mcolT", [128, nca], f32, kind="ExternalInput").ap()
    mcolb_d = nc.dram_tensor("mcolb", [128, nca], bf16, kind="ExternalInput").ap()
    mcolbi_d = nc.dram_tensor("mcolbi", [128, nca], bf16, kind="ExternalInput").ap()
    rloc_d = nc.dram_tensor("rloc_b", [128, nca], bf16, kind="ExternalInput").ap()
    iota_d = nc.dram_tensor("iota_b", [128, 128], bf16, kind="ExternalInput").ap()
    w1t_d = nc.dram_tensor("w1t", [IN_CH, F], f32, kind="ExternalInput").ap()
    b1f_d = nc.dram_tensor("b1f", [F, 1], f32, kind="ExternalInput").ap()
    w2t_d = nc.dram_tensor("w2t", [F, OUT_CH], f32, kind="ExternalInput").ap()
    b2_d = nc.dram_tensor("b2", [OUT_CH, 1], f32, kind="ExternalInput").ap()
    QC = 4 + F
    wqm_d = nc.dram_tensor("wqm", [F, NL * QC], f32, kind="ExternalInput").ap()
    cfb_d = nc.dram_tensor("cfb", [128, NL * FD], f32, kind="ExternalInput").ap()
    ident_d = nc.dram_tensor("ident", [128, 128], f32, kind="ExternalInput").ap()
    out_d = nc.dram_tensor("out", [SH, OUT_CH], f32, kind="ExternalOutput").ap()

    xcs = nc.dram_tensor("xcs", [SH, 4], f32)
    XCFULL = nc.dram_tensor("XCFULL", [NPAD, 4], f32, addr_space="Shared")
    XC2 = nc.dram_tensor("XC2", [NPAD // 2, 64], f32)
    xfs2 = nc.dram_tensor("xfs2", [SH // 2, 256], bf16)
    XF2 = nc.dram_tensor("XF2", [NPAD // 2, 256], bf16, addr_space="Shared")
    RG = [list(range(NCORES))]

    with tile.TileContext(nc) as tc:
        with tc.tile_pool(name="const", bufs=1) as constp, \
             tc.tile_pool(name="big", bufs=1) as bigp, \
             tc.tile_pool(name="wide", bufs=1) as widep, \
             tc.tile_pool(name="gath", bufs=3) as gathp, \
             tc.tile_pool(name="fea", bufs=4) as feap, \
             tc.tile_pool(name="selp", bufs=6) as selp, \
             tc.tile_pool(name="seln", bufs=3) as selnp, \
             tc.tile_pool(name="work", bufs=3) as workp, \
             tc.tile_pool(name="msgp", bufs=4) as msgp, \
             tc.tile_pool(name="ps", bufs=2, space="PSUM") as psp, \
             tc.tile_pool(name="ps2", bufs=2, space="PSUM") as ps2p, \
             tc.tile_pool(name="ps1", bufs=4, space="PSUM") as ps1p:

            def C(name, shape, src, dt=f32):
                t = constp.tile(shape, dt, tag=name, name=name)
                nc.sync.dma_start(t[:], src)
                return t

            ident = C("ident", [128, 128], ident_d[:])
            iota_b = C("iota", [128, 128], iota_d[:], dt=bf16)
            w1t = C("w1t", [IN_CH, F], w1t_d[:])
            b1f = C("b1f", [F, 1], b1f_d[:])
            w2t = C("w2t", [F, OUT_CH], w2t_d[:])
            b2sb = C("b2", [OUT_CH, 1], b2_d[:])
            wqm = C("wqm", [F, NL * QC], wqm_d[:])
            cfb = C("cfb", [128, NL * FD], cfb_d[:])
            mcol = C("mcol", [128, nca], mcol_d[:])
            mcolb = C("mcolb", [128, nca], mcolb_d[:], dt=bf16)
            mcolbi = C("mcolbi", [128, nca], mcolbi_d[:], dt=bf16)
            rloc_b = C("rloc", [128, nca], rloc_d[:], dt=bf16)
            idxc_sb = C("idxc", [128, ncalls * 64], idxc_d[:], dt=i16)
            idxf_sb = C("idxf", [128, ncalls * 64], idxf_d[:], dt=i16)

            slabT = bigp.tile([128, NW, F], f32, tag="slabT")
            ctile = bigp.tile([128, NW, 4], f32, tag="ctile")
            ctb = bigp.tile([128, NW, 4], bf16, tag="ctb")
            xfN = bigp.tile([128, NW, F], bf16, tag="xfN")
            z1a = bigp.tile([128, NW, F], f32, tag="z1a")
            dinv_sh = bigp.tile([128, NW], f32, tag="dinvsh")
            diag_sh = bigp.tile([128, NW], f32, tag="diagsh")
            ccall = bigp.tile([128, nca, 4], f32, tag="ccall")
            rcall = bigp.tile([128, nca, 4], f32, tag="rcall")
            c2b = bigp.tile([128, nca], bf16, tag="c2b")
            s2b = bigp.tile([128, nca], bf16, tag="s2b")
            w2b = bigp.tile([128, nca], bf16, tag="w2b")

            def tpose(src_ap, pdim, fdim, tag="tx"):
                pt = ps1p.tile([128, 128], f32, tag="tp", name="tp")
                nc.tensor.transpose(pt[:fdim, :pdim], src_ap,
                                    ident[:pdim, :pdim])
                dst = workp.tile([128, 128], f32, tag=tag, name=tag)
                nc.scalar.copy(dst[:fdim, :pdim], pt[:fdim, :pdim])
                return dst

            def contrib_write(L):
                """ctile -> xcs -> AllGather -> XCFULL -> expand into XC2."""
                nc.sync.dma_start(
                    xcs[:].rearrange("(w p) f -> p w f", p=128), ctile[:, :, :])
                nc.gpsimd.collective_compute(
                    "AllGather", ALU.bypass, replica_groups=RG,
                    ins=[xcs[:]], outs=[XCFULL[:]])
                ct2 = widep.tile([128, NPAD // 256, 8], f32, tag="ct2",
                                 name="ct2")
                nc.sync.dma_start(
                    ct2[:, :, :],
                    XCFULL[:].rearrange("(c p two) f -> p c (two f)",
                                        p=128, two=2))
                nc.sync.dma_start(
                    XC2[:, 0:8].rearrange("(c p) f8 -> p c f8", p=128),
                    ct2[:, :, :])

            # ---------------- lin1 + layer-0 contribs ----------------
            for t in range(NW):
                xT = workp.tile([128, 128], f32, tag="xt")
                nc.sync.dma_start(xT[:], x_sh[:, t * 128:(t + 1) * 128])
                hp = psp.tile([128, 128], f32, tag="mm")
                nc.tensor.matmul(hp[:F, :128], w1t[:], xT[:IN_CH, :128],
                                 start=True, stop=True)
                e1 = workp.tile([F, 128], f32, tag="e1")
                nc.scalar.activation(e1[:, :], hp[:F, :128], AF.Exp,
                                     bias=b1f[:, :])
                nc.vector.tensor_scalar(e1[:, :], e1[:, :], 1.0, -1.0,
                                        ALU.min, ALU.add)
                r1 = workp.tile([F, 128], f32, tag="r1")
                nc.scalar.activation(r1[:, :], hp[:F, :128], AF.Relu,
                                     bias=b1f[:, :])
                hF = workp.tile([F, 128], f32, tag="hF")
                nc.vector.tensor_add(hF[:, :], e1[:, :], r1[:, :])
                hN = ps1p.tile([128, 128], f32, tag="tp")
                nc.tensor.transpose(hN[:128, :F], hF[:, :128], ident[:F, :F])
                nc.vector.tensor_copy(slabT[:, t, 0:F], hN[:128, :F])
                # contribs + q = (Wl (x) Wr) x in one node-major matmul
                # (stationary = hF, contraction over F)
                qcp = ps2p.tile([128, 128], f32, tag="m2")
                nc.tensor.matmul(qcp[:, 0:QC], hF[:, :128], wqm[:, 0:QC],
                                 start=True, stop=True)
                nc.vector.tensor_copy(ctile[:, t, :], qcp[:, 0:4])
                nc.vector.tensor_copy(ctb[:, t, :], qcp[:, 0:4])
                nc.vector.tensor_copy(z1a[:, t, 0:F], qcp[:, 4:QC])
            contrib_write(0)

            # =================== layers ===================
            for L in range(NL):
                # ---- phase 1: contribs ----
                # row side: expand per-window ctile to edge slots via
                # streamed transposed one-hots (selnT) on the PE array —
                # depends only on local ctile, so it runs under the XC
                # AllGather + expand; col side gathers wait on XC2.
                GB = 4  # calls per select batch (32 chunks)

                def psel(gt, mt, dst, ks, nk):
                    d = msgp.tile([128, GB * CCH, 4], f32, tag="d4", name="d4")
                    nc.vector.tensor_sub(d[:, :nk, :], gt[:, :nk, 4:8],
                                         gt[:, :nk, 0:4])
                    nc.vector.tensor_mul(
                        d[:, :nk, :], d[:, :nk, :],
                        mt[:, ks].unsqueeze(2).broadcast_to([128, nk, 4]))
                    nc.vector.tensor_add(dst[:, ks, :], gt[:, :nk, 0:4],
                                         d[:, :nk, :])

                for g in range(ncalls):
                    selnw = selnp.tile([128, CALL], bf16, tag="selnw",
                                       name="selnw")
                    nc.sync.dma_start(selnw[:, :],
                                      selnT_d[:, g * CALL:(g + 1) * CALL])
                    rcp = ps1p.tile([128, 128], f32, tag="tp", name="rcp")
                    for j in range(CCH):
                        k = g * CCH + j
                        w = winchunks[k][2]
                        nc.tensor.matmul(rcp[:, j * 4:(j + 1) * 4],
                                         selnw[:, j * 128:(j + 1) * 128],
                                         ctb[:, w, :], start=True, stop=True)
                    nc.scalar.copy(rcall[:, g * CCH:(g + 1) * CCH, :],
                                   rcp[:, 0:32])
                # ---- learner algebra + deg, pipelined per ccall batch ----
                def wt(tag):
                    return widep.tile([128, nca], f32, tag=tag, name=tag)
                rc, cc = rcall, ccall
                ta, tb = wt("ta"), wt("tb")
                af, ab = wt("af"), wt("ab")
                u1, u2 = wt("u1"), wt("u2")
                w2e, t1, t2 = wt("w2e"), wt("t1"), wt("t2")
                A2, R2 = wt("A2"), wt("R2")
                de, dr = wt("de"), wt("dr")
                ce, se, cr, sr = ta, tb, u1, u2      # buffer reuse
                c_e, s_e = A2, R2
                degt = psp.tile([128, 128], f32, tag="mm", name="deg")
                degP = degt[:, 0:NW]

                def learner_deg_slice(k0s, k1s):
                    ks = slice(k0s, k1s)
                    nc.vector.tensor_add(ta[:, ks], rc[:, ks, 0], cc[:, ks, 1])
                    nc.vector.tensor_add(tb[:, ks], cc[:, ks, 0], rc[:, ks, 1])
                    nc.scalar.activation(af[:, ks], ta[:, ks], AF.Tanh)
                    nc.scalar.activation(ab[:, ks], tb[:, ks], AF.Tanh)
                    nc.vector.tensor_add(ta[:, ks], rc[:, ks, 2], cc[:, ks, 3])
                    nc.vector.tensor_add(tb[:, ks], cc[:, ks, 2], rc[:, ks, 3])
                    nc.scalar.activation(u1[:, ks], ta[:, ks], AF.Tanh,
                                         scale=0.5)
                    nc.scalar.activation(u2[:, ks], tb[:, ks], AF.Tanh,
                                         scale=0.5)
                    nc.vector.tensor_mul(t1[:, ks], u1[:, ks], u2[:, ks])
                    nc.vector.tensor_add(t2[:, ks], u1[:, ks], u2[:, ks])
                    nc.vector.tensor_add(t1[:, ks], t1[:, ks], t2[:, ks])
                    nc.vector.tensor_scalar(w2e[:, ks], t1[:, ks], 0.25, 0.25,
                                            ALU.mult, ALU.add)
                    nc.vector.tensor_mul(w2e[:, ks], w2e[:, ks], w2e[:, ks])
                    nc.vector.tensor_mul(A2[:, ks], af[:, ks], af[:, ks])
                    nc.vector.tensor_mul(R2[:, ks], ab[:, ks], ab[:, ks])
                    nc.vector.tensor_scalar(de[:, ks], A2[:, ks], 1.0, None,
                                            ALU.add)
                    nc.vector.reciprocal(de[:, ks], de[:, ks])
                    nc.vector.tensor_scalar(dr[:, ks], R2[:, ks], 1.0, None,
                                            ALU.add)
                    nc.vector.reciprocal(dr[:, ks], dr[:, ks])
                    nc.vector.tensor_scalar(t1[:, ks], A2[:, ks], -1.0, 1.0,
                                            ALU.mult, ALU.add)
                    nc.vector.tensor_mul(ce[:, ks], t1[:, ks], de[:, ks])
                    nc.vector.tensor_scalar(t1[:, ks], af[:, ks], 2.0, None,
                                            ALU.mult)
                    nc.vector.tensor_mul(se[:, ks], t1[:, ks], de[:, ks])
                    nc.vector.tensor_scalar(t1[:, ks], R2[:, ks], -1.0, 1.0,
                                            ALU.mult, ALU.add)
                    nc.vector.tensor_mul(cr[:, ks], t1[:, ks], dr[:, ks])
                    nc.vector.tensor_scalar(t1[:, ks], ab[:, ks], 2.0, None,
                                            ALU.mult)
                    nc.vector.tensor_mul(sr[:, ks], t1[:, ks], dr[:, ks])
                    nc.vector.tensor_mul(t1[:, ks], ce[:, ks], cr[:, ks])
                    nc.vector.tensor_mul(t2[:, ks], se[:, ks], sr[:, ks])
                    nc.vector.tensor_add(c_e[:, ks], t1[:, ks], t2[:, ks])
                    nc.vector.tensor_mul(t1[:, ks], sr[:, ks], ce[:, ks])
                    nc.vector.tensor_mul(t2[:, ks], se[:, ks], cr[:, ks])
                    nc.vector.tensor_sub(s_e[:, ks], t1[:, ks], t2[:, ks])
                    # rotation coefs in bf16 (w2 folded in)
                    nc.vector.tensor_mul(c2b[:, ks], c_e[:, ks], w2e[:, ks])
                    nc.vector.tensor_mul(s2b[:, ks], s_e[:, ks], w2e[:, ks])
                    nc.vector.tensor_copy(w2b[:, ks], w2e[:, ks])
                    for gg in range(k0s, k1s, CCH):
                        selw = selp.tile([128, CCH, 128], bf16, tag="selw",
                                         name="selw")
                        nc.vector.tensor_tensor(
                            selw[:, :, :],
                            iota_b[:, :].unsqueeze(1).broadcast_to(
                                [128, CCH, 128]),
                            rloc_b[:, gg:gg + CCH].unsqueeze(2).broadcast_to(
                                [128, CCH, 128]),
                            ALU.is_equal)
                        for j in range(CCH):
                            (k, reg, w, st, sp) = winchunks[gg + j]
                            nc.tensor.matmul(degP[:, w:w + 1], selw[:, j, :],
                                             w2b[:, k:k + 1],
                                             start=st, stop=sp)

                for g0 in range(0, ncalls, GB):
                    gb = min(GB, ncalls - g0)
                    gt = gathp.tile([128, GB * CCH, 8], f32, tag="gc",
                                    name="gc")
                    for j in range(gb):
                        g = g0 + j
                        sl = slice(g * 64, (g + 1) * 64)
                        raw_gather(
                            nc.gpsimd, gt[:, j * CCH:(j + 1) * CCH, :],
                            XC2[:, 0:8], idxc_sb[:, sl], CALL, 8, 64,
                            queue_num=g % 4)
                    psel(gt, mcol, ccall,
                         slice(g0 * CCH, (g0 + gb) * CCH), gb * CCH)
                    learner_deg_slice(g0 * CCH, (g0 + gb) * CCH)

                deg = wt("ta")
                nc.vector.tensor_copy(deg[:, 0:NW], degP)
                nc.vector.tensor_scalar(diag_sh[:, :], deg[:, 0:NW], 1e30, 1.0,
                                        ALU.mult, ALU.min)
                nc.vector.tensor_scalar(deg[:, 0:NW], deg[:, 0:NW], 1e-30,
                                        None, ALU.max)
                rrec = wt("tb")
                nc.vector.reciprocal(rrec[:, 0:NW], deg[:, 0:NW])
                nc.scalar.activation(dinv_sh[:, :], rrec[:, 0:NW], AF.Sqrt)
                ny = wt("u1")
                nc.vector.tensor_mul(ny[:, 0:NW], dinv_sh[:, :], dinv_sh[:, :])
                nc.vector.tensor_mul(ny[:, 0:NW], ny[:, 0:NW], deg[:, 0:NW])
                nc.vector.tensor_scalar(ny[:, 0:NW], ny[:, 0:NW], -0.5, 1.5,
                                        ALU.mult, ALU.add)
                nc.vector.tensor_mul(dinv_sh[:, :], dinv_sh[:, :], ny[:, 0:NW])
                nc.vector.tensor_mul(dinv_sh[:, :], dinv_sh[:, :],
                                     diag_sh[:, :])

                # ---- feature table: dinv * (Wl (x) Wr) xc (q precomputed in
                # z1a; Wr folded in - the stalk rotation commutes with it),
                # written 2-node-packed into xfs2 rows (node pair per 512B) --
                for t in range(NW):
                    nc.scalar.activation(xfN[:, t, 0:F], z1a[:, t, 0:F],
                                         AF.Identity,
                                         scale=dinv_sh[:, t:t + 1])
                xf2lo = xfs2[:, 0:F].rearrange("(w p2) f -> p2 w f", p2=64)
                xf2hi = xfs2[:, F:2 * F].rearrange("(w p2) f -> p2 w f",
                                                   p2=64)
                nc.sync.dma_start(xf2lo[:, :, :], xfN[0:64, :, :])
                nc.sync.dma_start(xf2hi[:, :, :], xfN[64:128, :, :])

                nc.gpsimd.collective_compute(
                    "AllGather", ALU.bypass, replica_groups=RG,
                    ins=[xfs2[:]], outs=[XF2[:]])

                # ---- phase 4: x-update per window (fused into messages) ----
                def phase4_win(t, aggP):
                    z1 = workp.tile([128, F], f32, tag="z1")
                    z2 = workp.tile([128, F], f32, tag="z2")
                    nc.scalar.activation(z2[:, :], aggP, AF.Identity,
                                         scale=dinv_sh[:, t:t + 1])
                    z1s = workp.tile([128, F], f32, tag="z1s")
                    nc.scalar.activation(z1s[:, :], z1a[:, t, 0:F],
                                         AF.Identity,
                                         scale=diag_sh[:, t:t + 1])
                    nc.vector.tensor_sub(z1[:, :], z1s[:, :], z2[:, :])
                    ez = workp.tile([128, F], f32, tag="ez")
                    nc.scalar.activation(ez[:, :], z1[:, :], AF.Exp)
                    nc.vector.tensor_scalar(ez[:, :], ez[:, :], 1.0, -1.0,
                                            ALU.min, ALU.add)
                    rz = workp.tile([128, F], f32, tag="rz")
                    nc.scalar.activation(rz[:, :], z1[:, :], AF.Relu)
                    nc.vector.tensor_add(ez[:, :], ez[:, :], rz[:, :])
                    for i in range(FD):
                        blk = slice(i * HID, (i + 1) * HID)
                        cf = cfb[:, L * FD + i:L * FD + i + 1]
                        nc.vector.tensor_scalar(slabT[:, t, blk],
                                                slabT[:, t, blk], cf, None,
                                                ALU.mult)
                    nc.vector.tensor_sub(slabT[:, t, 0:F], slabT[:, t, 0:F],
                                         ez[:, :])
                    if L + 1 < NL:
                        xpT = tpose(slabT[:, t, 0:F], 128, F)
                        qcp = ps2p.tile([128, 128], f32, tag="m2")
                        nc.tensor.matmul(qcp[:, 0:QC], xpT[:F, :128],
                                         wqm[:, (L + 1) * QC:(L + 2) * QC],
                                         start=True, stop=True)
                        nc.vector.tensor_copy(ctile[:, t, :], qcp[:, 0:4])
                        nc.vector.tensor_copy(ctb[:, t, :], qcp[:, 0:4])
                        nc.vector.tensor_copy(z1a[:, t, 0:F], qcp[:, 4:QC])
                def lin2_win(t):
                    xT = tpose(slabT[:, t, 0:F], 128, F)
                    op = ps1p.tile([128, 128], f32, tag="tp")
                    nc.tensor.matmul(op[:OUT_CH, :128], w2t[:, :],
                                     xT[:F, :128], start=True, stop=True)
                    ob = workp.tile([OUT_CH, 128], f32, tag="l2ob")
                    nc.scalar.activation(ob[:, :], op[:OUT_CH, :128],
                                         AF.Identity, bias=b2sb[:, :])
                    oN = ps1p.tile([128, 128], f32, tag="tp")
                    nc.tensor.transpose(oN[:128, :OUT_CH], ob[:, :128],
                                        ident[:OUT_CH, :OUT_CH])
                    os_ = workp.tile([128, OUT_CH], f32, tag="l2os")
                    nc.vector.tensor_copy(os_[:, :], oN[:128, :OUT_CH])
                    nc.sync.dma_start(out_d[t * 128:(t + 1) * 128, :],
                                      os_[:, :])


                # ---- messages: gather + parity-select + rotate + aggregate --
                aggP = None
                cur = None
                for ci in range(ncalls):
                    k0 = ci * CCH
                    gf2 = feap.tile([128, CCH, 2 * F], bf16, tag="gf",
                                    name="gf")
                    raw_gather(
                        nc.gpsimd, gf2[:, :, :], XF2[:, 0:2 * F],
                        idxf_sb[:, ci * 64:(ci + 1) * 64], CALL, 2 * F, 256,
                        queue_num=ci % 4)
                    ksl = slice(k0, k0 + CCH)
                    # exact bf16 parity select: lo*(1-m) + hi*m (m in {0,1}
                    # so every product and the sum are exact)
                    gf = feap.tile([128, CCH, F], bf16, tag="gfs", name="gfs")
                    gft = feap.tile([128, CCH, F], bf16, tag="gft", name="gft")
                    nc.vector.tensor_mul(
                        gf[:, :, :], gf2[:, :, 0:F],
                        mcolbi[:, ksl].unsqueeze(2).broadcast_to([128, CCH, F]))
                    nc.vector.tensor_mul(
                        gft[:, :, :], gf2[:, :, F:2 * F],
                        mcolb[:, ksl].unsqueeze(2).broadcast_to([128, CCH, F]))
                    nc.vector.tensor_add(gf[:, :, :], gf[:, :, :],
                                         gft[:, :, :])
                    msg = msgp.tile([128, CCH, F], bf16, tag="msg", name="msg")
                    c2r = c2b[:, ksl].unsqueeze(2).broadcast_to([128, CCH, HID])
                    s2r = s2b[:, ksl].unsqueeze(2).broadcast_to([128, CCH, HID])
                    w2r = w2b[:, ksl].unsqueeze(2).broadcast_to([128, CCH, HID])
                    g0b = gf[:, :, 0:HID]
                    g1b = gf[:, :, HID:2 * HID]
                    g2b = gf[:, :, 2 * HID:3 * HID]
                    tA = msgp.tile([128, CCH, HID], bf16, tag="tA", name="tA")
                    tB = msgp.tile([128, CCH, HID], bf16, tag="tB", name="tB")
                    nc.vector.tensor_mul(tA[:, :, :], g0b, c2r)
                    nc.vector.tensor_mul(tB[:, :, :], g1b, s2r)
                    nc.vector.tensor_sub(msg[:, :, 0:HID], tA[:, :, :],
                                         tB[:, :, :])
                    nc.vector.tensor_mul(tA[:, :, :], g0b, s2r)
                    nc.vector.tensor_mul(tB[:, :, :], g1b, c2r)
                    nc.vector.tensor_add(msg[:, :, HID:2 * HID], tA[:, :, :],
                                         tB[:, :, :])
                    nc.vector.tensor_mul(msg[:, :, 2 * HID:3 * HID], g2b, w2r)
                    selw = selp.tile([128, CCH, 128], bf16, tag="selw",
                                     name="selw")
                    nc.vector.tensor_tensor(
                        selw[:, :, :],
                        iota_b[:, :].unsqueeze(1).broadcast_to(
                            [128, CCH, 128]),
                        rloc_b[:, k0:k0 + CCH].unsqueeze(2).broadcast_to(
                            [128, CCH, 128]),
                        ALU.is_equal)
                    for j in range(CCH):
                        (k, reg, w, st, sp) = winchunks[k0 + j]
                        if st:
                            aggPt = psp.tile([128, 128], f32, tag="mm")
                            aggP = aggPt[:, 0:F]
                            cur = (reg, w)
                        assert cur == (reg, w)
                        nc.tensor.matmul(aggP, selw[:, j, :], msg[:, j, :],
                                         start=st, stop=sp)
                        if sp:
                            phase4_win(w, aggP)
                            if L + 1 == NL:
                                lin2_win(w)

                if L + 1 < NL:
                    contrib_write(L + 1)

    nc.compile()
    return nc


def kernel(x, edge_index, W1, b1, W2, b2, W_left, W_right, eps,
           W_sheaf, W_wt):
    from concourse.bass_utils import run_bass_kernel_spmd
    in_maps, pad_id = _host_prep(x, edge_index, W1, b1, W2, b2, W_left,
                                 W_right, eps, W_sheaf, W_wt)
    plan = _CACHE['plan']
    key = (plan['Q'],)
    if _CACHE.get('key') != key:
        _CACHE['nc'] = _build_program(plan)
        _CACHE['key'] = key
    nc = _CACHE['nc']
    res = run_bass_kernel_spmd(nc, in_maps, list(range(NCORES)))
    full = np.concatenate([res.results[c]["out"] for c in range(NCORES)],
                          axis=0)
    return full[pad_id].astype(np.float32)



# revision 73
# speedup vs baseline: 9.4769x; 1.8044x over previous
"""Trainium2 Bass kernel for DiscreteBundleSheafDiffusion (D=2, FD=3, HID=32).

Redesign vs baseline: all per-edge gathers go through batched dma_gather
(1024 indices per call, int16 wrapped+replicated index tables) instead of
canonical [128,1] indirect DMAs; one-hot row-selection matrices for the
segment-sum matmuls are generated on-chip per chunk via tensor_scalar
is_equal (4x DVE mode) instead of streamed from HBM; the gathered feature
table holds dinv[v] * (Wl (x) I) xc[v] in bf16 (so per-edge work is only a
2D rotation + w2 scale, and the dinv AllGather disappears); aggregation
matmuls run in bf16 with f32 PSUM accumulate.

Tables: contribs (4 sheaf/weight projections per node) live 2-node-packed
in XC2 [NPAD/2, 64] f32 (256B rows, parity-selected after gather, index =
node>>1 fits int16); features live in XF [NPAD, 128] bf16 (256B rows) with
chunks class-sorted by col < 32768 (LO) vs >= 32768 (HI) so gathers address
XF[0:HALF] / XF[HALF:] with int16 indices. Per-window chunk capacities
QL/QH are data-derived maxima, uniform across cores (single SPMD program).
"""
import sys
sys.path.insert(0, '/opt/trn_rl_repo')
import numpy as np

N_NODES = 50000
E0 = 200000
IN_CH = 128
OUT_CH = 32
N_LAYERS = 2
FD, HID = 3, 32
F = FD * HID
NCORES = 8
SHR = 6250
SH = 6272
NW = SH // 128
NPAD = NCORES * SH
CALL = 1024           # indices per dma_gather call (SWDGE ring caps ~1024)
CCH = CALL // 128     # chunks per feature/contrib call (8)
IW = CALL // 16       # wrapped idx columns per call

_CACHE = {}


def _spectral_normalize_np(W, iters=20):
    W = np.asarray(W, np.float32)
    u = np.full((W.shape[0],), 1.0 / np.sqrt(W.shape[0]), np.float32)
    for _ in range(iters):
        v = W.T @ u
        v = v / (np.linalg.norm(v) + np.float32(1e-12))
        u2 = W @ v
        u = u2 / (np.linalg.norm(u2) + np.float32(1e-12))
    v = W.T @ u
    v = v / (np.linalg.norm(v) + np.float32(1e-12))
    sigma = u @ W @ v
    return W / sigma


def _wrap_calls(seq2d):
    """seq2d: [ncalls, CALL] int -> [128, ncalls*IW] int16 wrapped+replicated."""
    ncalls = seq2d.shape[0]
    out = np.zeros((128, ncalls * IW), np.int16)
    for j in range(ncalls):
        w = seq2d[j].reshape(IW, 16).T.astype(np.int16)   # [16, IW]
        out[:, j * IW:(j + 1) * IW] = np.tile(w, (8, 1))
    return out


def _plan_chunks(edge_index):
    """Window-sorted single-class chunk packing; returns plan + per-core
    edge arrays. Capacity Q per window is the max over cores/windows so a
    single SPMD program covers every core (short windows pad with invalid
    slots: rloc = -1 -> all-zero sel columns)."""
    ei = np.asarray(edge_index)
    row = ei[0].astype(np.int64)
    col = ei[1].astype(np.int64)
    n_ids = np.arange(N_NODES)
    pad_id = (n_ids // SHR) * SH + (n_ids % SHR)
    rowp = pad_id[row]
    colp = pad_id[col]

    cores = []
    q = 0
    for c in range(NCORES):
        m = (rowp // SH) == c
        r = (rowp[m] - c * SH).astype(np.int64)
        cl = colp[m].astype(np.int64)
        order = np.lexsort((cl, r))
        r, cl = r[order], cl[order]
        w = r // 128
        for ww in range(NW):
            cnt = int((w == ww).sum())
            q = max(q, (cnt + 127) // 128)
        cores.append((r, cl))

    nca = -(-NW * q // CCH) * CCH
    plan = dict(Q=q, NCA=nca)

    # chunk k -> (window, start, stop) in k order; padding chunks land in
    # the last window (all-invalid, contribute zeros).
    winchunks = []
    for k in range(nca):
        w = min(k // q, NW - 1)
        k0 = w * q
        k1 = nca if w == NW - 1 else (w + 1) * q
        winchunks.append((k, 0, w, k == k0, k == k1 - 1))
    plan['winchunks'] = winchunks
    return plan, cores, pad_id


def _host_prep(x, edge_index, W1, b1, W2, b2, W_left, W_right, eps,
               W_sheaf, W_wt):
    plan, cores, pad_id = _plan_chunks(edge_index)
    _CACHE['plan'] = plan
    q, nca = plan['Q'], plan['NCA']
    x = np.asarray(x, np.float32)

    in_maps = []
    for c in range(NCORES):
        r, cl = cores[c]
        colp_arr = np.zeros((nca, 128), np.int64)      # global padded col id
        rloc = np.full((nca, 128), -1, np.int64)
        valid = np.zeros((nca, 128), bool)
        w = r // 128
        for ww in range(NW):
            msel = (w == ww)
            rw, cw = r[msel], cl[msel]
            cnt = rw.shape[0]
            assert cnt <= q * 128, f"window overflow {cnt} > {q * 128}"
            base = ww * q
            for qq in range((cnt + 127) // 128):
                a, b = qq * 128, min(qq * 128 + 128, cnt)
                k = base + qq
                colp_arr[k, :b - a] = cw[a:b]
                rloc[k, :b - a] = rw[a:b] - ww * 128
                valid[k, :b - a] = True

        # contrib gather: 2-node XC2 rows keyed by col>>1 (pair 2k,2k+1);
        # feature gather: 2-node XF2 rows keyed by (core, win, p%64)
        # (pair p, p+64 within a window - DMA-expressible write pattern)
        ncalls = nca // CCH
        iC = (colp_arr >> 1).reshape(ncalls, CALL)
        idxc_w = _wrap_calls(iC)
        core_of = colp_arr // SH
        lr_of = colp_arr % SH
        w_of = lr_of // 128
        p_of = lr_of % 128
        iF = core_of * (SH // 2) + w_of * 64 + (p_of % 64)
        iF[~valid] = 0
        idxf_w = _wrap_calls(iF.reshape(ncalls, CALL))
        import ml_dtypes as _mldt
        mcolT = (colp_arr & 1).T.astype(np.float32).copy()   # [128, NCA]
        mcolb = (p_of >= 64).T.astype(_mldt.bfloat16)        # feature parity
        mcolbi = (p_of < 64).T.astype(_mldt.bfloat16)        # 1 - parity
        rloc_b = rloc.T.astype(_mldt.bfloat16).copy()         # [128, NCA]
        # transposed one-hots for on-chip row-contrib expansion:
        # selnT[n, k*128+j] = 1 iff rloc[k, j] == n  (layer-independent)
        selnT = np.equal.outer(
            np.arange(128, dtype=np.int64), rloc).astype(
                _mldt.bfloat16).reshape(128, nca * 128)
        in_maps.append({
            "idxc_w": idxc_w, "idxf_w": idxf_w, "mcolT": mcolT,
            "mcolb": mcolb, "mcolbi": mcolbi, "rloc_b": rloc_b,
            "selnT": selnT,
        })

    import ml_dtypes
    W1 = np.asarray(W1, np.float32); b1 = np.asarray(b1, np.float32)
    W2 = np.asarray(W2, np.float32); b2 = np.asarray(b2, np.float32)
    NL = N_LAYERS
    QC = 4 + F   # contrib (4) + q = (Wl (x) Wr) x (F) per layer
    wqm = np.zeros((F, NL * QC), np.float32)
    cfb = np.zeros((128, NL * F), np.float32)
    for l in range(NL):
        sh_row = np.asarray(W_sheaf[l][1], np.float32)
        wt_row = np.asarray(W_wt[l][0], np.float32)
        wqm[:, l * QC + 0] = sh_row[:F]
        wqm[:, l * QC + 1] = sh_row[F:]
        wqm[:, l * QC + 2] = wt_row[:F]
        wqm[:, l * QC + 3] = wt_row[F:]
        Wl = _spectral_normalize_np(np.asarray(W_left[l], np.float32))
        Wr = _spectral_normalize_np(np.asarray(W_right[l], np.float32))
        wqm[:, l * QC + 4:(l + 1) * QC] = \
            np.kron(Wl, Wr).astype(np.float32).T
        cfl = 1.0 + np.tanh(np.asarray(eps[l], np.float32)).reshape(FD)
        cfb[:, l * F:(l + 1) * F] = np.repeat(cfl, HID)[None, :]

    xp = np.zeros((NPAD, IN_CH), np.float32)
    xp[pad_id] = x
    iota_b = np.tile(np.arange(128, dtype=np.float32)[None, :],
                     (128, 1)).astype(ml_dtypes.bfloat16)
    shared = {
        "w1t": W1.T.copy(), "b1f": b1.reshape(F, 1).copy(),
        "w2t": W2.T.copy(), "b2": b2.reshape(OUT_CH, 1).copy(),
        "wqm": wqm, "cfb": cfb,
        "iota_b": iota_b, "ident": np.eye(128, dtype=np.float32),
    }
    for c in range(NCORES):
        in_maps[c]["x_sh"] = xp[c * SH:(c + 1) * SH].T.copy()
        in_maps[c].update(shared)
    return in_maps, pad_id


# =================== bass program ===================
def _build_program(plan):
    import concourse.bacc as bacc
    import concourse.bass as bass
    import concourse.mybir as mybir
    from concourse import tile

    nca = plan['NCA']
    winchunks = plan['winchunks']
    ncalls = nca // CCH
    NL = N_LAYERS
    f32 = mybir.dt.float32
    bf16 = mybir.dt.bfloat16
    i16 = mybir.dt.int16
    AF = mybir.ActivationFunctionType
    ALU = mybir.AluOpType

    nc = bacc.Bacc("TRN2", target_bir_lowering=False, debug=False,
                   num_swdge_queues=4)

    def raw_gather(gps, out_ap, in_ap, idxs_ap, num_idxs, elem_size,
                   elem_step, queue_num=0):
        """dma_gather allowing elem_size < 256B (row stride must be %256B)."""
        stride_bytes = elem_step * mybir.dt.size(in_ap.dtype)
        assert stride_bytes % 256 == 0
        assert in_ap.ap[0][0] == elem_step
        assert in_ap.ap[-1][1] == out_ap.ap[-1][1] == elem_size
        _in_ap = gps.lower_ap_dma(in_ap, for_custom_bir_dma=True)
        _idxs_ap = gps.lower_ap(idxs_ap)
        _out_ap = gps.lower_ap(out_ap)
        return gps.add_instruction(
            mybir.InstDMAGatherAnt(
                name=gps.bass.get_next_instruction_name(),
                ins=[*_in_ap, _idxs_ap,
                     gps.lower_val_access(gps.to_reg(num_idxs))],
                outs=[_out_ap],
                transpose=False,
                num_idxs=num_idxs,
                elem_size=elem_size,
                stride_bytes_256=stride_bytes // 256,
                gen_mode=0,
                single_packet=True,
                queue_num=queue_num,
                sbuf_tokens_per_rank=0,
                sbuf_free_dim_per_rank=0,
                sbuf_free_dim_pad_per_rank=0,
                sbuf_byte_offset=0,
            ))


    x_sh = nc.dram_tensor("x_sh", [IN_CH, SH], f32, kind="ExternalInput").ap()
    idxc_d = nc.dram_tensor("idxc_w", [128, ncalls * IW], i16, kind="ExternalInput").ap()
    idxf_d = nc.dram_tensor("idxf_w", [128, ncalls * IW], i16, kind="ExternalInput").ap()
    selnT_d = nc.dram_tensor("selnT", [128, nca * 128], bf16, kind="ExternalInput").ap()
    mcol_d = nc.dram_tensor("mcolT", [128, nca], f32, kind="ExternalInput").ap()
    mcolb_d = nc.dram_tensor("mcolb", [128, nca], bf16, kind="ExternalInput").ap()
    mcolbi_d = nc.dram_tensor("mcolbi", [128, nca], bf16, kind="ExternalInput").ap()
    rloc_d = nc.dram_tensor("rloc_b", [128, nca], bf16, kind="ExternalInput").ap()
    iota_d = nc.dram_tensor("iota_b", [128, 128], bf16, kind="ExternalInput").ap()
    w1t_d = nc.dram_tensor("w1t", [IN_CH, F], f32, kind="ExternalInput").ap()
    b1f_d = nc.dram_tensor("b1f", [F, 1], f32, kind="ExternalInput").ap()
    w2t_d = nc.dram_tensor("w2t", [F, OUT_CH], f32, kind="ExternalInput").ap()
    b2_d = nc.dram_tensor("b2", [OUT_CH, 1], f32, kind="ExternalInput").ap()
    QC = 4 + F
    wqm_d = nc.dram_tensor("wqm", [F, NL * QC], f32, kind="ExternalInput").ap()
    cfb_d = nc.dram_tensor("cfb", [128, NL * F], f32, kind="ExternalInput").ap()
    ident_d = nc.dram_tensor("ident", [128, 128], f32, kind="ExternalInput").ap()
    out_d = nc.dram_tensor("out", [SH, OUT_CH], f32, kind="ExternalOutput").ap()

    xcs = nc.dram_tensor("xcs", [SH, 4], f32)
    XCFULL = nc.dram_tensor("XCFULL", [NPAD, 4], f32, addr_space="Shared")
    XC2 = nc.dram_tensor("XC2", [NPAD // 2, 64], f32)
    xfs2 = nc.dram_tensor("xfs2", [SH // 2, 256], bf16)
    XF2 = nc.dram_tensor("XF2", [NPAD // 2, 256], bf16, addr_space="Shared")
    RG = [list(range(NCORES))]

    with tile.TileContext(nc) as tc:
        with tc.tile_pool(name="const", bufs=1) as constp, \
             tc.tile_pool(name="big", bufs=1) as bigp, \
             tc.tile_pool(name="wide", bufs=1) as widep, \
             tc.tile_pool(name="gath", bufs=2) as gathp, \
             tc.tile_pool(name="fea", bufs=3) as feap, \
             tc.tile_pool(name="selp", bufs=3) as selp, \
             tc.tile_pool(name="seln", bufs=2) as selnp, \
             tc.tile_pool(name="work", bufs=3) as workp, \
             tc.tile_pool(name="msgp", bufs=3) as msgp, \
             tc.tile_pool(name="ps", bufs=2, space="PSUM") as psp, \
             tc.tile_pool(name="ps2", bufs=2, space="PSUM") as ps2p, \
             tc.tile_pool(name="ps1", bufs=4, space="PSUM") as ps1p:

            def C(name, shape, src, dt=f32):
                t = constp.tile(shape, dt, tag=name, name=name)
                nc.sync.dma_start(t[:], src)
                return t

            ident = C("ident", [128, 128], ident_d[:])
            iota_b = C("iota", [128, 128], iota_d[:], dt=bf16)
            w1t = C("w1t", [IN_CH, F], w1t_d[:])
            b1f = C("b1f", [F, 1], b1f_d[:])
            w2t = C("w2t", [F, OUT_CH], w2t_d[:])
            b2sb = C("b2", [OUT_CH, 1], b2_d[:])
            wqm = C("wqm", [F, NL * QC], wqm_d[:])
            cfb = C("cfb", [128, NL * F], cfb_d[:])
            mcol = C("mcol", [128, nca], mcol_d[:])
            mcolb = C("mcolb", [128, nca], mcolb_d[:], dt=bf16)
            mcolbi = C("mcolbi", [128, nca], mcolbi_d[:], dt=bf16)
            rloc_b = C("rloc", [128, nca], rloc_d[:], dt=bf16)
            idxc_sb = C("idxc", [128, ncalls * IW], idxc_d[:], dt=i16)
            idxf_sb = C("idxf", [128, ncalls * IW], idxf_d[:], dt=i16)

            slabT = bigp.tile([128, NW, F], f32, tag="slabT")
            ctile = bigp.tile([128, NW, 4], f32, tag="ctile")
            ctb = bigp.tile([128, NW, 4], bf16, tag="ctb")
            xfN = bigp.tile([128, NW, F], bf16, tag="xfN")
            z1a = bigp.tile([128, NW, F], f32, tag="z1a")
            dinv_sh = bigp.tile([128, NW], f32, tag="dinvsh")
            diag_sh = bigp.tile([128, NW], f32, tag="diagsh")
            ccall = bigp.tile([128, nca, 4], f32, tag="ccall")
            rcall = bigp.tile([128, nca, 4], f32, tag="rcall")
            c2b = bigp.tile([128, nca], bf16, tag="c2b")
            s2b = bigp.tile([128, nca], bf16, tag="s2b")
            w2b = bigp.tile([128, nca], bf16, tag="w2b")

            def tpose(src_ap, pdim, fdim, tag="tx"):
                pt = ps1p.tile([128, 128], f32, tag="tp", name="tp")
                nc.tensor.transpose(pt[:fdim, :pdim], src_ap,
                                    ident[:pdim, :pdim])
                dst = workp.tile([128, 128], f32, tag=tag, name=tag)
                nc.scalar.copy(dst[:fdim, :pdim], pt[:fdim, :pdim])
                return dst

            def contrib_write(L):
                """ctile -> xcs -> AllGather -> XCFULL -> expand into XC2."""
                nc.sync.dma_start(
                    xcs[:].rearrange("(w p) f -> p w f", p=128), ctile[:, :, :])
                nc.gpsimd.collective_compute(
                    "AllGather", ALU.bypass, replica_groups=RG,
                    ins=[xcs[:]], outs=[XCFULL[:]])
                ct2 = widep.tile([128, NPAD // 256, 8], f32, tag="ct2",
                                 name="ct2")
                nc.sync.dma_start(
                    ct2[:, :, :],
                    XCFULL[:].rearrange("(c p two) f -> p c (two f)",
                                        p=128, two=2))
                nc.sync.dma_start(
                    XC2[:, 0:8].rearrange("(c p) f8 -> p c f8", p=128),
                    ct2[:, :, :])

            # ---------------- lin1 + layer-0 contribs ----------------
            for t in range(NW):
                xT = workp.tile([128, 128], f32, tag="xt")
                nc.sync.dma_start(xT[:], x_sh[:, t * 128:(t + 1) * 128])
                hp = psp.tile([128, 128], f32, tag="mm")
                nc.tensor.matmul(hp[:F, :128], w1t[:], xT[:IN_CH, :128],
                                 start=True, stop=True)
                e1 = workp.tile([F, 128], f32, tag="e1")
                nc.scalar.activation(e1[:, :], hp[:F, :128], AF.Exp,
                                     bias=b1f[:, :])
                nc.vector.tensor_scalar(e1[:, :], e1[:, :], 1.0, -1.0,
                                        ALU.min, ALU.add)
                r1 = workp.tile([F, 128], f32, tag="r1")
                nc.scalar.activation(r1[:, :], hp[:F, :128], AF.Relu,
                                     bias=b1f[:, :])
                hF = workp.tile([F, 128], f32, tag="hF")
                nc.vector.tensor_add(hF[:, :], e1[:, :], r1[:, :])
                hN = ps1p.tile([128, 128], f32, tag="tp")
                nc.tensor.transpose(hN[:128, :F], hF[:, :128], ident[:F, :F])
                nc.vector.tensor_copy(slabT[:, t, 0:F], hN[:128, :F])
                # contribs + q = (Wl (x) Wr) x in one node-major matmul
                # (stationary = hF, contraction over F)
                qcp = ps2p.tile([128, 128], f32, tag="m2")
                nc.tensor.matmul(qcp[:, 0:QC], hF[:, :128], wqm[:, 0:QC],
                                 start=True, stop=True)
                nc.vector.tensor_copy(ctile[:, t, :], qcp[:, 0:4])
                nc.vector.tensor_copy(ctb[:, t, :], qcp[:, 0:4])
                nc.vector.tensor_copy(z1a[:, t, 0:F], qcp[:, 4:QC])
            contrib_write(0)

            # =================== layers ===================
            for L in range(NL):
                # ---- phase 1: contribs ----
                # row side: expand per-window ctile to edge slots via
                # streamed transposed one-hots (selnT) on the PE array —
                # depends only on local ctile, so it runs under the XC
                # AllGather + expand; col side gathers wait on XC2.
                GB = 4  # calls per select batch (32 chunks)

                def psel(gt, mt, dst, ks, nk):
                    d = msgp.tile([128, GB * CCH, 4], f32, tag="d4", name="d4")
                    nc.vector.tensor_sub(d[:, :nk, :], gt[:, :nk, 4:8],
                                         gt[:, :nk, 0:4])
                    nc.vector.tensor_mul(
                        d[:, :nk, :], d[:, :nk, :],
                        mt[:, ks].unsqueeze(2).broadcast_to([128, nk, 4]))
                    nc.vector.tensor_add(dst[:, ks, :], gt[:, :nk, 0:4],
                                         d[:, :nk, :])

                for g in range(ncalls):
                    selnw = selnp.tile([128, CALL], bf16, tag="selnw",
                                       name="selnw")
                    nc.sync.dma_start(selnw[:, :],
                                      selnT_d[:, g * CALL:(g + 1) * CALL])
                    rcp = ps1p.tile([128, 128], f32, tag="tp", name="rcp")
                    for j in range(CCH):
                        k = g * CCH + j
                        w = winchunks[k][2]
                        nc.tensor.matmul(rcp[:, j * 4:(j + 1) * 4],
                                         selnw[:, j * 128:(j + 1) * 128],
                                         ctb[:, w, :], start=True, stop=True)
                    nc.scalar.copy(rcall[:, g * CCH:(g + 1) * CCH, :],
                                   rcp[:, 0:4 * CCH])
                # ---- learner algebra + deg, pipelined per ccall batch ----
                def wt(tag):
                    return widep.tile([128, nca], f32, tag=tag, name=tag)
                rc, cc = rcall, ccall
                ta, tb = wt("ta"), wt("tb")
                af, ab = wt("af"), wt("ab")
                u1, u2 = wt("u1"), wt("u2")
                w2e, t1, t2 = wt("w2e"), wt("t1"), wt("t2")
                A2, R2 = wt("A2"), wt("R2")
                de, dr = wt("de"), wt("dr")
                ce, se, cr, sr = ta, tb, u1, u2      # buffer reuse
                c_e, s_e = A2, R2
                degt = psp.tile([128, 128], f32, tag="mm", name="deg")
                degP = degt[:, 0:NW]

                def learner_deg_slice(k0s, k1s):
                    ks = slice(k0s, k1s)
                    nc.vector.tensor_add(ta[:, ks], rc[:, ks, 0], cc[:, ks, 1])
                    nc.vector.tensor_add(tb[:, ks], cc[:, ks, 0], rc[:, ks, 1])
                    nc.scalar.activation(af[:, ks], ta[:, ks], AF.Tanh)
                    nc.scalar.activation(ab[:, ks], tb[:, ks], AF.Tanh)
                    nc.vector.tensor_add(ta[:, ks], rc[:, ks, 2], cc[:, ks, 3])
                    nc.vector.tensor_add(tb[:, ks], cc[:, ks, 2], rc[:, ks, 3])
                    nc.scalar.activation(u1[:, ks], ta[:, ks], AF.Tanh,
                                         scale=0.5)
                    nc.scalar.activation(u2[:, ks], tb[:, ks], AF.Tanh,
                                         scale=0.5)
                    nc.vector.tensor_mul(t1[:, ks], u1[:, ks], u2[:, ks])
                    nc.vector.tensor_add(t2[:, ks], u1[:, ks], u2[:, ks])
                    nc.vector.tensor_add(t1[:, ks], t1[:, ks], t2[:, ks])
                    nc.vector.tensor_scalar(w2e[:, ks], t1[:, ks], 0.25, 0.25,
                                            ALU.mult, ALU.add)
                    nc.vector.tensor_mul(w2e[:, ks], w2e[:, ks], w2e[:, ks])
                    nc.vector.tensor_mul(A2[:, ks], af[:, ks], af[:, ks])
                    nc.vector.tensor_mul(R2[:, ks], ab[:, ks], ab[:, ks])
                    nc.vector.tensor_scalar(de[:, ks], A2[:, ks], 1.0, None,
                                            ALU.add)
                    nc.vector.reciprocal(de[:, ks], de[:, ks])
                    nc.vector.tensor_scalar(dr[:, ks], R2[:, ks], 1.0, None,
                                            ALU.add)
                    nc.vector.reciprocal(dr[:, ks], dr[:, ks])
                    nc.vector.tensor_scalar(t1[:, ks], A2[:, ks], -1.0, 1.0,
                                            ALU.mult, ALU.add)
                    nc.vector.tensor_mul(ce[:, ks], t1[:, ks], de[:, ks])
                    nc.vector.tensor_scalar(t1[:, ks], af[:, ks], 2.0, None,
                                            ALU.mult)
                    nc.vector.tensor_mul(se[:, ks], t1[:, ks], de[:, ks])
                    nc.vector.tensor_scalar(t1[:, ks], R2[:, ks], -1.0, 1.0,
                                            ALU.mult, ALU.add)
                    nc.vector.tensor_mul(cr[:, ks], t1[:, ks], dr[:, ks])
                    nc.vector.tensor_scalar(t1[:, ks], ab[:, ks], 2.0, None,
                                            ALU.mult)
                    nc.vector.tensor_mul(sr[:, ks], t1[:, ks], dr[:, ks])
                    nc.vector.tensor_mul(t1[:, ks], ce[:, ks], cr[:, ks])
                    nc.vector.tensor_mul(t2[:, ks], se[:, ks], sr[:, ks])
                    nc.vector.tensor_add(c_e[:, ks], t1[:, ks], t2[:, ks])
                    nc.vector.tensor_mul(t1[:, ks], sr[:, ks], ce[:, ks])
                    nc.vector.tensor_mul(t2[:, ks], se[:, ks], cr[:, ks])
                    nc.vector.tensor_sub(s_e[:, ks], t1[:, ks], t2[:, ks])
                    # rotation coefs in bf16 (w2 folded in)
                    nc.vector.tensor_mul(c2b[:, ks], c_e[:, ks], w2e[:, ks])
                    nc.vector.tensor_mul(s2b[:, ks], s_e[:, ks], w2e[:, ks])
                    nc.vector.tensor_copy(w2b[:, ks], w2e[:, ks])
                    for gg in range(k0s, k1s, CCH):
                        selw = selp.tile([128, CCH, 128], bf16, tag="selw",
                                         name="selw")
                        nc.vector.tensor_tensor(
                            selw[:, :, :],
                            iota_b[:, :].unsqueeze(1).broadcast_to(
                                [128, CCH, 128]),
                            rloc_b[:, gg:gg + CCH].unsqueeze(2).broadcast_to(
                                [128, CCH, 128]),
                            ALU.is_equal)
                        for j in range(CCH):
                            (k, reg, w, st, sp) = winchunks[gg + j]
                            nc.tensor.matmul(degP[:, w:w + 1], selw[:, j, :],
                                             w2b[:, k:k + 1],
                                             start=st, stop=sp)

                for g0 in range(0, ncalls, GB):
                    gb = min(GB, ncalls - g0)
                    gt = gathp.tile([128, GB * CCH, 8], f32, tag="gc",
                                    name="gc")
                    for j in range(gb):
                        g = g0 + j
                        sl = slice(g * IW, (g + 1) * IW)
                        raw_gather(
                            nc.gpsimd, gt[:, j * CCH:(j + 1) * CCH, :],
                            XC2[:, 0:8], idxc_sb[:, sl], CALL, 8, 64,
                            queue_num=g % 4)
                    psel(gt, mcol, ccall,
                         slice(g0 * CCH, (g0 + gb) * CCH), gb * CCH)
                    learner_deg_slice(g0 * CCH, (g0 + gb) * CCH)

                deg = wt("ta")
                nc.vector.tensor_copy(deg[:, 0:NW], degP)
                nc.vector.tensor_scalar(diag_sh[:, :], deg[:, 0:NW], 1e30, 1.0,
                                        ALU.mult, ALU.min)
                nc.vector.tensor_scalar(deg[:, 0:NW], deg[:, 0:NW], 1e-30,
                                        None, ALU.max)
                rrec = wt("tb")
                nc.vector.reciprocal(rrec[:, 0:NW], deg[:, 0:NW])
                nc.scalar.activation(dinv_sh[:, :], rrec[:, 0:NW], AF.Sqrt)
                ny = wt("u1")
                nc.vector.tensor_mul(ny[:, 0:NW], dinv_sh[:, :], dinv_sh[:, :])
                nc.vector.tensor_mul(ny[:, 0:NW], ny[:, 0:NW], deg[:, 0:NW])
                nc.vector.tensor_scalar(ny[:, 0:NW], ny[:, 0:NW], -0.5, 1.5,
                                        ALU.mult, ALU.add)
                nc.vector.tensor_mul(dinv_sh[:, :], dinv_sh[:, :], ny[:, 0:NW])
                nc.vector.tensor_mul(dinv_sh[:, :], dinv_sh[:, :],
                                     diag_sh[:, :])

                # ---- feature table: dinv * (Wl (x) Wr) xc (q precomputed in
                # z1a; Wr folded in - the stalk rotation commutes with it),
                # written 2-node-packed into xfs2 rows (node pair per 512B) --
                for t in range(NW):
                    nc.scalar.activation(xfN[:, t, 0:F], z1a[:, t, 0:F],
                                         AF.Identity,
                                         scale=dinv_sh[:, t:t + 1])
                xf2lo = xfs2[:, 0:F].rearrange("(w p2) f -> p2 w f", p2=64)
                xf2hi = xfs2[:, F:2 * F].rearrange("(w p2) f -> p2 w f",
                                                   p2=64)
                nc.sync.dma_start(xf2lo[:, :, :], xfN[0:64, :, :])
                nc.sync.dma_start(xf2hi[:, :, :], xfN[64:128, :, :])

                nc.gpsimd.collective_compute(
                    "AllGather", ALU.bypass, replica_groups=RG,
                    ins=[xfs2[:]], outs=[XF2[:]])

                # ---- phase 4: x-update per window (fused into messages) ----
                def phase4_win(t, aggP):
                    z1 = workp.tile([128, F], f32, tag="z1")
                    z2 = workp.tile([128, F], f32, tag="z2")
                    nc.scalar.activation(z2[:, :], aggP, AF.Identity,
                                         scale=dinv_sh[:, t:t + 1])
                    z1s = workp.tile([128, F], f32, tag="z1s")
                    nc.scalar.activation(z1s[:, :], z1a[:, t, 0:F],
                                         AF.Identity,
                                         scale=diag_sh[:, t:t + 1])
                    nc.vector.tensor_sub(z1[:, :], z1s[:, :], z2[:, :])
                    ez = workp.tile([128, F], f32, tag="ez")
                    nc.scalar.activation(ez[:, :], z1[:, :], AF.Exp)
                    nc.vector.tensor_scalar(ez[:, :], ez[:, :], 1.0, -1.0,
                                            ALU.min, ALU.add)
                    rz = workp.tile([128, F], f32, tag="rz")
                    nc.scalar.activation(rz[:, :], z1[:, :], AF.Relu)
                    nc.vector.tensor_add(ez[:, :], ez[:, :], rz[:, :])
                    nc.vector.tensor_mul(slabT[:, t, 0:F], slabT[:, t, 0:F],
                                         cfb[:, L * F:(L + 1) * F])
                    nc.vector.tensor_sub(slabT[:, t, 0:F], slabT[:, t, 0:F],
                                         ez[:, :])
                    if L + 1 < NL:
                        xpT = tpose(slabT[:, t, 0:F], 128, F)
                        qcp = ps2p.tile([128, 128], f32, tag="m2")
                        nc.tensor.matmul(qcp[:, 0:QC], xpT[:F, :128],
                                         wqm[:, (L + 1) * QC:(L + 2) * QC],
                                         start=True, stop=True)
                        nc.vector.tensor_copy(ctile[:, t, :], qcp[:, 0:4])
                        nc.vector.tensor_copy(ctb[:, t, :], qcp[:, 0:4])
                        nc.vector.tensor_copy(z1a[:, t, 0:F], qcp[:, 4:QC])
                def lin2_win(t):
                    xT = tpose(slabT[:, t, 0:F], 128, F)
                    op = ps1p.tile([128, 128], f32, tag="tp")
                    nc.tensor.matmul(op[:OUT_CH, :128], w2t[:, :],
                                     xT[:F, :128], start=True, stop=True)
                    ob = workp.tile([OUT_CH, 128], f32, tag="l2ob")
                    nc.scalar.activation(ob[:, :], op[:OUT_CH, :128],
                                         AF.Identity, bias=b2sb[:, :])
                    oN = ps1p.tile([128, 128], f32, tag="tp")
                    nc.tensor.transpose(oN[:128, :OUT_CH], ob[:, :128],
                                        ident[:OUT_CH, :OUT_CH])
                    os_ = workp.tile([128, OUT_CH], f32, tag="l2os")
                    nc.vector.tensor_copy(os_[:, :], oN[:128, :OUT_CH])
                    nc.sync.dma_start(out_d[t * 128:(t + 1) * 128, :],
                                      os_[:, :])


                # ---- messages: gather + parity-select + rotate + aggregate.
                # Gathers stay 1024-idx (SWDGE ring limit); the DVE work is
                # batched over PAIRS of calls to halve per-op overheads. ----
                aggP = None
                cur = None
                MB = 2  # calls per DVE batch
                for cp0 in range(0, ncalls, MB):
                    npair = min(MB, ncalls - cp0)
                    nch = npair * CCH
                    k0 = cp0 * CCH
                    gf2 = feap.tile([128, MB * CCH, 2 * F], bf16, tag="gf",
                                    name="gf")
                    for u in range(npair):
                        ci = cp0 + u
                        raw_gather(
                            nc.gpsimd, gf2[:, u * CCH:(u + 1) * CCH, :],
                            XF2[:, 0:2 * F],
                            idxf_sb[:, ci * IW:(ci + 1) * IW], CALL, 2 * F,
                            256, queue_num=ci % 4)
                    ksl = slice(k0, k0 + nch)
                    # exact bf16 parity select: lo*(1-m) + hi*m (m in {0,1}
                    # so every product and the sum are exact)
                    gf = feap.tile([128, MB * CCH, F], bf16, tag="gfs",
                                   name="gfs")
                    gft = feap.tile([128, MB * CCH, F], bf16, tag="gft",
                                    name="gft")
                    nc.vector.tensor_mul(
                        gf[:, :nch, :], gf2[:, :nch, 0:F],
                        mcolbi[:, ksl].unsqueeze(2).broadcast_to(
                            [128, nch, F]))
                    nc.vector.tensor_mul(
                        gft[:, :nch, :], gf2[:, :nch, F:2 * F],
                        mcolb[:, ksl].unsqueeze(2).broadcast_to(
                            [128, nch, F]))
                    nc.vector.tensor_add(gf[:, :nch, :], gf[:, :nch, :],
                                         gft[:, :nch, :])
                    msg = msgp.tile([128, MB * CCH, F], bf16, tag="msg",
                                    name="msg")
                    c2r = c2b[:, ksl].unsqueeze(2).broadcast_to(
                        [128, nch, HID])
                    s2r = s2b[:, ksl].unsqueeze(2).broadcast_to(
                        [128, nch, HID])
                    w2r = w2b[:, ksl].unsqueeze(2).broadcast_to(
                        [128, nch, HID])
                    g0b = gf[:, :nch, 0:HID]
                    g1b = gf[:, :nch, HID:2 * HID]
                    g2b = gf[:, :nch, 2 * HID:3 * HID]
                    tA = msgp.tile([128, MB * CCH, HID], bf16, tag="tA",
                                   name="tA")
                    tB = msgp.tile([128, MB * CCH, HID], bf16, tag="tB",
                                   name="tB")
                    nc.vector.tensor_mul(tA[:, :nch, :], g0b, c2r)
                    nc.vector.tensor_mul(tB[:, :nch, :], g1b, s2r)
                    nc.vector.tensor_sub(msg[:, :nch, 0:HID], tA[:, :nch, :],
                                         tB[:, :nch, :])
                    nc.vector.tensor_mul(tA[:, :nch, :], g0b, s2r)
                    nc.vector.tensor_mul(tB[:, :nch, :], g1b, c2r)
                    nc.vector.tensor_add(msg[:, :nch, HID:2 * HID],
                                         tA[:, :nch, :], tB[:, :nch, :])
                    nc.vector.tensor_mul(msg[:, :nch, 2 * HID:3 * HID],
                                         g2b, w2r)
                    selw = selp.tile([128, MB * CCH, 128], bf16, tag="selw",
                                     name="selw")
                    nc.vector.tensor_tensor(
                        selw[:, :nch, :],
                        iota_b[:, :].unsqueeze(1).broadcast_to(
                            [128, nch, 128]),
                        rloc_b[:, ksl].unsqueeze(2).broadcast_to(
                            [128, nch, 128]),
                        ALU.is_equal)
                    for j in range(nch):
                        (k, reg, w, st, sp) = winchunks[k0 + j]
                        if st:
                            aggPt = psp.tile([128, 128], f32, tag="mm")
                            aggP = aggPt[:, 0:F]
                            cur = (reg, w)
                        assert cur == (reg, w)
                        nc.tensor.matmul(aggP, selw[:, j, :], msg[:, j, :],
                                         start=st, stop=sp)
                        if sp:
                            phase4_win(w, aggP)
                            if L + 1 == NL:
                                lin2_win(w)

                if L + 1 < NL:
                    contrib_write(L + 1)

    nc.compile()
    return nc


def kernel(x, edge_index, W1, b1, W2, b2, W_left, W_right, eps,
           W_sheaf, W_wt):
    from concourse.bass_utils import run_bass_kernel_spmd
    in_maps, pad_id = _host_prep(x, edge_index, W1, b1, W2, b2, W_left,
                                 W_right, eps, W_sheaf, W_wt)
    plan = _CACHE['plan']
    key = (plan['Q'],)
    if _CACHE.get('key') != key:
        _CACHE['nc'] = _build_program(plan)
        _CACHE['key'] = key
    nc = _CACHE['nc']
    res = run_bass_kernel_spmd(nc, in_maps, list(range(NCORES)))
    full = np.concatenate([res.results[c]["out"] for c in range(NCORES)],
                          axis=0)
    return full[pad_id].astype(np.float32)

